# revision 2
# baseline (speedup 1.0000x reference)
"""GAT (gnn_message_passing) Trainium2 Bass kernel — 8-core SPMD, v2.

Contract: kernel(**inputs) -> np.ndarray with FULL inputs / FULL output.
Self-contained: hardcodes shapes; only imports the container's concourse stack.

v2 design vs v1:
  - bf16 edge path: shared table rows are 256x bf16 (512B), scatter matmuls,
    indicator and row-scaling all bf16.
  - No Lrelu on the Activation engine (no act-table reloads): attention
    leaky-relu is max(t, 0.2t) on DVE; MLP leaky-relu is
    relu(x+b) - relu(-a*x - a*b) via two Relu activations + one DVE subtract.
  - Stage A consumes host-transposed x (no PE transposes) and emits node-major
    rows [h | a_s | a_d] with one matmul against an augmented [Wg|Wg@as|Wg@ad].
  - KDW=1: each core writes its row slice straight into the shared DRAM table
    at a partition_id()-based dynamic offset, then a tiny AllGather acts as a
    barrier. KDW=0 falls back to two real bf16 AllGathers.
"""
import sys

for _p in ("/opt/trn_rl_repo", "/root/.axon_site/_ro/trn_rl_repo"):
    if _p not in sys.path:
        sys.path.append(_p)

import os
import numpy as np
import ml_dtypes

BF16 = ml_dtypes.bfloat16
# KDW modes: 0 = two half-table AllGathers (quarter row scheme),
#            1 = direct shared write + barrier (broken: scratchpad is only
#                pair-shared, kept for reference),
#            2 = ONE fat AllGather of the full 512B-pitch table, plain order
_KDW = int(os.environ.get("KDW", "2"))

# ---------------- problem constants (hardcoded per contract) ----------------
N = 50000
NF = 513
NFP = 640            # padded feature dim (5 * 128)
NMEL = 128
H, C = 4, 32
HC = H * C           # 128
E = 800000
NEG_ATT = 0.2
NEG_MLP = 0.01

NCORES = 8
TPC = 49             # tiles per core
NT = 128             # nodes per tile
NPC = TPC * NT       # 6272 nodes per core
NPAD = NCORES * NPC  # 50176
RDX = 256            # table row pitch in bf16 elems (512 B)
SPLIT = 4 * NPC      # 25088: table A/B row split (int16 idx headroom)
QSR = (0, 3072, NPC)  # KDW=0 quarter split (rows per AllGather region)

_CACHE = {}


def _prep(edge_index):
    """Host-side edge preprocessing. Returns per-core index/metadata arrays."""
    src = np.asarray(edge_index[0], dtype=np.int64)
    dst = np.asarray(edge_index[1], dtype=np.int64)
    loop = np.arange(N, dtype=np.int64)
    src = np.concatenate([src, loop])
    dst = np.concatenate([dst, loop])

    tile_g = dst // NT                # global tile id 0..391
    if _KDW >= 1:
        half = (src >= SPLIT).astype(np.int64)
        src_row = src - half * SPLIT
    else:
        r, l = src // NPC, src % NPC
        half = (l >= QSR[1]).astype(np.int64)
        src_row = np.where(half == 1,
                           r * (NPC - QSR[1]) + (l - QSR[1]),
                           r * QSR[1] + l)
    order = np.lexsort((src, dst, half, tile_g))
    src_row, dst, tile_g, half = (src_row[order], dst[order], tile_g[order],
                                  half[order])

    NTILES_G = NPAD // NT            # 392
    cnt = np.zeros((NTILES_G, 2), dtype=np.int64)
    np.add.at(cnt, (tile_g, half), 1)
    starts = np.zeros((NTILES_G, 2), dtype=np.int64)
    starts.reshape(-1)[1:] = np.cumsum(cnt.reshape(-1))[:-1]

    # chunks per (slot, half): max over cores
    cores = np.arange(NCORES)
    cpt = np.zeros((TPC, 2), dtype=np.int64)
    for s in range(TPC):
        t_ids = cores * TPC + s
        for hf in range(2):
            cpt[s, hf] = max(1, int(np.ceil(cnt[t_ids, hf].max() / NT)))
    TOTC = int(cpt.sum())
    TOTIDX = TOTC * NT

    src_rel = np.zeros((NCORES, TOTC, NT), dtype=np.int64)
    ad_idx = np.zeros((NCORES, TOTC, NT), dtype=np.int64)
    dst_rel = np.full((NCORES, TOTC, NT), 999.0, dtype=np.float32)
    dloc_all = np.zeros((NCORES, TOTC, NT), dtype=np.int64)
    valid = np.zeros((NCORES, TOTC, NT), dtype=bool)

    for k in range(NCORES):
        coff = 0
        for s in range(TPC):
            t = k * TPC + s
            for hf in range(2):
                nch = int(cpt[s, hf])
                st, cn = starts[t, hf], int(cnt[t, hf])
                src_rel[k, coff:coff + nch].reshape(-1)[:cn] = src_row[st:st + cn]
                ad_idx[k, coff:coff + nch].reshape(-1)[:cn] = dst[st:st + cn] % NPC
                dloc_all[k, coff:coff + nch].reshape(-1)[:cn] = dst[st:st + cn] % NT
                valid[k, coff:coff + nch].reshape(-1)[:cn] = True
                coff += nch
        assert coff == TOTC

    assert src_rel.min() >= 0 and src_rel.max() <= 32767

    # per-chunk dst windows: 64-wide when the cross-core span fits, else 128
    woff = np.zeros(TOTC, dtype=np.int64)
    wlen = np.full(TOTC, 128, dtype=np.int64)
    for c in range(TOTC):
        v = valid[:, c, :]
        if v.any():
            dl = dloc_all[:, c, :][v]
            lo, hi = int(dl.min()), int(dl.max())
            wo = 0 if lo < 64 else 64
            if hi < wo + 64:
                woff[c] = wo
                wlen[c] = 64

    for k in range(NCORES):
        dr = dloc_all[k] - woff[:, None]
        dst_rel[k][valid[k]] = dr[valid[k]].astype(np.float32)

    # wrapped int16 index layout: [128, TOTIDX//16]
    def wrap(a):
        fl = a.reshape(NCORES, TOTIDX)
        w = fl.reshape(NCORES, TOTIDX // 16, 16).transpose(0, 2, 1)
        return np.tile(w, (1, 8, 1)).astype(np.int16)

    src_w = wrap(src_rel)
    ad_w = wrap(ad_idx)
    dst_col = dst_rel.transpose(0, 2, 1).astype(BF16)  # [NCORES, 128, TOTC]

    meta = {
        "cpt": cpt, "woff": woff, "wlen": wlen, "TOTC": TOTC,
        "TOTIDX": TOTIDX,
    }
    return src_w, ad_w, dst_col, meta


def _build(meta):
    import concourse.bass as bass
    import concourse.bacc as bacc
    import concourse.mybir as mybir
    import concourse.tile as tile

    f32 = mybir.dt.float32
    bf16 = mybir.dt.bfloat16
    i16 = mybir.dt.int16
    AF = mybir.ActivationFunctionType
    OP = mybir.AluOpType

    cpt, woff, wlen = meta["cpt"], meta["woff"], meta["wlen"]
    TOTC, TOTIDX = meta["TOTC"], meta["TOTIDX"]

    nc = bacc.Bacc("TRN2", target_bir_lowering=False, debug=False)

    # ---- I/O ----
    xT_sl = nc.dram_tensor("xT_sl", [NFP, NPC], bf16, kind="ExternalInput")
    idx_src = nc.dram_tensor("idx_src", [128, TOTIDX // 16], i16, kind="ExternalInput")
    idx_ad = nc.dram_tensor("idx_ad", [128, TOTIDX // 16], i16, kind="ExternalInput")
    dst_col = nc.dram_tensor("dst_col", [128, TOTC], bf16, kind="ExternalInput")
    fb_p = nc.dram_tensor("fb_p", [NFP, NMEL], bf16, kind="ExternalInput")
    Wg_d = nc.dram_tensor("Wg", [NMEL, HC], f32, kind="ExternalInput")
    attb_s = nc.dram_tensor("attb_s", [HC, 4], f32, kind="ExternalInput")
    attb_d = nc.dram_tensor("attb_d", [HC, 4], f32, kind="ExternalInput")
    bias_bc = nc.dram_tensor("bias_bc", [128, HC], bf16, kind="ExternalInput")
    W1_d = nc.dram_tensor("W1", [HC, 256], bf16, kind="ExternalInput")
    b1_d = nc.dram_tensor("b1", [128, 4], f32, kind="ExternalInput")   # [b1 | -a*b1]
    W2_d = nc.dram_tensor("W2", [256, HC], bf16, kind="ExternalInput")
    b2_d = nc.dram_tensor("b2", [128, 2], f32, kind="ExternalInput")   # [b2 | -a*b2]
    W3_d = nc.dram_tensor("W3", [HC, 10], bf16, kind="ExternalInput")
    b3_d = nc.dram_tensor("b3", [128, 2], f32, kind="ExternalInput")   # [b3 | -a*b3]
    eye_f = nc.dram_tensor("eye_f", [128, 128], f32, kind="ExternalInput")
    eye_b = nc.dram_tensor("eye_b", [128, 128], bf16, kind="ExternalInput")
    iota_d = nc.dram_tensor("iota", [128, 128], bf16, kind="ExternalInput")
    ones_d = nc.dram_tensor("ones", [128, 16], bf16, kind="ExternalInput")
    flag_d = nc.dram_tensor("flagz", [1, 16], bf16, kind="ExternalInput")
    outT = nc.dram_tensor("outT", [10, NPC], f32, kind="ExternalOutput")

    core_ids = list(range(NCORES))

    with tile.TileContext(nc) as tc:
        with (
            tc.tile_pool(name="dram", bufs=1, space="DRAM") as dpool,
            tc.tile_pool(name="const", bufs=1) as cpool,
        ):
            if _KDW == 1:
                # one shared table in plain node order; barrier flag separate
                Hfull = dpool.tile([NPAD, RDX], bf16, addr_space="Shared")
                Bar = dpool.tile([8, 16], bf16, addr_space="Shared")
                flag_loc = dpool.tile([1, 16], bf16)
            elif _KDW == 2:
                Hext_loc = dpool.tile([NPC, RDX], bf16)
                Hfull = dpool.tile([NPAD, RDX], bf16, addr_space="Shared")
            else:
                Hext_loc = dpool.tile([NPC, RDX], bf16)
                Hfull_a = dpool.tile([8 * QSR[1], RDX], bf16, addr_space="Shared")
                Hfull_b = dpool.tile([8 * (NPC - QSR[1]), RDX], bf16,
                                     addr_space="Shared")
            adrep = dpool.tile([NPC, 128], bf16)

            # ---- constants to SBUF ----
            fb_t = cpool.tile([128, 5, NMEL], bf16)
            nc.sync.dma_start(fb_t[:], fb_p.rearrange("(b p) m -> p b m", p=128))
            Wg_t = cpool.tile([128, HC], f32)
            nc.sync.dma_start(Wg_t[:], Wg_d[:])
            atts_t = cpool.tile([128, 4], f32)
            nc.sync.dma_start(atts_t[:], attb_s[:])
            attd_t = cpool.tile([128, 4], f32)
            nc.sync.dma_start(attd_t[:], attb_d[:])
            bias_t = cpool.tile([128, HC], bf16)
            nc.sync.dma_start(bias_t[:], bias_bc[:])
            W1_t = cpool.tile([128, 256], bf16)
            nc.sync.dma_start(W1_t[:], W1_d[:])
            b1_t = cpool.tile([128, 4], f32)
            nc.sync.dma_start(b1_t[:], b1_d[:])
            W2_t = cpool.tile([128, 2, HC], bf16)
            nc.sync.dma_start(W2_t[:], W2_d.rearrange("(b p) m -> p b m", p=128))
            b2_t = cpool.tile([128, 2], f32)
            nc.sync.dma_start(b2_t[:], b2_d[:])
            W3_t = cpool.tile([128, 10], bf16)
            nc.sync.dma_start(W3_t[:], W3_d[:])
            b3_t = cpool.tile([128, 2], f32)
            nc.sync.dma_start(b3_t[:], b3_d[:])
            eyef_t = cpool.tile([128, 128], f32)
            nc.sync.dma_start(eyef_t[:], eye_f[:])
            eyeb_t = cpool.tile([128, 128], bf16)
            nc.sync.dma_start(eyeb_t[:], eye_b[:])
            iota_t = cpool.tile([128, 128], bf16)
            nc.sync.dma_start(iota_t[:], iota_d[:])
            ones_t = cpool.tile([128, 16], bf16)
            nc.sync.dma_start(ones_t[:], ones_d[:])
            isrc_t = cpool.tile([128, TOTIDX // 16], i16)
            nc.sync.dma_start(isrc_t[:], idx_src[:])
            iad_t = cpool.tile([128, TOTIDX // 16], i16)
            nc.sync.dma_start(iad_t[:], idx_ad[:])
            dcol_t = cpool.tile([128, TOTC], bf16)
            nc.sync.dma_start(dcol_t[:], dst_col[:])

            # Wgaug [mel 128, 136] bf16 = [Wg | Wg@att_s | Wg@att_d]
            Wgaug_t = cpool.tile([128, 136], bf16)
            with tc.tile_pool(name="cpsum", bufs=1, space="PSUM") as cpsum:
                WgT_ps = cpsum.tile([128, 128], f32)
                nc.tensor.transpose(WgT_ps[:], Wg_t[:], eyef_t[:])
                WgT_t = cpool.tile([128, 128], f32)
                nc.vector.tensor_copy(WgT_t[:], WgT_ps[:])
                Wgatt_ps = cpsum.tile([128, 8], f32)
                nc.tensor.matmul(Wgatt_ps[:, 0:4], WgT_t[:], atts_t[:])
                nc.tensor.matmul(Wgatt_ps[:, 4:8], WgT_t[:], attd_t[:])
                nc.vector.tensor_copy(Wgaug_t[:, 0:128], Wg_t[:])
                nc.vector.tensor_copy(Wgaug_t[:, 128:136], Wgatt_ps[:])

            # ================= stage A =================
            bar = None
            rows_sb = cpool.tile([128, TPC, 136], bf16, name="rows_sb") if _KDW == 1 else None
            with (
                tc.tile_pool(name="sa_sb", bufs=2) as sa,
                tc.tile_pool(name="sa_ps", bufs=2, space="PSUM") as saps,
                tc.tile_pool(name="sa_ps1", bufs=2, space="PSUM") as saps1,
            ):
                QEND = {24: 0, 49: 1}
                for g0 in range(0, TPC, 4):
                    gsz = min(4, TPC - g0)
                    gn = gsz * NT
                    h1T_ps = saps.tile([128, 512], f32, tag="h1T")
                    for b in range(5):
                        xtb = sa.tile([128, 512], bf16, tag="xtb", bufs=6)
                        nc.sync.dma_start(
                            xtb[:, 0:gn],
                            xT_sl[b * 128:(b + 1) * 128,
                                  g0 * NT:g0 * NT + gn])
                        nc.tensor.matmul(
                            h1T_ps[:, 0:gn], fb_t[:, b, :], xtb[:, 0:gn],
                            start=(b == 0), stop=(b == 4))
                    h1T = sa.tile([128, 512], bf16, tag="h1Ts")
                    nc.scalar.activation(h1T[:, 0:gn], h1T_ps[:, 0:gn], AF.Copy)
                    for u in range(gsz):
                        s = g0 + u
                        h_ps = saps1.tile([128, 136], f32, tag="hps")
                        nc.tensor.matmul(
                            h_ps[:], h1T[:, u * NT:(u + 1) * NT], Wgaug_t[:])
                        if _KDW == 1:
                            hrow = rows_sb[:, s, :]
                            nc.scalar.activation(hrow, h_ps[:], AF.Copy)
                            hoff = rows_sb.offset + s * 136
                        else:
                            hrow_t = sa.tile([128, 136], bf16, tag="hrow")
                            hrow = hrow_t[:]
                            nc.scalar.activation(hrow, h_ps[:], AF.Copy)
                            hoff = hrow_t.offset
                            nc.sync.dma_start(
                                Hext_loc[s * NT:(s + 1) * NT, 0:132],
                                hrow_t[:, 0:132])
                        adr = sa.tile([128, 128], bf16, tag="adr")
                        nc.vector.tensor_copy(
                            adr[:].rearrange("p (a b) -> p a b", a=32, b=4),
                            bass.AP(hrow.tensor, hoff + 132,
                                    [hrow.ap[0], [0, 32], [1, 4]]))
                        nc.sync.dma_start(adrep[s * NT:(s + 1) * NT, :], adr[:])
                    if _KDW == 0 and (g0 + gsz) in QEND:
                        q = QEND[g0 + gsz]
                        hf_out = Hfull_a if q == 0 else Hfull_b
                        nc.gpsimd.collective_compute(
                            "AllGather", mybir.AluOpType.bypass,
                            ins=[Hext_loc[QSR[q]:QSR[q + 1], :]],
                            outs=[hf_out[:]],
                            replica_groups=[core_ids])
                if _KDW == 2:
                    nc.gpsimd.collective_compute(
                        "AllGather", mybir.AluOpType.bypass,
                        ins=[Hext_loc[:]],
                        outs=[Hfull[:]],
                        replica_groups=[core_ids])
                if _KDW == 1:
                    import concourse.bass as _b
                    fz = sa.tile([1, 16], bf16, tag="fz")
                    nc.vector.memset(fz[:], 0.0)
                    nc.sync.dma_start(flag_loc[:, :], fz[:])
                    rk = nc.sync.partition_id()
                    rk_off = rk * (NPC * RDX)
                    # single write of the whole slice into the shared table
                    w = nc.sync.dma_start(
                        bass.AP(Hfull.tensor, rk_off + Hfull.offset,
                                [[RDX, NT], [NT * RDX, TPC], [1, 132]]),
                        rows_sb[:, :, 0:132])
                    bar = nc.gpsimd.collective_compute(
                        "AllGather", mybir.AluOpType.bypass,
                        ins=[flag_loc[0:1, 0:16]],
                        outs=[Bar[:, :]],
                        replica_groups=[core_ids])
                    _b._add_dep_helper(bar.ins, w.ins, sync=True,
                                       reason="barrier after shared write")

            # gather table views
            if _KDW >= 1:
                TA = Hfull[0:NPAD, :]
                TB = Hfull[SPLIT:NPAD, :]
            else:
                TA = Hfull_a[:]
                TB = Hfull_b[:]

            # ================= edge phase + MLP =================
            coffs = np.concatenate([[0], np.cumsum(cpt.sum(axis=1))]).astype(int)
            CPTA_MAX = int(cpt[:, 0].max())
            CPTB_MAX = int(cpt[:, 1].max())
            TOT_MAX = int((cpt[:, 0] + cpt[:, 1]).max())

            with (
                tc.tile_pool(name="eg_g", bufs=3) as egg,
                tc.tile_pool(name="eg_sb", bufs=2) as egs,
                tc.tile_pool(name="eg_acc", bufs=3, space="PSUM") as egacc,
                tc.tile_pool(name="eg_tp", bufs=2, space="PSUM") as egtp,
                tc.tile_pool(name="mlp_sb", bufs=2) as msb,
                tc.tile_pool(name="mlp_ps", bufs=1, space="PSUM") as mps,
            ):
                actT4 = None
                gsz = 4
                for s in range(TPC):
                    cA, cB = int(cpt[s, 0]), int(cpt[s, 1])
                    tot = cA + cB
                    coff = int(coffs[s])

                    acc = egacc.tile([128, 132], f32, tag="acc")
                    nc.vector.memset(acc[:], 0.0)

                    ad = egg.tile([128, TOT_MAX, 128], bf16, tag="ad")
                    nc.gpsimd.dma_gather(
                        ad[:, 0:tot, :], adrep[:],
                        iad_t[:, coff * 8:(coff + tot) * 8],
                        num_idxs=tot * NT, num_idxs_reg=tot * NT,
                        elem_size=128, single_packet=False)

                    gA = egg.tile([128, CPTA_MAX, RDX], bf16, tag="gA")
                    giA = nc.gpsimd.dma_gather(
                        gA[:, 0:cA, :], TA,
                        isrc_t[:, coff * 8:(coff + cA) * 8],
                        num_idxs=cA * NT, num_idxs_reg=cA * NT,
                        elem_size=RDX, single_packet=False)
                    gB = egg.tile([128, CPTB_MAX, RDX], bf16, tag="gB")
                    giB = nc.gpsimd.dma_gather(
                        gB[:, 0:cB, :], TB,
                        isrc_t[:, (coff + cA) * 8:(coff + tot) * 8],
                        num_idxs=cB * NT, num_idxs_reg=cB * NT,
                        elem_size=RDX, single_packet=False)
                    if bar is not None:
                        import concourse.bass as _b
                        for gi in (giA, giB):
                            if gi is not None:
                                _b._add_dep_helper(
                                    gi.ins, bar.ins, sync=True,
                                    reason="gather after shared-table barrier")

                    ind = egs.tile([128, TOT_MAX, 128], bf16, tag="ind")
                    for (gt, c0, nh) in ((gA, 0, cA), (gB, cA, cB)):
                        if nh == 0:
                            continue
                        # t = a_s + a_d ; lrelu = max(t, 0.2t) ; ex = exp
                        tt = egs.tile([128, TOT_MAX, 4], bf16, tag="tt", bufs=2)
                        nc.vector.tensor_tensor(
                            tt[:, 0:nh, :], gt[:, 0:nh, 128:132],
                            ad[:, c0:c0 + nh, 0:4], OP.add)
                        t2 = egs.tile([128, TOT_MAX, 4], bf16, tag="t2", bufs=2)
                        nc.vector.tensor_scalar(
                            t2[:, 0:nh, :], tt[:, 0:nh, :], NEG_ATT, None,
                            OP.mult)
                        nc.vector.tensor_tensor(
                            tt[:, 0:nh, :], tt[:, 0:nh, :], t2[:, 0:nh, :],
                            OP.max)
                        nc.scalar.activation(
                            tt[:, 0:nh, :], tt[:, 0:nh, :], AF.Exp)
                        # msg *= ex (per head block)
                        g4 = bass.AP(
                            gt.tensor, gt.offset,
                            [gt.ap[0], [RDX, nh], [32, 4], [1, 32]])
                        exb = bass.AP(
                            tt.tensor, tt.offset,
                            [tt.ap[0], [4, nh], [1, 4], [0, 32]])
                        nc.vector.tensor_tensor(g4, g4, exb, OP.mult)
                        # ex -> row cols 128:132 (Activation engine copy)
                        nc.scalar.activation(
                            gt[:, 0:nh, 128:132], tt[:, 0:nh, :], AF.Copy)
                        # indicator
                        iob = bass.AP(
                            iota_t.tensor, iota_t.offset,
                            [iota_t.ap[0], [0, nh], [1, 128]])
                        dcb = bass.AP(
                            dcol_t.tensor, dcol_t.offset + coff + c0,
                            [dcol_t.ap[0], [1, nh], [0, 128]])
                        nc.vector.tensor_tensor(
                            ind[:, c0:c0 + nh, :], iob, dcb, OP.is_equal)
                        for c in range(nh):
                            wo = int(woff[coff + c0 + c])
                            wl = int(wlen[coff + c0 + c])
                            nc.tensor.matmul(
                                acc[wo:wo + wl, :],
                                ind[:, c0 + c, 0:wl], gt[:, c, 0:132],
                                start=False, stop=(c0 + c == tot - 1),
                                skip_group_check=True)

                    # normalize + bias + ELU (node-major)
                    dinv = egs.tile([128, 4], f32, tag="dinv")
                    nc.vector.tensor_scalar(
                        dinv[:], acc[:, 128:132], 1e-12, None, OP.add)
                    nc.vector.reciprocal(dinv[:], dinv[:])
                    gat = egs.tile([128, 128], bf16, tag="gat")
                    ga = bass.AP(gat.tensor, gat.offset,
                                 [gat.ap[0], [32, 4], [1, 32]])
                    aa = bass.AP(acc.tensor, acc.offset,
                                 [acc.ap[0], [32, 4], [1, 32]])
                    db = bass.AP(dinv.tensor, dinv.offset,
                                 [dinv.ap[0], [1, 4], [0, 32]])
                    nc.vector.tensor_tensor(ga, aa, db, OP.mult)
                    nc.vector.tensor_tensor(gat[:], gat[:], bias_t[:], OP.add)
                    # ELU = relu(x) - relu(1 - exp(x))
                    t1 = egs.tile([128, 128], bf16, tag="t1")
                    nc.scalar.activation(t1[:], gat[:], AF.Exp)
                    nc.scalar.activation(t1[:], t1[:], AF.Relu, scale=-1.0,
                                         bias=1.0)
                    nc.scalar.activation(gat[:], gat[:], AF.Relu)
                    nc.vector.tensor_sub(gat[:], gat[:], t1[:])
                    # transpose -> actT4
                    sub = s % 4
                    if sub == 0:
                        gsz = min(4, TPC - s)
                        actT4 = msb.tile([128, 4 * NT], bf16, tag="actT4")
                    tp = egtp.tile([128, 128], bf16, tag="tp2")
                    nc.tensor.transpose(tp[:], gat[:], eyeb_t[:])
                    nc.vector.tensor_copy(actT4[:, sub * NT:(sub + 1) * NT],
                                          tp[:])

                    if sub == gsz - 1:
                        g0 = s - sub
                        gn = gsz * NT
                        # L1: lrelu(x+b) = relu(x+b) - relu(-a*x - a*b)
                        a1 = msb.tile([128, 2, 512], bf16, tag="a1")
                        r2 = msb.tile([128, 512], bf16, tag="r2")
                        for j in range(2):
                            o1 = mps.tile([128, 512], f32, tag="o1")
                            nc.tensor.matmul(
                                o1[:, 0:gn], W1_t[:, j * 128:(j + 1) * 128],
                                actT4[:, 0:gn])
                            nc.scalar.activation(
                                a1[:, j, 0:gn], o1[:, 0:gn], AF.Relu,
                                bias=b1_t[:, j:j + 1])
                            nc.scalar.activation(
                                r2[:, 0:gn], o1[:, 0:gn], AF.Relu,
                                scale=-NEG_MLP, bias=b1_t[:, 2 + j:3 + j])
                            nc.vector.tensor_sub(
                                a1[:, j, 0:gn], a1[:, j, 0:gn], r2[:, 0:gn])
                        o2 = mps.tile([128, 512], f32, tag="o2")
                        for j in range(2):
                            nc.tensor.matmul(
                                o2[:, 0:gn], W2_t[:, j, :], a1[:, j, 0:gn],
                                start=(j == 0), stop=(j == 1))
                        a2 = msb.tile([128, 512], bf16, tag="a2")
                        r2b = msb.tile([128, 512], bf16, tag="r2b")
                        nc.scalar.activation(
                            a2[:, 0:gn], o2[:, 0:gn], AF.Relu,
                            bias=b2_t[:, 0:1])
                        nc.scalar.activation(
                            r2b[:, 0:gn], o2[:, 0:gn], AF.Relu,
                            scale=-NEG_MLP, bias=b2_t[:, 1:2])
                        nc.vector.tensor_sub(
                            a2[:, 0:gn], a2[:, 0:gn], r2b[:, 0:gn])
                        o3 = mps.tile([16, 512], f32, tag="sm", name="o3_t")
                        nc.tensor.matmul(o3[0:10, 0:gn], W3_t[:], a2[:, 0:gn])
                        z = msb.tile([16, 512], bf16, tag="z")
                        zr = msb.tile([16, 512], bf16, tag="zr")
                        nc.scalar.activation(
                            z[0:10, 0:gn], o3[0:10, 0:gn], AF.Relu,
                            bias=b3_t[0:10, 0:1])
                        nc.scalar.activation(
                            zr[0:10, 0:gn], o3[0:10, 0:gn], AF.Relu,
                            scale=-NEG_MLP, bias=b3_t[0:10, 1:2])
                        nc.vector.tensor_sub(
                            z[0:10, 0:gn], z[0:10, 0:gn], zr[0:10, 0:gn])
                        nc.scalar.activation(z[0:10, 0:gn], z[0:10, 0:gn],
                                             AF.Exp)
                        ssum = mps.tile([16, 512], f32, tag="sm",
                                        name="ssum_t")[0:1, :]
                        nc.tensor.matmul(
                            ssum[:, 0:gn], ones_t[0:10, 0:1], z[0:10, 0:gn])
                        sinv = msb.tile([1, 512], bf16, tag="sinv")
                        with nc.allow_low_precision(reason="softmax recip"):
                            nc.vector.reciprocal(sinv[:, 0:gn], ssum[:, 0:gn])
                        sx = mps.tile([16, 512], f32, tag="sm", name="sx_t")
                        nc.tensor.matmul(
                            sx[0:10, 0:gn], ones_t[0:1, 0:10], sinv[:, 0:gn])
                        res = msb.tile([16, 512], f32, tag="res")
                        nc.vector.tensor_mul(
                            res[0:10, 0:gn], z[0:10, 0:gn], sx[0:10, 0:gn])
                        nc.sync.dma_start(
                            outT[:, g0 * NT:g0 * NT + gn], res[0:10, 0:gn])

    nc.compile()
    return nc


def _inputs_per_core(inputs, src_w, ad_w, dst_col, meta):
    x = np.asarray(inputs["x"], dtype=np.float32)
    fb = np.asarray(inputs["fb"], dtype=np.float32)
    Wg = np.asarray(inputs["Wg"], dtype=np.float32)
    bias_g = np.asarray(inputs["bias_g"], dtype=np.float32)
    att_src = np.asarray(inputs["att_src"], dtype=np.float32)
    att_dst = np.asarray(inputs["att_dst"], dtype=np.float32)
    W1 = np.asarray(inputs["W1"], dtype=np.float32)
    b1 = np.asarray(inputs["b1"], dtype=np.float32)
    W2 = np.asarray(inputs["W2"], dtype=np.float32)
    b2 = np.asarray(inputs["b2"], dtype=np.float32)
    W3 = np.asarray(inputs["W3"], dtype=np.float32)
    b3 = np.asarray(inputs["b3"], dtype=np.float32)

    x_pad = np.zeros((NPAD, NFP), dtype=np.float32)
    x_pad[:N, :NF] = x
    fb_pad = np.zeros((NFP, NMEL), dtype=np.float32)
    fb_pad[:NF] = fb

    att_blk_s = np.zeros((HC, 4), dtype=np.float32)
    att_blk_d = np.zeros((HC, 4), dtype=np.float32)
    for h in range(H):
        att_blk_s[h * C:(h + 1) * C, h] = att_src[h]
        att_blk_d[h * C:(h + 1) * C, h] = att_dst[h]

    b1p = np.zeros((128, 4), dtype=np.float32)
    b1p[:, 0] = b1[:128]
    b1p[:, 1] = b1[128:]
    b1p[:, 2:4] = -NEG_MLP * b1p[:, 0:2]
    b2p = np.zeros((128, 2), dtype=np.float32)
    b2p[:, 0] = b2
    b2p[:, 1] = -NEG_MLP * b2
    b3p = np.zeros((128, 2), dtype=np.float32)
    b3p[:10, 0] = b3
    b3p[:10, 1] = -NEG_MLP * b3

    common = {
        "fb_p": fb_pad.astype(BF16), "Wg": Wg,
        "attb_s": att_blk_s, "attb_d": att_blk_d,
        "bias_bc": np.tile(bias_g[None, :], (128, 1)).astype(BF16),
        "W1": W1.astype(BF16), "b1": b1p,
        "W2": W2.astype(BF16), "b2": b2p,
        "W3": W3.astype(BF16), "b3": b3p,
        "eye_f": np.eye(128, dtype=np.float32),
        "eye_b": np.eye(128).astype(BF16),
        "iota": np.tile(np.arange(128, dtype=np.float32)[None, :],
                        (128, 1)).astype(BF16),
        "ones": np.ones((128, 16)).astype(BF16),
        "flagz": np.zeros((1, 16)).astype(BF16),
    }
    xT_pad = np.ascontiguousarray(x_pad.T.astype(BF16))  # [640, NPAD]
    maps = []
    for k in range(NCORES):
        m = dict(common)
        m["xT_sl"] = np.ascontiguousarray(xT_pad[:, k * NPC:(k + 1) * NPC])
        m["idx_src"] = src_w[k]
        m["idx_ad"] = ad_w[k]
        m["dst_col"] = dst_col[k]
        maps.append(m)
    return maps


def kernel(**inputs):
    from concourse.bass_utils import run_bass_kernel_spmd

    src_w, ad_w, dst_col, meta = _prep(inputs["edge_index"])
    key = ("nc", meta["TOTC"], tuple(meta["cpt"].reshape(-1)),
           tuple(meta["woff"]))
    if key not in _CACHE:
        _CACHE.clear()
        _CACHE[key] = _build(meta)
    nc = _CACHE[key]
    maps = _inputs_per_core(inputs, src_w, ad_w, dst_col, meta)
    res = run_bass_kernel_spmd(nc, maps, core_ids=list(range(NCORES)))
    out = np.zeros((NPAD, 10), dtype=np.float32)
    for k in range(NCORES):
        out[k * NPC:(k + 1) * NPC] = res.results[k]["outT"].T
    return out[:N]


# revision 3
# speedup vs baseline: 1.0038x; 1.0038x over previous
"""GAT (gnn_message_passing) Trainium2 Bass kernel — 8-core SPMD, v2.

Contract: kernel(**inputs) -> np.ndarray with FULL inputs / FULL output.
Self-contained: hardcodes shapes; only imports the container's concourse stack.

v2 design vs v1:
  - bf16 edge path: shared table rows are 256x bf16 (512B), scatter matmuls,
    indicator and row-scaling all bf16.
  - No Lrelu on the Activation engine (no act-table reloads): attention
    leaky-relu is max(t, 0.2t) on DVE; MLP leaky-relu is
    relu(x+b) - relu(-a*x - a*b) via two Relu activations + one DVE subtract.
  - Stage A consumes host-transposed x (no PE transposes) and emits node-major
    rows [h | a_s | a_d] with one matmul against an augmented [Wg|Wg@as|Wg@ad].
  - KDW=1: each core writes its row slice straight into the shared DRAM table
    at a partition_id()-based dynamic offset, then a tiny AllGather acts as a
    barrier. KDW=0 falls back to two real bf16 AllGathers.
"""
import sys

for _p in ("/opt/trn_rl_repo", "/root/.axon_site/_ro/trn_rl_repo"):
    if _p not in sys.path:
        sys.path.append(_p)

import os
import numpy as np
import ml_dtypes

BF16 = ml_dtypes.bfloat16
# KDW modes: 0 = two half-table AllGathers (quarter row scheme),
#            1 = direct shared write + barrier (broken: scratchpad is only
#                pair-shared, kept for reference),
#            2 = ONE fat AllGather of the full 512B-pitch table, plain order
_KDW = int(os.environ.get("KDW", "2"))

# ---------------- problem constants (hardcoded per contract) ----------------
N = 50000
NF = 513
NFP = 640            # padded feature dim (5 * 128)
NMEL = 128
H, C = 4, 32
HC = H * C           # 128
E = 800000
NEG_ATT = 0.2
NEG_MLP = 0.01

NCORES = 8
TPC = 49             # tiles per core
NT = 128             # nodes per tile
NPC = TPC * NT       # 6272 nodes per core
NPAD = NCORES * NPC  # 50176
RDX = 256            # table row pitch in bf16 elems (512 B)
SPLIT = 4 * NPC      # 25088: table A/B row split (int16 idx headroom)
QSR = (0, 3072, NPC)  # KDW=0 quarter split (rows per AllGather region)

_CACHE = {}


def _prep(edge_index):
    """Host-side edge preprocessing. Returns per-core index/metadata arrays."""
    src = np.asarray(edge_index[0], dtype=np.int64)
    dst = np.asarray(edge_index[1], dtype=np.int64)
    loop = np.arange(N, dtype=np.int64)
    src = np.concatenate([src, loop])
    dst = np.concatenate([dst, loop])

    tile_g = dst // NT                # global tile id 0..391
    if _KDW >= 1:
        half = (src >= SPLIT).astype(np.int64)
        src_row = src - half * SPLIT
    else:
        r, l = src // NPC, src % NPC
        half = (l >= QSR[1]).astype(np.int64)
        src_row = np.where(half == 1,
                           r * (NPC - QSR[1]) + (l - QSR[1]),
                           r * QSR[1] + l)
    order = np.lexsort((src, dst, half, tile_g))
    src_row, dst, tile_g, half = (src_row[order], dst[order], tile_g[order],
                                  half[order])

    NTILES_G = NPAD // NT            # 392
    cnt = np.zeros((NTILES_G, 2), dtype=np.int64)
    np.add.at(cnt, (tile_g, half), 1)
    starts = np.zeros((NTILES_G, 2), dtype=np.int64)
    starts.reshape(-1)[1:] = np.cumsum(cnt.reshape(-1))[:-1]

    # chunks per (slot, half): max over cores
    cores = np.arange(NCORES)
    cpt = np.zeros((TPC, 2), dtype=np.int64)
    for s in range(TPC):
        t_ids = cores * TPC + s
        for hf in range(2):
            cpt[s, hf] = max(1, int(np.ceil(cnt[t_ids, hf].max() / NT)))
    TOTC = int(cpt.sum())
    TOTIDX = TOTC * NT

    src_rel = np.zeros((NCORES, TOTC, NT), dtype=np.int64)
    ad_idx = np.zeros((NCORES, TOTC, NT), dtype=np.int64)
    dst_rel = np.full((NCORES, TOTC, NT), 999.0, dtype=np.float32)
    dloc_all = np.zeros((NCORES, TOTC, NT), dtype=np.int64)
    valid = np.zeros((NCORES, TOTC, NT), dtype=bool)

    for k in range(NCORES):
        coff = 0
        for s in range(TPC):
            t = k * TPC + s
            for hf in range(2):
                nch = int(cpt[s, hf])
                st, cn = starts[t, hf], int(cnt[t, hf])
                src_rel[k, coff:coff + nch].reshape(-1)[:cn] = src_row[st:st + cn]
                ad_idx[k, coff:coff + nch].reshape(-1)[:cn] = (
                    dst[st:st + cn] % NPC - (s // 2) * 2 * NT)
                dloc_all[k, coff:coff + nch].reshape(-1)[:cn] = dst[st:st + cn] % NT
                valid[k, coff:coff + nch].reshape(-1)[:cn] = True
                coff += nch
        assert coff == TOTC

    assert src_rel.min() >= 0 and src_rel.max() <= 32767

    # per-chunk dst windows: 64-wide when the cross-core span fits, else 128
    woff = np.zeros(TOTC, dtype=np.int64)
    wlen = np.full(TOTC, 128, dtype=np.int64)
    for c in range(TOTC):
        v = valid[:, c, :]
        if v.any():
            dl = dloc_all[:, c, :][v]
            lo, hi = int(dl.min()), int(dl.max())
            wo = 0 if lo < 64 else 64
            if hi < wo + 64:
                woff[c] = wo
                wlen[c] = 64

    for k in range(NCORES):
        dr = dloc_all[k] - woff[:, None]
        dst_rel[k][valid[k]] = dr[valid[k]].astype(np.float32)

    # ---- regroup chunks: G slots per gather group, per (group, half) with
    # wl=64 chunks first so the indicator op can run width-aware ----
    G = 2
    coffs0 = np.concatenate([[0], np.cumsum(cpt.sum(axis=1))]).astype(int)
    perm = []          # new order -> original chunk index
    groups = []        # per group: dict
    for g0 in range(0, TPC, G):
        sl = [s for s in range(g0, min(g0 + G, TPC))]
        ginfo = {"slots": sl, "halves": []}
        for hf in range(2):
            idxs = []
            for s in sl:
                base = coffs0[s] + (0 if hf == 0 else int(cpt[s, 0]))
                idxs += [(base + j, s) for j in range(int(cpt[s, hf]))]
            idxs.sort(key=lambda t: 0 if wlen[t[0]] == 64 else 1)
            n64 = sum(1 for (c, _) in idxs if wlen[c] == 64)
            ginfo["halves"].append({
                "n": len(idxs), "n64": n64,
                "slot_of": [s for (_, s) in idxs],
            })
            perm += [c for (c, _) in idxs]
        groups.append(ginfo)
    perm = np.array(perm, dtype=np.int64)
    assert len(perm) == TOTC and len(set(perm.tolist())) == TOTC

    src_rel = src_rel[:, perm]
    ad_idx = ad_idx[:, perm]
    dst_rel = dst_rel[:, perm]
    woff = woff[perm]
    wlen = wlen[perm]

    # wrapped int16 index layout: [128, TOTIDX//16]
    def wrap(a):
        fl = a.reshape(NCORES, TOTIDX)
        w = fl.reshape(NCORES, TOTIDX // 16, 16).transpose(0, 2, 1)
        return np.tile(w, (1, 8, 1)).astype(np.int16)

    src_w = wrap(src_rel)
    ad_w = wrap(ad_idx)
    dst_col = dst_rel.transpose(0, 2, 1).astype(BF16)  # [NCORES, 128, TOTC]

    meta = {
        "cpt": cpt, "woff": woff, "wlen": wlen, "TOTC": TOTC,
        "TOTIDX": TOTIDX, "groups": groups, "G": G,
    }
    return src_w, ad_w, dst_col, meta


def _build(meta):
    import concourse.bass as bass
    import concourse.bacc as bacc
    import concourse.mybir as mybir
    import concourse.tile as tile

    f32 = mybir.dt.float32
    bf16 = mybir.dt.bfloat16
    i16 = mybir.dt.int16
    AF = mybir.ActivationFunctionType
    OP = mybir.AluOpType

    cpt, woff, wlen = meta["cpt"], meta["woff"], meta["wlen"]
    TOTC, TOTIDX = meta["TOTC"], meta["TOTIDX"]

    nc = bacc.Bacc("TRN2", target_bir_lowering=False, debug=False)

    # ---- I/O ----
    xT_sl = nc.dram_tensor("xT_sl", [NFP, NPC], bf16, kind="ExternalInput")
    idx_src = nc.dram_tensor("idx_src", [128, TOTIDX // 16], i16, kind="ExternalInput")
    idx_ad = nc.dram_tensor("idx_ad", [128, TOTIDX // 16], i16, kind="ExternalInput")
    dst_col = nc.dram_tensor("dst_col", [128, TOTC], bf16, kind="ExternalInput")
    fb_p = nc.dram_tensor("fb_p", [NFP, NMEL], bf16, kind="ExternalInput")
    Wg_d = nc.dram_tensor("Wg", [NMEL, HC], f32, kind="ExternalInput")
    attb_s = nc.dram_tensor("attb_s", [HC, 4], f32, kind="ExternalInput")
    attb_d = nc.dram_tensor("attb_d", [HC, 4], f32, kind="ExternalInput")
    bias_bc = nc.dram_tensor("bias_bc", [128, HC], bf16, kind="ExternalInput")
    W1_d = nc.dram_tensor("W1", [HC, 256], bf16, kind="ExternalInput")
    b1_d = nc.dram_tensor("b1", [128, 4], f32, kind="ExternalInput")   # [b1 | -a*b1]
    W2_d = nc.dram_tensor("W2", [256, HC], bf16, kind="ExternalInput")
    b2_d = nc.dram_tensor("b2", [128, 2], f32, kind="ExternalInput")   # [b2 | -a*b2]
    W3_d = nc.dram_tensor("W3", [HC, 10], bf16, kind="ExternalInput")
    b3_d = nc.dram_tensor("b3", [128, 2], f32, kind="ExternalInput")   # [b3 | -a*b3]
    eye_f = nc.dram_tensor("eye_f", [128, 128], f32, kind="ExternalInput")
    eye_b = nc.dram_tensor("eye_b", [128, 128], bf16, kind="ExternalInput")
    iota_d = nc.dram_tensor("iota", [128, 128], bf16, kind="ExternalInput")
    ones_d = nc.dram_tensor("ones", [128, 16], bf16, kind="ExternalInput")
    flag_d = nc.dram_tensor("flagz", [1, 16], bf16, kind="ExternalInput")
    outT = nc.dram_tensor("outT", [10, NPC], f32, kind="ExternalOutput")

    core_ids = list(range(NCORES))

    with tile.TileContext(nc) as tc:
        with (
            tc.tile_pool(name="dram", bufs=1, space="DRAM") as dpool,
            tc.tile_pool(name="const", bufs=1) as cpool,
        ):
            if _KDW == 1:
                # one shared table in plain node order; barrier flag separate
                Hfull = dpool.tile([NPAD, RDX], bf16, addr_space="Shared")
                Bar = dpool.tile([8, 16], bf16, addr_space="Shared")
                flag_loc = dpool.tile([1, 16], bf16)
            elif _KDW == 2:
                Hext_loc = dpool.tile([NPC, RDX], bf16)
                Hfull = dpool.tile([NPAD, RDX], bf16, addr_space="Shared")
            else:
                Hext_loc = dpool.tile([NPC, RDX], bf16)
                Hfull_a = dpool.tile([8 * QSR[1], RDX], bf16, addr_space="Shared")
                Hfull_b = dpool.tile([8 * (NPC - QSR[1]), RDX], bf16,
                                     addr_space="Shared")
            adrep = dpool.tile([NPC, 128], bf16)

            # ---- constants to SBUF ----
            fb_t = cpool.tile([128, 5, NMEL], bf16)
            nc.sync.dma_start(fb_t[:], fb_p.rearrange("(b p) m -> p b m", p=128))
            Wg_t = cpool.tile([128, HC], f32)
            nc.sync.dma_start(Wg_t[:], Wg_d[:])
            atts_t = cpool.tile([128, 4], f32)
            nc.sync.dma_start(atts_t[:], attb_s[:])
            attd_t = cpool.tile([128, 4], f32)
            nc.sync.dma_start(attd_t[:], attb_d[:])
            bias_t = cpool.tile([128, HC], bf16)
            nc.sync.dma_start(bias_t[:], bias_bc[:])
            W1_t = cpool.tile([128, 256], bf16)
            nc.sync.dma_start(W1_t[:], W1_d[:])
            b1_t = cpool.tile([128, 4], f32)
            nc.sync.dma_start(b1_t[:], b1_d[:])
            W2_t = cpool.tile([128, 2, HC], bf16)
            nc.sync.dma_start(W2_t[:], W2_d.rearrange("(b p) m -> p b m", p=128))
            b2_t = cpool.tile([128, 2], f32)
            nc.sync.dma_start(b2_t[:], b2_d[:])
            W3_t = cpool.tile([128, 10], bf16)
            nc.sync.dma_start(W3_t[:], W3_d[:])
            b3_t = cpool.tile([128, 2], f32)
            nc.sync.dma_start(b3_t[:], b3_d[:])
            eyef_t = cpool.tile([128, 128], f32)
            nc.sync.dma_start(eyef_t[:], eye_f[:])
            eyeb_t = cpool.tile([128, 128], bf16)
            nc.sync.dma_start(eyeb_t[:], eye_b[:])
            iota_t = cpool.tile([128, 128], bf16)
            nc.sync.dma_start(iota_t[:], iota_d[:])
            ones_t = cpool.tile([128, 16], bf16)
            nc.sync.dma_start(ones_t[:], ones_d[:])
            isrc_t = cpool.tile([128, TOTIDX // 16], i16)
            nc.sync.dma_start(isrc_t[:], idx_src[:])
            iad_t = cpool.tile([128, TOTIDX // 16], i16)
            nc.sync.dma_start(iad_t[:], idx_ad[:])
            dcol_t = cpool.tile([128, TOTC], bf16)
            nc.sync.dma_start(dcol_t[:], dst_col[:])

            # Wgaug [mel 128, 136] bf16 = [Wg | Wg@att_s | Wg@att_d]
            Wgaug_t = cpool.tile([128, 136], bf16)
            with tc.tile_pool(name="cpsum", bufs=1, space="PSUM") as cpsum:
                WgT_ps = cpsum.tile([128, 128], f32)
                nc.tensor.transpose(WgT_ps[:], Wg_t[:], eyef_t[:])
                WgT_t = cpool.tile([128, 128], f32)
                nc.vector.tensor_copy(WgT_t[:], WgT_ps[:])
                Wgatt_ps = cpsum.tile([128, 8], f32)
                nc.tensor.matmul(Wgatt_ps[:, 0:4], WgT_t[:], atts_t[:])
                nc.tensor.matmul(Wgatt_ps[:, 4:8], WgT_t[:], attd_t[:])
                nc.vector.tensor_copy(Wgaug_t[:, 0:128], Wg_t[:])
                nc.vector.tensor_copy(Wgaug_t[:, 128:136], Wgatt_ps[:])

            # ================= stage A =================
            bar = None
            rows_sb = cpool.tile([128, TPC, 136], bf16, name="rows_sb") if _KDW == 1 else None
            with (
                tc.tile_pool(name="sa_sb", bufs=2) as sa,
                tc.tile_pool(name="sa_ps", bufs=2, space="PSUM") as saps,
                tc.tile_pool(name="sa_ps1", bufs=2, space="PSUM") as saps1,
            ):
                QEND = {24: 0, 49: 1}
                for g0 in range(0, TPC, 4):
                    gsz = min(4, TPC - g0)
                    gn = gsz * NT
                    h1T_ps = saps.tile([128, 512], f32, tag="h1T")
                    for b in range(5):
                        xtb = sa.tile([128, 512], bf16, tag="xtb", bufs=6)
                        nc.sync.dma_start(
                            xtb[:, 0:gn],
                            xT_sl[b * 128:(b + 1) * 128,
                                  g0 * NT:g0 * NT + gn])
                        nc.tensor.matmul(
                            h1T_ps[:, 0:gn], fb_t[:, b, :], xtb[:, 0:gn],
                            start=(b == 0), stop=(b == 4))
                    h1T = sa.tile([128, 512], bf16, tag="h1Ts")
                    nc.scalar.activation(h1T[:, 0:gn], h1T_ps[:, 0:gn], AF.Copy)
                    for u in range(gsz):
                        s = g0 + u
                        h_ps = saps1.tile([128, 136], f32, tag="hps")
                        nc.tensor.matmul(
                            h_ps[:], h1T[:, u * NT:(u + 1) * NT], Wgaug_t[:])
                        if _KDW == 1:
                            hrow = rows_sb[:, s, :]
                            nc.scalar.activation(hrow, h_ps[:], AF.Copy)
                            hoff = rows_sb.offset + s * 136
                        else:
                            hrow_t = sa.tile([128, 136], bf16, tag="hrow")
                            hrow = hrow_t[:]
                            nc.scalar.activation(hrow, h_ps[:], AF.Copy)
                            hoff = hrow_t.offset
                            nc.sync.dma_start(
                                Hext_loc[s * NT:(s + 1) * NT, 0:132],
                                hrow_t[:, 0:132])
                        adr = sa.tile([128, 128], bf16, tag="adr")
                        nc.vector.tensor_copy(
                            adr[:].rearrange("p (a b) -> p a b", a=32, b=4),
                            bass.AP(hrow.tensor, hoff + 132,
                                    [hrow.ap[0], [0, 32], [1, 4]]))
                        nc.sync.dma_start(adrep[s * NT:(s + 1) * NT, :], adr[:])
                    if _KDW == 0 and (g0 + gsz) in QEND:
                        q = QEND[g0 + gsz]
                        hf_out = Hfull_a if q == 0 else Hfull_b
                        nc.gpsimd.collective_compute(
                            "AllGather", mybir.AluOpType.bypass,
                            ins=[Hext_loc[QSR[q]:QSR[q + 1], :]],
                            outs=[hf_out[:]],
                            replica_groups=[core_ids])
                if _KDW == 2:
                    nc.gpsimd.collective_compute(
                        "AllGather", mybir.AluOpType.bypass,
                        ins=[Hext_loc[:]],
                        outs=[Hfull[:]],
                        replica_groups=[core_ids])
                if _KDW == 1:
                    import concourse.bass as _b
                    fz = sa.tile([1, 16], bf16, tag="fz")
                    nc.vector.memset(fz[:], 0.0)
                    nc.sync.dma_start(flag_loc[:, :], fz[:])
                    rk = nc.sync.partition_id()
                    rk_off = rk * (NPC * RDX)
                    # single write of the whole slice into the shared table
                    w = nc.sync.dma_start(
                        bass.AP(Hfull.tensor, rk_off + Hfull.offset,
                                [[RDX, NT], [NT * RDX, TPC], [1, 132]]),
                        rows_sb[:, :, 0:132])
                    bar = nc.gpsimd.collective_compute(
                        "AllGather", mybir.AluOpType.bypass,
                        ins=[flag_loc[0:1, 0:16]],
                        outs=[Bar[:, :]],
                        replica_groups=[core_ids])
                    _b._add_dep_helper(bar.ins, w.ins, sync=True,
                                       reason="barrier after shared write")

            # compact per-edge a_d staging: [128, TOTC, 4] bf16 (~8KB/prt)
            adall = cpool.tile([128, TOTC, 4], bf16, name="adall")

            # gather table views
            if _KDW >= 1:
                TA = Hfull[0:NPAD, :]
                TB = Hfull[SPLIT:NPAD, :]
            else:
                TA = Hfull_a[:]
                TB = Hfull_b[:]

            # ================= edge phase + MLP =================
            groups = meta["groups"]
            gstarts = []
            p = 0
            for gi in groups:
                gstarts.append(p)
                p += gi["halves"][0]["n"] + gi["halves"][1]["n"]
            assert p == TOTC
            GA_MAX = max(gi["halves"][0]["n"] for gi in groups)
            GB_MAX = max(gi["halves"][1]["n"] for gi in groups)
            TOTG_MAX = max(gi["halves"][0]["n"] + gi["halves"][1]["n"]
                           for gi in groups)

            with (
                tc.tile_pool(name="eg_g", bufs=2) as egg,
                tc.tile_pool(name="eg_sb", bufs=2) as egs,
                tc.tile_pool(name="eg_acc", bufs=3, space="PSUM") as egacc,
                tc.tile_pool(name="eg_tp", bufs=2, space="PSUM") as egtp,
                tc.tile_pool(name="mlp_sb", bufs=2) as msb,
                tc.tile_pool(name="mlp_ps", bufs=1, space="PSUM") as mps,
            ):
                # --- a_d prefetch: runs on DMA engines during the AllGather
                # (adrep slices are ready as soon as stage A passes the slot;
                # compact values land in the persistent adall tile) ---
                for ginfo, gstart in zip(groups, gstarts):
                    g0 = ginfo["slots"][0]
                    gext = len(ginfo["slots"]) * NT
                    tot_g = ginfo["halves"][0]["n"] + ginfo["halves"][1]["n"]
                    adp = egg.tile([128, TOTG_MAX, 128], bf16, tag="adp",
                                   bufs=3)
                    nc.gpsimd.dma_gather(
                        adp[:, 0:tot_g, :], adrep[g0 * NT:g0 * NT + gext, :],
                        iad_t[:, gstart * 8:(gstart + tot_g) * 8],
                        num_idxs=tot_g * NT, num_idxs_reg=tot_g * NT,
                        elem_size=128, single_packet=False)
                    nc.scalar.activation(
                        adall[:, gstart:gstart + tot_g, :],
                        adp[:, 0:tot_g, 0:4], AF.Copy)

                actT4 = None
                gsz = 4
                for ginfo, gstart in zip(groups, gstarts):
                    sl = ginfo["slots"]
                    hA, hB = ginfo["halves"]
                    nA, nB = hA["n"], hB["n"]
                    tot_g = nA + nB
                    # last (half, chunk-in-half) per slot for the stop flag
                    last_of = {}
                    for hf, hh in ((0, hA), (1, hB)):
                        for j, s in enumerate(hh["slot_of"]):
                            last_of[s] = (hf, j)

                    accs = {}
                    for s in sl:
                        acc = egacc.tile([128, 132], f32, tag="acc",
                                         name=f"acc_s{s % 2}")
                        nc.vector.memset(acc[:], 0.0)
                        accs[s] = acc

                    ad = adall[:, gstart:gstart + tot_g, :]

                    gA = egg.tile([128, GA_MAX, RDX], bf16, tag="gA")
                    giA = nc.gpsimd.dma_gather(
                        gA[:, 0:nA, :], TA,
                        isrc_t[:, gstart * 8:(gstart + nA) * 8],
                        num_idxs=nA * NT, num_idxs_reg=nA * NT,
                        elem_size=RDX, single_packet=False)
                    gB = egg.tile([128, GB_MAX, RDX], bf16, tag="gB")
                    giB = nc.gpsimd.dma_gather(
                        gB[:, 0:nB, :], TB,
                        isrc_t[:, (gstart + nA) * 8:(gstart + tot_g) * 8],
                        num_idxs=nB * NT, num_idxs_reg=nB * NT,
                        elem_size=RDX, single_packet=False)
                    if bar is not None:
                        import concourse.bass as _b
                        for gi_ in (giA, giB):
                            if gi_ is not None:
                                _b._add_dep_helper(
                                    gi_.ins, bar.ins, sync=True,
                                    reason="gather after shared-table barrier")

                    ind = egs.tile([128, TOTG_MAX, 128], bf16, tag="ind")
                    for (gt, hh, c0) in ((gA, hA, 0), (gB, hB, nA)):
                        nh = hh["n"]
                        if nh == 0:
                            continue
                        # t = a_s + a_d ; lrelu = max(t, 0.2t) ; ex = exp
                        tt = egs.tile([128, TOTG_MAX, 4], bf16, tag="tt",
                                      bufs=2)
                        nc.vector.tensor_tensor(
                            tt[:, 0:nh, :], gt[:, 0:nh, 128:132],
                            ad[:, c0:c0 + nh, :], OP.add)
                        t2 = egs.tile([128, TOTG_MAX, 4], bf16, tag="t2",
                                      bufs=2)
                        nc.vector.tensor_scalar(
                            t2[:, 0:nh, :], tt[:, 0:nh, :], NEG_ATT, None,
                            OP.mult)
                        nc.vector.tensor_tensor(
                            tt[:, 0:nh, :], tt[:, 0:nh, :], t2[:, 0:nh, :],
                            OP.max)
                        nc.scalar.activation(
                            tt[:, 0:nh, :], tt[:, 0:nh, :], AF.Exp)
                        # msg *= ex (per head block)
                        g4 = bass.AP(
                            gt.tensor, gt.offset,
                            [gt.ap[0], [RDX, nh], [32, 4], [1, 32]])
                        exb = bass.AP(
                            tt.tensor, tt.offset,
                            [tt.ap[0], [4, nh], [1, 4], [0, 32]])
                        nc.vector.tensor_tensor(g4, g4, exb, OP.mult)
                        # ex -> row cols 128:132 (Activation engine copy)
                        nc.scalar.activation(
                            gt[:, 0:nh, 128:132], tt[:, 0:nh, :], AF.Copy)
                        # indicator, width-aware (wl=64 chunks ordered first)
                        n64 = hh["n64"]
                        for lo, ncnt, w in ((0, n64, 64), (n64, nh - n64, 128)):
                            if ncnt == 0:
                                continue
                            iob = bass.AP(
                                iota_t.tensor, iota_t.offset,
                                [iota_t.ap[0], [0, ncnt], [1, w]])
                            dcb = bass.AP(
                                dcol_t.tensor,
                                dcol_t.offset + gstart + c0 + lo,
                                [dcol_t.ap[0], [1, ncnt], [0, w]])
                            io = bass.AP(
                                ind.tensor, ind.offset + (c0 + lo) * 128,
                                [ind.ap[0], [128, ncnt], [1, w]])
                            nc.vector.tensor_tensor(io, iob, dcb, OP.is_equal)
                        for j in range(nh):
                            s = hh["slot_of"][j]
                            cglob = gstart + c0 + j
                            wo = int(woff[cglob])
                            wl = int(wlen[cglob])
                            nc.tensor.matmul(
                                accs[s][wo:wo + wl, :],
                                ind[:, c0 + j, 0:wl], gt[:, j, 0:132],
                                start=False,
                                stop=(last_of[s] == ((0 if c0 == 0 else 1), j)),
                                skip_group_check=True)

                    for s in sl:
                        acc = accs[s]
                        # normalize + bias + ELU (node-major)
                        dinv = egs.tile([128, 4], f32, tag="dinv")
                        nc.vector.tensor_scalar(
                            dinv[:], acc[:, 128:132], 1e-12, None, OP.add)
                        nc.vector.reciprocal(dinv[:], dinv[:])
                        gat = egs.tile([128, 128], bf16, tag="gat")
                        ga = bass.AP(gat.tensor, gat.offset,
                                     [gat.ap[0], [32, 4], [1, 32]])
                        aa = bass.AP(acc.tensor, acc.offset,
                                     [acc.ap[0], [32, 4], [1, 32]])
                        db = bass.AP(dinv.tensor, dinv.offset,
                                     [dinv.ap[0], [1, 4], [0, 32]])
                        nc.vector.tensor_tensor(ga, aa, db, OP.mult)
                        nc.vector.tensor_tensor(gat[:], gat[:], bias_t[:],
                                                OP.add)
                        # ELU = relu(x) - relu(1 - exp(x))
                        t1 = egs.tile([128, 128], bf16, tag="t1")
                        nc.scalar.activation(t1[:], gat[:], AF.Exp)
                        nc.scalar.activation(t1[:], t1[:], AF.Relu, scale=-1.0,
                                             bias=1.0)
                        nc.scalar.activation(gat[:], gat[:], AF.Relu)
                        nc.vector.tensor_sub(gat[:], gat[:], t1[:])
                        # transpose -> actT4
                        sub = s % 4
                        if sub == 0:
                            gsz = min(4, TPC - s)
                            actT4 = msb.tile([128, 4 * NT], bf16, tag="actT4")
                        tp = egtp.tile([128, 128], bf16, tag="tp2")
                        nc.tensor.transpose(tp[:], gat[:], eyeb_t[:])
                        nc.vector.tensor_copy(
                            actT4[:, sub * NT:(sub + 1) * NT], tp[:])
                        self_mlp = (sub == gsz - 1)
                        if self_mlp:
                        g0 = s - sub
                        gn = gsz * NT
                        # L1: lrelu(x+b) = relu(x+b) - relu(-a*x - a*b)
                        a1 = msb.tile([128, 2, 512], bf16, tag="a1")
                        r2 = msb.tile([128, 512], bf16, tag="r2")
                        for j in range(2):
                            o1 = mps.tile([128, 512], f32, tag="o1")
                            nc.tensor.matmul(
                                o1[:, 0:gn], W1_t[:, j * 128:(j + 1) * 128],
                                actT4[:, 0:gn])
                            nc.scalar.activation(
                                a1[:, j, 0:gn], o1[:, 0:gn], AF.Relu,
                                bias=b1_t[:, j:j + 1])
                            nc.scalar.activation(
                                r2[:, 0:gn], o1[:, 0:gn], AF.Relu,
                                scale=-NEG_MLP, bias=b1_t[:, 2 + j:3 + j])
                            nc.vector.tensor_sub(
                                a1[:, j, 0:gn], a1[:, j, 0:gn], r2[:, 0:gn])
                        o2 = mps.tile([128, 512], f32, tag="o2")
                        for j in range(2):
                            nc.tensor.matmul(
                                o2[:, 0:gn], W2_t[:, j, :], a1[:, j, 0:gn],
                                start=(j == 0), stop=(j == 1))
                        a2 = msb.tile([128, 512], bf16, tag="a2")
                        r2b = msb.tile([128, 512], bf16, tag="r2b")
                        nc.scalar.activation(
                            a2[:, 0:gn], o2[:, 0:gn], AF.Relu,
                            bias=b2_t[:, 0:1])
                        nc.scalar.activation(
                            r2b[:, 0:gn], o2[:, 0:gn], AF.Relu,
                            scale=-NEG_MLP, bias=b2_t[:, 1:2])
                        nc.vector.tensor_sub(
                            a2[:, 0:gn], a2[:, 0:gn], r2b[:, 0:gn])
                        o3 = mps.tile([16, 512], f32, tag="sm", name="o3_t")
                        nc.tensor.matmul(o3[0:10, 0:gn], W3_t[:], a2[:, 0:gn])
                        z = msb.tile([16, 512], bf16, tag="z")
                        zr = msb.tile([16, 512], bf16, tag="zr")
                        nc.scalar.activation(
                            z[0:10, 0:gn], o3[0:10, 0:gn], AF.Relu,
                            bias=b3_t[0:10, 0:1])
                        nc.scalar.activation(
                            zr[0:10, 0:gn], o3[0:10, 0:gn], AF.Relu,
                            scale=-NEG_MLP, bias=b3_t[0:10, 1:2])
                        nc.vector.tensor_sub(
                            z[0:10, 0:gn], z[0:10, 0:gn], zr[0:10, 0:gn])
                        nc.scalar.activation(z[0:10, 0:gn], z[0:10, 0:gn],
                                             AF.Exp)
                        ssum = mps.tile([16, 512], f32, tag="sm",
                                        name="ssum_t")[0:1, :]
                        nc.tensor.matmul(
                            ssum[:, 0:gn], ones_t[0:10, 0:1], z[0:10, 0:gn])
                        sinv = msb.tile([1, 512], bf16, tag="sinv")
                        with nc.allow_low_precision(reason="softmax recip"):
                            nc.vector.reciprocal(sinv[:, 0:gn], ssum[:, 0:gn])
                        sx = mps.tile([16, 512], f32, tag="sm", name="sx_t")
                        nc.tensor.matmul(
                            sx[0:10, 0:gn], ones_t[0:1, 0:10], sinv[:, 0:gn])
                        res = msb.tile([16, 512], f32, tag="res")
                        nc.vector.tensor_mul(
                            res[0:10, 0:gn], z[0:10, 0:gn], sx[0:10, 0:gn])
                        nc.sync.dma_start(
                            outT[:, g0 * NT:g0 * NT + gn], res[0:10, 0:gn])

    nc.compile()
    return nc


def _inputs_per_core(inputs, src_w, ad_w, dst_col, meta):
    x = np.asarray(inputs["x"], dtype=np.float32)
    fb = np.asarray(inputs["fb"], dtype=np.float32)
    Wg = np.asarray(inputs["Wg"], dtype=np.float32)
    bias_g = np.asarray(inputs["bias_g"], dtype=np.float32)
    att_src = np.asarray(inputs["att_src"], dtype=np.float32)
    att_dst = np.asarray(inputs["att_dst"], dtype=np.float32)
    W1 = np.asarray(inputs["W1"], dtype=np.float32)
    b1 = np.asarray(inputs["b1"], dtype=np.float32)
    W2 = np.asarray(inputs["W2"], dtype=np.float32)
    b2 = np.asarray(inputs["b2"], dtype=np.float32)
    W3 = np.asarray(inputs["W3"], dtype=np.float32)
    b3 = np.asarray(inputs["b3"], dtype=np.float32)

    x_pad = np.zeros((NPAD, NFP), dtype=np.float32)
    x_pad[:N, :NF] = x
    fb_pad = np.zeros((NFP, NMEL), dtype=np.float32)
    fb_pad[:NF] = fb

    att_blk_s = np.zeros((HC, 4), dtype=np.float32)
    att_blk_d = np.zeros((HC, 4), dtype=np.float32)
    for h in range(H):
        att_blk_s[h * C:(h + 1) * C, h] = att_src[h]
        att_blk_d[h * C:(h + 1) * C, h] = att_dst[h]

    b1p = np.zeros((128, 4), dtype=np.float32)
    b1p[:, 0] = b1[:128]
    b1p[:, 1] = b1[128:]
    b1p[:, 2:4] = -NEG_MLP * b1p[:, 0:2]
    b2p = np.zeros((128, 2), dtype=np.float32)
    b2p[:, 0] = b2
    b2p[:, 1] = -NEG_MLP * b2
    b3p = np.zeros((128, 2), dtype=np.float32)
    b3p[:10, 0] = b3
    b3p[:10, 1] = -NEG_MLP * b3

    common = {
        "fb_p": fb_pad.astype(BF16), "Wg": Wg,
        "attb_s": att_blk_s, "attb_d": att_blk_d,
        "bias_bc": np.tile(bias_g[None, :], (128, 1)).astype(BF16),
        "W1": W1.astype(BF16), "b1": b1p,
        "W2": W2.astype(BF16), "b2": b2p,
        "W3": W3.astype(BF16), "b3": b3p,
        "eye_f": np.eye(128, dtype=np.float32),
        "eye_b": np.eye(128).astype(BF16),
        "iota": np.tile(np.arange(128, dtype=np.float32)[None, :],
                        (128, 1)).astype(BF16),
        "ones": np.ones((128, 16)).astype(BF16),
        "flagz": np.zeros((1, 16)).astype(BF16),
    }
    xT_pad = np.ascontiguousarray(x_pad.T.astype(BF16))  # [640, NPAD]
    maps = []
    for k in range(NCORES):
        m = dict(common)
        m["xT_sl"] = np.ascontiguousarray(xT_pad[:, k * NPC:(k + 1) * NPC])
        m["idx_src"] = src_w[k]
        m["idx_ad"] = ad_w[k]
        m["dst_col"] = dst_col[k]
        maps.append(m)
    return maps


def kernel(**inputs):
    from concourse.bass_utils import run_bass_kernel_spmd

    src_w, ad_w, dst_col, meta = _prep(inputs["edge_index"])
    key = ("nc", meta["TOTC"], tuple(meta["cpt"].reshape(-1)),
           tuple(meta["woff"]))
    if key not in _CACHE:
        _CACHE.clear()
        _CACHE[key] = _build(meta)
    nc = _CACHE[key]
    maps = _inputs_per_core(inputs, src_w, ad_w, dst_col, meta)
    res = run_bass_kernel_spmd(nc, maps, core_ids=list(range(NCORES)))
    out = np.zeros((NPAD, 10), dtype=np.float32)
    for k in range(NCORES):
        out[k * NPC:(k + 1) * NPC] = res.results[k]["outT"].T
    return out[:N]


# revision 4
# speedup vs baseline: 1.0897x; 1.0856x over previous
"""GAT (gnn_message_passing) Trainium2 Bass kernel — 8-core SPMD, v2.

Contract: kernel(**inputs) -> np.ndarray with FULL inputs / FULL output.
Self-contained: hardcodes shapes; only imports the container's concourse stack.

v2 design vs v1:
  - bf16 edge path: shared table rows are 256x bf16 (512B), scatter matmuls,
    indicator and row-scaling all bf16.
  - No Lrelu on the Activation engine (no act-table reloads): attention
    leaky-relu is max(t, 0.2t) on DVE; MLP leaky-relu is
    relu(x+b) - relu(-a*x - a*b) via two Relu activations + one DVE subtract.
  - Stage A consumes host-transposed x (no PE transposes) and emits node-major
    rows [h | a_s | a_d] with one matmul against an augmented [Wg|Wg@as|Wg@ad].
  - KDW=1: each core writes its row slice straight into the shared DRAM table
    at a partition_id()-based dynamic offset, then a tiny AllGather acts as a
    barrier. KDW=0 falls back to two real bf16 AllGathers.
"""
import sys

for _p in ("/opt/trn_rl_repo", "/root/.axon_site/_ro/trn_rl_repo"):
    if _p not in sys.path:
        sys.path.append(_p)

import os
import numpy as np
import ml_dtypes

BF16 = ml_dtypes.bfloat16
# KDW modes: 0 = two half-table AllGathers (quarter row scheme),
#            1 = direct shared write + barrier (broken: scratchpad is only
#                pair-shared, kept for reference),
#            2 = ONE fat AllGather of the full 512B-pitch table, plain order
_KDW = int(os.environ.get("KDW", "2"))

# ---------------- problem constants (hardcoded per contract) ----------------
N = 50000
NF = 513
NFP = 640            # padded feature dim (5 * 128)
NMEL = 128
H, C = 4, 32
HC = H * C           # 128
E = 800000
NEG_ATT = 0.2
NEG_MLP = 0.01

NCORES = 8
TPC = 49             # tiles per core
NT = 128             # nodes per tile
NPC = TPC * NT       # 6272 nodes per core
NPAD = NCORES * NPC  # 50176
RDX = 256            # table row pitch in bf16 elems (512 B)
SPLIT = 4 * NPC      # 25088: table A/B row split (int16 idx headroom)
QSR = (0, 3072, NPC)  # KDW=0 quarter split (rows per AllGather region)

_CACHE = {}


def _prep(edge_index):
    """Host-side edge preprocessing. Returns per-core index/metadata arrays."""
    src = np.asarray(edge_index[0], dtype=np.int64)
    dst = np.asarray(edge_index[1], dtype=np.int64)
    loop = np.arange(N, dtype=np.int64)
    src = np.concatenate([src, loop])
    dst = np.concatenate([dst, loop])

    tile_g = dst // NT                # global tile id 0..391
    if _KDW >= 1:
        half = (src >= SPLIT).astype(np.int64)
        src_row = src - half * SPLIT
    else:
        r, l = src // NPC, src % NPC
        half = (l >= QSR[1]).astype(np.int64)
        src_row = np.where(half == 1,
                           r * (NPC - QSR[1]) + (l - QSR[1]),
                           r * QSR[1] + l)
    order = np.lexsort((src, dst, half, tile_g))
    src_row, dst, tile_g, half = (src_row[order], dst[order], tile_g[order],
                                  half[order])

    NTILES_G = NPAD // NT            # 392
    cnt = np.zeros((NTILES_G, 2), dtype=np.int64)
    np.add.at(cnt, (tile_g, half), 1)
    starts = np.zeros((NTILES_G, 2), dtype=np.int64)
    starts.reshape(-1)[1:] = np.cumsum(cnt.reshape(-1))[:-1]

    # chunks per (slot, half): max over cores
    cores = np.arange(NCORES)
    cpt = np.zeros((TPC, 2), dtype=np.int64)
    for s in range(TPC):
        t_ids = cores * TPC + s
        for hf in range(2):
            cpt[s, hf] = max(1, int(np.ceil(cnt[t_ids, hf].max() / NT)))
    TOTC = int(cpt.sum())
    TOTIDX = TOTC * NT

    src_rel = np.zeros((NCORES, TOTC, NT), dtype=np.int64)
    ad_idx = np.zeros((NCORES, TOTC, NT), dtype=np.int64)
    dst_rel = np.full((NCORES, TOTC, NT), 999.0, dtype=np.float32)
    dloc_all = np.zeros((NCORES, TOTC, NT), dtype=np.int64)
    valid = np.zeros((NCORES, TOTC, NT), dtype=bool)

    for k in range(NCORES):
        coff = 0
        for s in range(TPC):
            t = k * TPC + s
            for hf in range(2):
                nch = int(cpt[s, hf])
                st, cn = starts[t, hf], int(cnt[t, hf])
                src_rel[k, coff:coff + nch].reshape(-1)[:cn] = src_row[st:st + cn]
                ad_idx[k, coff:coff + nch].reshape(-1)[:cn] = (
                    dst[st:st + cn] % NPC - (s // 2) * 2 * NT)
                dloc_all[k, coff:coff + nch].reshape(-1)[:cn] = dst[st:st + cn] % NT
                valid[k, coff:coff + nch].reshape(-1)[:cn] = True
                coff += nch
        assert coff == TOTC

    assert src_rel.min() >= 0 and src_rel.max() <= 32767

    # per-chunk dst windows: 64-wide when the cross-core span fits, else 128
    woff = np.zeros(TOTC, dtype=np.int64)
    wlen = np.full(TOTC, 128, dtype=np.int64)
    for c in range(TOTC):
        v = valid[:, c, :]
        if v.any():
            dl = dloc_all[:, c, :][v]
            lo, hi = int(dl.min()), int(dl.max())
            wo = 0 if lo < 64 else 64
            if hi < wo + 64:
                woff[c] = wo
                wlen[c] = 64

    for k in range(NCORES):
        dr = dloc_all[k] - woff[:, None]
        dst_rel[k][valid[k]] = dr[valid[k]].astype(np.float32)

    # ---- regroup chunks: G slots per gather group, per (group, half) with
    # wl=64 chunks first so the indicator op can run width-aware ----
    G = 2
    coffs0 = np.concatenate([[0], np.cumsum(cpt.sum(axis=1))]).astype(int)
    perm = []          # new order -> original chunk index
    groups = []        # per group: dict
    for g0 in range(0, TPC, G):
        sl = [s for s in range(g0, min(g0 + G, TPC))]
        ginfo = {"slots": sl, "halves": []}
        for hf in range(2):
            idxs = []
            for s in sl:
                base = coffs0[s] + (0 if hf == 0 else int(cpt[s, 0]))
                idxs += [(base + j, s) for j in range(int(cpt[s, hf]))]
            idxs.sort(key=lambda t: 0 if wlen[t[0]] == 64 else 1)
            n64 = sum(1 for (c, _) in idxs if wlen[c] == 64)
            ginfo["halves"].append({
                "n": len(idxs), "n64": n64,
                "slot_of": [s for (_, s) in idxs],
            })
            perm += [c for (c, _) in idxs]
        groups.append(ginfo)
    perm = np.array(perm, dtype=np.int64)
    assert len(perm) == TOTC and len(set(perm.tolist())) == TOTC

    src_rel = src_rel[:, perm]
    ad_idx = ad_idx[:, perm]
    dst_rel = dst_rel[:, perm]
    woff = woff[perm]
    wlen = wlen[perm]

    # wrapped int16 index layout: [128, TOTIDX//16]
    def wrap(a):
        fl = a.reshape(NCORES, TOTIDX)
        w = fl.reshape(NCORES, TOTIDX // 16, 16).transpose(0, 2, 1)
        return np.tile(w, (1, 8, 1)).astype(np.int16)

    src_w = wrap(src_rel)
    ad_w = wrap(ad_idx)
    dst_col = dst_rel.transpose(0, 2, 1).astype(BF16)  # [NCORES, 128, TOTC]

    meta = {
        "cpt": cpt, "woff": woff, "wlen": wlen, "TOTC": TOTC,
        "TOTIDX": TOTIDX, "groups": groups, "G": G,
    }
    return src_w, ad_w, dst_col, meta


def _build(meta):
    import concourse.bass as bass
    import concourse.bacc as bacc
    import concourse.mybir as mybir
    import concourse.tile as tile

    f32 = mybir.dt.float32
    bf16 = mybir.dt.bfloat16
    i16 = mybir.dt.int16
    AF = mybir.ActivationFunctionType
    OP = mybir.AluOpType

    cpt, woff, wlen = meta["cpt"], meta["woff"], meta["wlen"]
    TOTC, TOTIDX = meta["TOTC"], meta["TOTIDX"]

    nc = bacc.Bacc("TRN2", target_bir_lowering=False, debug=False)

    # ---- I/O ----
    xT_sl = nc.dram_tensor("xT_sl", [NFP, NPC], bf16, kind="ExternalInput")
    idx_src = nc.dram_tensor("idx_src", [128, TOTIDX // 16], i16, kind="ExternalInput")
    idx_ad = nc.dram_tensor("idx_ad", [128, TOTIDX // 16], i16, kind="ExternalInput")
    dst_col = nc.dram_tensor("dst_col", [128, TOTC], bf16, kind="ExternalInput")
    fb_p = nc.dram_tensor("fb_p", [NFP, NMEL], bf16, kind="ExternalInput")
    Wg_d = nc.dram_tensor("Wg", [NMEL, HC], f32, kind="ExternalInput")
    attb_s = nc.dram_tensor("attb_s", [HC, 4], f32, kind="ExternalInput")
    attb_d = nc.dram_tensor("attb_d", [HC, 4], f32, kind="ExternalInput")
    bias_bc = nc.dram_tensor("bias_bc", [128, HC], bf16, kind="ExternalInput")
    W1_d = nc.dram_tensor("W1", [HC, 256], bf16, kind="ExternalInput")
    b1_d = nc.dram_tensor("b1", [128, 4], f32, kind="ExternalInput")   # [b1 | -a*b1]
    W2_d = nc.dram_tensor("W2", [256, HC], bf16, kind="ExternalInput")
    b2_d = nc.dram_tensor("b2", [128, 2], f32, kind="ExternalInput")   # [b2 | -a*b2]
    W3_d = nc.dram_tensor("W3", [HC, 10], bf16, kind="ExternalInput")
    b3_d = nc.dram_tensor("b3", [128, 2], f32, kind="ExternalInput")   # [b3 | -a*b3]
    eye_f = nc.dram_tensor("eye_f", [128, 128], f32, kind="ExternalInput")
    eye_b = nc.dram_tensor("eye_b", [128, 128], bf16, kind="ExternalInput")
    iota_d = nc.dram_tensor("iota", [128, 128], bf16, kind="ExternalInput")
    ones_d = nc.dram_tensor("ones", [128, 16], bf16, kind="ExternalInput")
    flag_d = nc.dram_tensor("flagz", [1, 16], bf16, kind="ExternalInput")
    outT = nc.dram_tensor("outT", [10, NPC], f32, kind="ExternalOutput")

    core_ids = list(range(NCORES))

    with tile.TileContext(nc) as tc:
        with (
            tc.tile_pool(name="dram", bufs=1, space="DRAM") as dpool,
            tc.tile_pool(name="const", bufs=1) as cpool,
        ):
            if _KDW == 1:
                # one shared table in plain node order; barrier flag separate
                Hfull = dpool.tile([NPAD, RDX], bf16, addr_space="Shared")
                Bar = dpool.tile([8, 16], bf16, addr_space="Shared")
                flag_loc = dpool.tile([1, 16], bf16)
            elif _KDW == 2:
                Hext_loc = dpool.tile([NPC, RDX], bf16)
                Hfull = dpool.tile([NPAD, RDX], bf16, addr_space="Shared")
            else:
                Hext_loc = dpool.tile([NPC, RDX], bf16)
                Hfull_a = dpool.tile([8 * QSR[1], RDX], bf16, addr_space="Shared")
                Hfull_b = dpool.tile([8 * (NPC - QSR[1]), RDX], bf16,
                                     addr_space="Shared")
            adrep = dpool.tile([NPC, 128], bf16)

            # ---- constants to SBUF ----
            fb_t = cpool.tile([128, 5, NMEL], bf16)
            nc.sync.dma_start(fb_t[:], fb_p.rearrange("(b p) m -> p b m", p=128))
            Wg_t = cpool.tile([128, HC], f32)
            nc.sync.dma_start(Wg_t[:], Wg_d[:])
            atts_t = cpool.tile([128, 4], f32)
            nc.sync.dma_start(atts_t[:], attb_s[:])
            attd_t = cpool.tile([128, 4], f32)
            nc.sync.dma_start(attd_t[:], attb_d[:])
            bias_t = cpool.tile([128, HC], bf16)
            nc.sync.dma_start(bias_t[:], bias_bc[:])
            W1_t = cpool.tile([128, 256], bf16)
            nc.sync.dma_start(W1_t[:], W1_d[:])
            b1_t = cpool.tile([128, 4], f32)
            nc.sync.dma_start(b1_t[:], b1_d[:])
            W2_t = cpool.tile([128, 2, HC], bf16)
            nc.sync.dma_start(W2_t[:], W2_d.rearrange("(b p) m -> p b m", p=128))
            b2_t = cpool.tile([128, 2], f32)
            nc.sync.dma_start(b2_t[:], b2_d[:])
            W3_t = cpool.tile([128, 10], bf16)
            nc.sync.dma_start(W3_t[:], W3_d[:])
            b3_t = cpool.tile([128, 2], f32)
            nc.sync.dma_start(b3_t[:], b3_d[:])
            eyef_t = cpool.tile([128, 128], f32)
            nc.sync.dma_start(eyef_t[:], eye_f[:])
            eyeb_t = cpool.tile([128, 128], bf16)
            nc.sync.dma_start(eyeb_t[:], eye_b[:])
            iota_t = cpool.tile([128, 128], bf16)
            nc.sync.dma_start(iota_t[:], iota_d[:])
            ones_t = cpool.tile([128, 16], bf16)
            nc.sync.dma_start(ones_t[:], ones_d[:])
            isrc_t = cpool.tile([128, TOTIDX // 16], i16)
            nc.sync.dma_start(isrc_t[:], idx_src[:])
            iad_t = cpool.tile([128, TOTIDX // 16], i16)
            nc.sync.dma_start(iad_t[:], idx_ad[:])
            dcol_t = cpool.tile([128, TOTC], bf16)
            nc.sync.dma_start(dcol_t[:], dst_col[:])

            # Wgaug [mel 128, 136] bf16 = [Wg | Wg@att_s | Wg@att_d]
            Wgaug_t = cpool.tile([128, 136], bf16)
            with tc.tile_pool(name="cpsum", bufs=1, space="PSUM") as cpsum:
                WgT_ps = cpsum.tile([128, 128], f32)
                nc.tensor.transpose(WgT_ps[:], Wg_t[:], eyef_t[:])
                WgT_t = cpool.tile([128, 128], f32)
                nc.vector.tensor_copy(WgT_t[:], WgT_ps[:])
                Wgatt_ps = cpsum.tile([128, 8], f32)
                nc.tensor.matmul(Wgatt_ps[:, 0:4], WgT_t[:], atts_t[:])
                nc.tensor.matmul(Wgatt_ps[:, 4:8], WgT_t[:], attd_t[:])
                nc.vector.tensor_copy(Wgaug_t[:, 0:128], Wg_t[:])
                nc.vector.tensor_copy(Wgaug_t[:, 128:136], Wgatt_ps[:])

            # ================= stage A =================
            bar = None
            rows_sb = cpool.tile([128, TPC, 136], bf16, name="rows_sb") if _KDW == 1 else None
            with (
                tc.tile_pool(name="sa_sb", bufs=2) as sa,
                tc.tile_pool(name="sa_ps", bufs=2, space="PSUM") as saps,
                tc.tile_pool(name="sa_ps1", bufs=2, space="PSUM") as saps1,
            ):
                QEND = {24: 0, 49: 1}
                for g0 in range(0, TPC, 4):
                    gsz = min(4, TPC - g0)
                    gn = gsz * NT
                    h1T_ps = saps.tile([128, 512], f32, tag="h1T")
                    for b in range(5):
                        xtb = sa.tile([128, 512], bf16, tag="xtb", bufs=6)
                        nc.sync.dma_start(
                            xtb[:, 0:gn],
                            xT_sl[b * 128:(b + 1) * 128,
                                  g0 * NT:g0 * NT + gn])
                        nc.tensor.matmul(
                            h1T_ps[:, 0:gn], fb_t[:, b, :], xtb[:, 0:gn],
                            start=(b == 0), stop=(b == 4))
                    h1T = sa.tile([128, 512], bf16, tag="h1Ts")
                    nc.scalar.activation(h1T[:, 0:gn], h1T_ps[:, 0:gn], AF.Copy)
                    for u in range(gsz):
                        s = g0 + u
                        h_ps = saps1.tile([128, 136], f32, tag="hps")
                        nc.tensor.matmul(
                            h_ps[:], h1T[:, u * NT:(u + 1) * NT], Wgaug_t[:])
                        if _KDW == 1:
                            hrow = rows_sb[:, s, :]
                            nc.scalar.activation(hrow, h_ps[:], AF.Copy)
                            hoff = rows_sb.offset + s * 136
                        else:
                            hrow_t = sa.tile([128, 136], bf16, tag="hrow")
                            hrow = hrow_t[:]
                            nc.scalar.activation(hrow, h_ps[:], AF.Copy)
                            hoff = hrow_t.offset
                            nc.sync.dma_start(
                                Hext_loc[s * NT:(s + 1) * NT, 0:132],
                                hrow_t[:, 0:132])
                        adr = sa.tile([128, 128], bf16, tag="adr")
                        nc.vector.tensor_copy(
                            adr[:].rearrange("p (a b) -> p a b", a=32, b=4),
                            bass.AP(hrow.tensor, hoff + 132,
                                    [hrow.ap[0], [0, 32], [1, 4]]))
                        nc.sync.dma_start(adrep[s * NT:(s + 1) * NT, :], adr[:])
                    if _KDW == 0 and (g0 + gsz) in QEND:
                        q = QEND[g0 + gsz]
                        hf_out = Hfull_a if q == 0 else Hfull_b
                        nc.gpsimd.collective_compute(
                            "AllGather", mybir.AluOpType.bypass,
                            ins=[Hext_loc[QSR[q]:QSR[q + 1], :]],
                            outs=[hf_out[:]],
                            replica_groups=[core_ids])
                if _KDW == 2:
                    nc.gpsimd.collective_compute(
                        "AllGather", mybir.AluOpType.bypass,
                        ins=[Hext_loc[:]],
                        outs=[Hfull[:]],
                        replica_groups=[core_ids])
                if _KDW == 1:
                    import concourse.bass as _b
                    fz = sa.tile([1, 16], bf16, tag="fz")
                    nc.vector.memset(fz[:], 0.0)
                    nc.sync.dma_start(flag_loc[:, :], fz[:])
                    rk = nc.sync.partition_id()
                    rk_off = rk * (NPC * RDX)
                    # single write of the whole slice into the shared table
                    w = nc.sync.dma_start(
                        bass.AP(Hfull.tensor, rk_off + Hfull.offset,
                                [[RDX, NT], [NT * RDX, TPC], [1, 132]]),
                        rows_sb[:, :, 0:132])
                    bar = nc.gpsimd.collective_compute(
                        "AllGather", mybir.AluOpType.bypass,
                        ins=[flag_loc[0:1, 0:16]],
                        outs=[Bar[:, :]],
                        replica_groups=[core_ids])
                    _b._add_dep_helper(bar.ins, w.ins, sync=True,
                                       reason="barrier after shared write")

            # compact per-edge a_d staging: [128, TOTC, 4] bf16 (~8KB/prt)
            adall = cpool.tile([128, TOTC, 4], bf16, name="adall")

            # gather table views
            if _KDW >= 1:
                TA = Hfull[0:NPAD, :]
                TB = Hfull[SPLIT:NPAD, :]
            else:
                TA = Hfull_a[:]
                TB = Hfull_b[:]

            # ================= edge phase + MLP =================
            groups = meta["groups"]
            gstarts = []
            p = 0
            for gi in groups:
                gstarts.append(p)
                p += gi["halves"][0]["n"] + gi["halves"][1]["n"]
            assert p == TOTC
            GA_MAX = max(gi["halves"][0]["n"] for gi in groups)
            GB_MAX = max(gi["halves"][1]["n"] for gi in groups)
            TOTG_MAX = max(gi["halves"][0]["n"] + gi["halves"][1]["n"]
                           for gi in groups)

            with (
                tc.tile_pool(name="eg_g", bufs=3) as egg,
                tc.tile_pool(name="eg_sb", bufs=3) as egs,
                tc.tile_pool(name="eg_acc", bufs=3, space="PSUM") as egacc,
                tc.tile_pool(name="eg_tp", bufs=2, space="PSUM") as egtp,
                tc.tile_pool(name="mlp_sb", bufs=2) as msb,
                tc.tile_pool(name="mlp_ps", bufs=1, space="PSUM") as mps,
            ):
                # --- a_d prefetch: runs on DMA engines during the AllGather
                # (adrep slices are ready as soon as stage A passes the slot;
                # compact values land in the persistent adall tile) ---
                for ginfo, gstart in zip(groups, gstarts):
                    g0 = ginfo["slots"][0]
                    gext = len(ginfo["slots"]) * NT
                    tot_g = ginfo["halves"][0]["n"] + ginfo["halves"][1]["n"]
                    adp = egg.tile([128, TOTG_MAX, 128], bf16, tag="adp",
                                   bufs=3)
                    nc.gpsimd.dma_gather(
                        adp[:, 0:tot_g, :], adrep[g0 * NT:g0 * NT + gext, :],
                        iad_t[:, gstart * 8:(gstart + tot_g) * 8],
                        num_idxs=tot_g * NT, num_idxs_reg=tot_g * NT,
                        elem_size=128, single_packet=False)
                    nc.scalar.activation(
                        adall[:, gstart:gstart + tot_g, :],
                        adp[:, 0:tot_g, 0:4], AF.Copy)

                actT4 = None
                gsz = 4
                for ginfo, gstart in zip(groups, gstarts):
                    sl = ginfo["slots"]
                    hA, hB = ginfo["halves"]
                    nA, nB = hA["n"], hB["n"]
                    tot_g = nA + nB
                    # last (half, chunk-in-half) per slot for the stop flag
                    last_of = {}
                    for hf, hh in ((0, hA), (1, hB)):
                        for j, s in enumerate(hh["slot_of"]):
                            last_of[s] = (hf, j)

                    accs = {}
                    for s in sl:
                        acc = egacc.tile([128, 132], f32, tag="acc",
                                         name=f"acc_s{s % 2}")
                        nc.vector.memset(acc[:], 0.0)
                        accs[s] = acc

                    ad = adall[:, gstart:gstart + tot_g, :]

                    gA = egg.tile([128, GA_MAX, RDX], bf16, tag="gA")
                    giA = nc.gpsimd.dma_gather(
                        gA[:, 0:nA, :], TA,
                        isrc_t[:, gstart * 8:(gstart + nA) * 8],
                        num_idxs=nA * NT, num_idxs_reg=nA * NT,
                        elem_size=RDX, single_packet=False)
                    gB = egg.tile([128, GB_MAX, RDX], bf16, tag="gB")
                    giB = nc.gpsimd.dma_gather(
                        gB[:, 0:nB, :], TB,
                        isrc_t[:, (gstart + nA) * 8:(gstart + tot_g) * 8],
                        num_idxs=nB * NT, num_idxs_reg=nB * NT,
                        elem_size=RDX, single_packet=False)
                    if bar is not None:
                        import concourse.bass as _b
                        for gi_ in (giA, giB):
                            if gi_ is not None:
                                _b._add_dep_helper(
                                    gi_.ins, bar.ins, sync=True,
                                    reason="gather after shared-table barrier")

                    ind = egs.tile([128, TOTG_MAX, 128], bf16, tag="ind")
                    for (gt, hh, c0) in ((gA, hA, 0), (gB, hB, nA)):
                        nh = hh["n"]
                        if nh == 0:
                            continue
                        # t = a_s + a_d ; lrelu = max(t, 0.2t) ; ex = exp
                        tt = egs.tile([128, TOTG_MAX, 4], bf16, tag="tt",
                                      bufs=2)
                        nc.vector.tensor_tensor(
                            tt[:, 0:nh, :], gt[:, 0:nh, 128:132],
                            ad[:, c0:c0 + nh, :], OP.add)
                        t2 = egs.tile([128, TOTG_MAX, 4], bf16, tag="t2",
                                      bufs=2)
                        nc.vector.tensor_scalar(
                            t2[:, 0:nh, :], tt[:, 0:nh, :], NEG_ATT, None,
                            OP.mult)
                        nc.vector.tensor_tensor(
                            tt[:, 0:nh, :], tt[:, 0:nh, :], t2[:, 0:nh, :],
                            OP.max)
                        nc.scalar.activation(
                            tt[:, 0:nh, :], tt[:, 0:nh, :], AF.Exp)
                        # msg *= ex (per head block)
                        g4 = bass.AP(
                            gt.tensor, gt.offset,
                            [gt.ap[0], [RDX, nh], [32, 4], [1, 32]])
                        exb = bass.AP(
                            tt.tensor, tt.offset,
                            [tt.ap[0], [4, nh], [1, 4], [0, 32]])
                        nc.vector.tensor_tensor(g4, g4, exb, OP.mult)
                        # ex -> row cols 128:132 (Activation engine copy)
                        nc.scalar.activation(
                            gt[:, 0:nh, 128:132], tt[:, 0:nh, :], AF.Copy)
                        # indicator, width-aware (wl=64 chunks ordered first)
                        n64 = hh["n64"]
                        for lo, ncnt, w in ((0, n64, 64), (n64, nh - n64, 128)):
                            if ncnt == 0:
                                continue
                            iob = bass.AP(
                                iota_t.tensor, iota_t.offset,
                                [iota_t.ap[0], [0, ncnt], [1, w]])
                            dcb = bass.AP(
                                dcol_t.tensor,
                                dcol_t.offset + gstart + c0 + lo,
                                [dcol_t.ap[0], [1, ncnt], [0, w]])
                            io = bass.AP(
                                ind.tensor, ind.offset + (c0 + lo) * 128,
                                [ind.ap[0], [128, ncnt], [1, w]])
                            nc.vector.tensor_tensor(io, iob, dcb, OP.is_equal)
                        for j in range(nh):
                            s = hh["slot_of"][j]
                            cglob = gstart + c0 + j
                            wo = int(woff[cglob])
                            wl = int(wlen[cglob])
                            nc.tensor.matmul(
                                accs[s][wo:wo + wl, :],
                                ind[:, c0 + j, 0:wl], gt[:, j, 0:132],
                                start=False,
                                stop=(last_of[s] == ((0 if c0 == 0 else 1), j)),
                                skip_group_check=True)

                    for s in sl:
                        acc = accs[s]
                        # normalize + bias + ELU (node-major)
                        dinv = egs.tile([128, 4], f32, tag="dinv")
                        nc.vector.tensor_scalar(
                            dinv[:], acc[:, 128:132], 1e-12, None, OP.add)
                        nc.vector.reciprocal(dinv[:], dinv[:])
                        gat = egs.tile([128, 128], bf16, tag="gat")
                        ga = bass.AP(gat.tensor, gat.offset,
                                     [gat.ap[0], [32, 4], [1, 32]])
                        aa = bass.AP(acc.tensor, acc.offset,
                                     [acc.ap[0], [32, 4], [1, 32]])
                        db = bass.AP(dinv.tensor, dinv.offset,
                                     [dinv.ap[0], [1, 4], [0, 32]])
                        nc.vector.tensor_tensor(ga, aa, db, OP.mult)
                        nc.vector.tensor_tensor(gat[:], gat[:], bias_t[:],
                                                OP.add)
                        # ELU = relu(x) - relu(1 - exp(x))
                        t1 = egs.tile([128, 128], bf16, tag="t1")
                        nc.scalar.activation(t1[:], gat[:], AF.Exp)
                        nc.scalar.activation(t1[:], t1[:], AF.Relu, scale=-1.0,
                                             bias=1.0)
                        nc.scalar.activation(gat[:], gat[:], AF.Relu)
                        nc.vector.tensor_sub(gat[:], gat[:], t1[:])
                        # transpose -> actT4
                        sub = s % 4
                        if sub == 0:
                            gsz = min(4, TPC - s)
                            actT4 = msb.tile([128, 4 * NT], bf16, tag="actT4")
                        tp = egtp.tile([128, 128], bf16, tag="tp2")
                        nc.tensor.transpose(tp[:], gat[:], eyeb_t[:])
                        nc.vector.tensor_copy(
                            actT4[:, sub * NT:(sub + 1) * NT], tp[:])
                        self_mlp = (sub == gsz - 1)
                        if self_mlp:
                        g0 = s - sub
                        gn = gsz * NT
                        # L1: lrelu(x+b) = relu(x+b) - relu(-a*x - a*b)
                        a1 = msb.tile([128, 2, 512], bf16, tag="a1")
                        r2 = msb.tile([128, 512], bf16, tag="r2")
                        for j in range(2):
                            o1 = mps.tile([128, 512], f32, tag="o1")
                            nc.tensor.matmul(
                                o1[:, 0:gn], W1_t[:, j * 128:(j + 1) * 128],
                                actT4[:, 0:gn])
                            nc.scalar.activation(
                                a1[:, j, 0:gn], o1[:, 0:gn], AF.Relu,
                                bias=b1_t[:, j:j + 1])
                            nc.scalar.activation(
                                r2[:, 0:gn], o1[:, 0:gn], AF.Relu,
                                scale=-NEG_MLP, bias=b1_t[:, 2 + j:3 + j])
                            nc.vector.tensor_sub(
                                a1[:, j, 0:gn], a1[:, j, 0:gn], r2[:, 0:gn])
                        o2 = mps.tile([128, 512], f32, tag="o2")
                        for j in range(2):
                            nc.tensor.matmul(
                                o2[:, 0:gn], W2_t[:, j, :], a1[:, j, 0:gn],
                                start=(j == 0), stop=(j == 1))
                        a2 = msb.tile([128, 512], bf16, tag="a2")
                        r2b = msb.tile([128, 512], bf16, tag="r2b")
                        nc.scalar.activation(
                            a2[:, 0:gn], o2[:, 0:gn], AF.Relu,
                            bias=b2_t[:, 0:1])
                        nc.scalar.activation(
                            r2b[:, 0:gn], o2[:, 0:gn], AF.Relu,
                            scale=-NEG_MLP, bias=b2_t[:, 1:2])
                        nc.vector.tensor_sub(
                            a2[:, 0:gn], a2[:, 0:gn], r2b[:, 0:gn])
                        o3 = mps.tile([16, 512], f32, tag="sm", name="o3_t")
                        nc.tensor.matmul(o3[0:10, 0:gn], W3_t[:], a2[:, 0:gn])
                        z = msb.tile([16, 512], bf16, tag="z")
                        zr = msb.tile([16, 512], bf16, tag="zr")
                        nc.scalar.activation(
                            z[0:10, 0:gn], o3[0:10, 0:gn], AF.Relu,
                            bias=b3_t[0:10, 0:1])
                        nc.scalar.activation(
                            zr[0:10, 0:gn], o3[0:10, 0:gn], AF.Relu,
                            scale=-NEG_MLP, bias=b3_t[0:10, 1:2])
                        nc.vector.tensor_sub(
                            z[0:10, 0:gn], z[0:10, 0:gn], zr[0:10, 0:gn])
                        nc.scalar.activation(z[0:10, 0:gn], z[0:10, 0:gn],
                                             AF.Exp)
                        ssum = mps.tile([16, 512], f32, tag="sm",
                                        name="ssum_t")[0:1, :]
                        nc.tensor.matmul(
                            ssum[:, 0:gn], ones_t[0:10, 0:1], z[0:10, 0:gn])
                        sinv = msb.tile([1, 512], bf16, tag="sinv")
                        with nc.allow_low_precision(reason="softmax recip"):
                            nc.vector.reciprocal(sinv[:, 0:gn], ssum[:, 0:gn])
                        sx = mps.tile([16, 512], f32, tag="sm", name="sx_t")
                        nc.tensor.matmul(
                            sx[0:10, 0:gn], ones_t[0:1, 0:10], sinv[:, 0:gn])
                        res = msb.tile([16, 512], f32, tag="res")
                        nc.vector.tensor_mul(
                            res[0:10, 0:gn], z[0:10, 0:gn], sx[0:10, 0:gn])
                        nc.sync.dma_start(
                            outT[:, g0 * NT:g0 * NT + gn], res[0:10, 0:gn])

    nc.compile()
    return nc


def _inputs_per_core(inputs, src_w, ad_w, dst_col, meta):
    x = np.asarray(inputs["x"], dtype=np.float32)
    fb = np.asarray(inputs["fb"], dtype=np.float32)
    Wg = np.asarray(inputs["Wg"], dtype=np.float32)
    bias_g = np.asarray(inputs["bias_g"], dtype=np.float32)
    att_src = np.asarray(inputs["att_src"], dtype=np.float32)
    att_dst = np.asarray(inputs["att_dst"], dtype=np.float32)
    W1 = np.asarray(inputs["W1"], dtype=np.float32)
    b1 = np.asarray(inputs["b1"], dtype=np.float32)
    W2 = np.asarray(inputs["W2"], dtype=np.float32)
    b2 = np.asarray(inputs["b2"], dtype=np.float32)
    W3 = np.asarray(inputs["W3"], dtype=np.float32)
    b3 = np.asarray(inputs["b3"], dtype=np.float32)

    x_pad = np.zeros((NPAD, NFP), dtype=np.float32)
    x_pad[:N, :NF] = x
    fb_pad = np.zeros((NFP, NMEL), dtype=np.float32)
    fb_pad[:NF] = fb

    att_blk_s = np.zeros((HC, 4), dtype=np.float32)
    att_blk_d = np.zeros((HC, 4), dtype=np.float32)
    for h in range(H):
        att_blk_s[h * C:(h + 1) * C, h] = att_src[h]
        att_blk_d[h * C:(h + 1) * C, h] = att_dst[h]

    b1p = np.zeros((128, 4), dtype=np.float32)
    b1p[:, 0] = b1[:128]
    b1p[:, 1] = b1[128:]
    b1p[:, 2:4] = -NEG_MLP * b1p[:, 0:2]
    b2p = np.zeros((128, 2), dtype=np.float32)
    b2p[:, 0] = b2
    b2p[:, 1] = -NEG_MLP * b2
    b3p = np.zeros((128, 2), dtype=np.float32)
    b3p[:10, 0] = b3
    b3p[:10, 1] = -NEG_MLP * b3

    common = {
        "fb_p": fb_pad.astype(BF16), "Wg": Wg,
        "attb_s": att_blk_s, "attb_d": att_blk_d,
        "bias_bc": np.tile(bias_g[None, :], (128, 1)).astype(BF16),
        "W1": W1.astype(BF16), "b1": b1p,
        "W2": W2.astype(BF16), "b2": b2p,
        "W3": W3.astype(BF16), "b3": b3p,
        "eye_f": np.eye(128, dtype=np.float32),
        "eye_b": np.eye(128).astype(BF16),
        "iota": np.tile(np.arange(128, dtype=np.float32)[None, :],
                        (128, 1)).astype(BF16),
        "ones": np.ones((128, 16)).astype(BF16),
        "flagz": np.zeros((1, 16)).astype(BF16),
    }
    xT_pad = np.ascontiguousarray(x_pad.T.astype(BF16))  # [640, NPAD]
    maps = []
    for k in range(NCORES):
        m = dict(common)
        m["xT_sl"] = np.ascontiguousarray(xT_pad[:, k * NPC:(k + 1) * NPC])
        m["idx_src"] = src_w[k]
        m["idx_ad"] = ad_w[k]
        m["dst_col"] = dst_col[k]
        maps.append(m)
    return maps


def kernel(**inputs):
    from concourse.bass_utils import run_bass_kernel_spmd

    src_w, ad_w, dst_col, meta = _prep(inputs["edge_index"])
    key = ("nc", meta["TOTC"], tuple(meta["cpt"].reshape(-1)),
           tuple(meta["woff"]))
    if key not in _CACHE:
        _CACHE.clear()
        _CACHE[key] = _build(meta)
    nc = _CACHE[key]
    maps = _inputs_per_core(inputs, src_w, ad_w, dst_col, meta)
    res = run_bass_kernel_spmd(nc, maps, core_ids=list(range(NCORES)))
    out = np.zeros((NPAD, 10), dtype=np.float32)
    for k in range(NCORES):
        out[k * NPC:(k + 1) * NPC] = res.results[k]["outT"].T
    return out[:N]


# revision 5
# speedup vs baseline: 1.0963x; 1.0060x over previous
"""GAT (gnn_message_passing) Trainium2 Bass kernel — 8-core SPMD, v2.

Contract: kernel(**inputs) -> np.ndarray with FULL inputs / FULL output.
Self-contained: hardcodes shapes; only imports the container's concourse stack.

v2 design vs v1:
  - bf16 edge path: shared table rows are 256x bf16 (512B), scatter matmuls,
    indicator and row-scaling all bf16.
  - No Lrelu on the Activation engine (no act-table reloads): attention
    leaky-relu is max(t, 0.2t) on DVE; MLP leaky-relu is
    relu(x+b) - relu(-a*x - a*b) via two Relu activations + one DVE subtract.
  - Stage A consumes host-transposed x (no PE transposes) and emits node-major
    rows [h | a_s | a_d] with one matmul against an augmented [Wg|Wg@as|Wg@ad].
  - KDW=1: each core writes its row slice straight into the shared DRAM table
    at a partition_id()-based dynamic offset, then a tiny AllGather acts as a
    barrier. KDW=0 falls back to two real bf16 AllGathers.
"""
import sys

for _p in ("/opt/trn_rl_repo", "/root/.axon_site/_ro/trn_rl_repo"):
    if _p not in sys.path:
        sys.path.append(_p)

import os
import numpy as np
import ml_dtypes

BF16 = ml_dtypes.bfloat16
# KDW modes: 0 = two half-table AllGathers (quarter row scheme),
#            1 = direct shared write + barrier (broken: scratchpad is only
#                pair-shared, kept for reference),
#            2 = ONE fat AllGather of the full 512B-pitch table, plain order
_KDW = int(os.environ.get("KDW", "2"))

# ---------------- problem constants (hardcoded per contract) ----------------
N = 50000
NF = 513
NFP = 640            # padded feature dim (5 * 128)
NMEL = 128
H, C = 4, 32
HC = H * C           # 128
E = 800000
NEG_ATT = 0.2
NEG_MLP = 0.01

NCORES = 8
TPC = 49             # tiles per core
NT = 128             # nodes per tile
NPC = TPC * NT       # 6272 nodes per core
NPAD = NCORES * NPC  # 50176
RDX = 256            # table row pitch in bf16 elems (512 B)
SPLIT = 4 * NPC      # 25088: table A/B row split (int16 idx headroom)
QSR = (0, 3072, NPC)  # KDW=0 quarter split (rows per AllGather region)

_CACHE = {}


def _prep(edge_index):
    """Host-side edge preprocessing. Returns per-core index/metadata arrays."""
    src = np.asarray(edge_index[0], dtype=np.int64)
    dst = np.asarray(edge_index[1], dtype=np.int64)
    loop = np.arange(N, dtype=np.int64)
    src = np.concatenate([src, loop])
    dst = np.concatenate([dst, loop])

    tile_g = dst // NT                # global tile id 0..391
    if _KDW >= 1:
        half = (src >= SPLIT).astype(np.int64)
        src_row = src - half * SPLIT
    else:
        r, l = src // NPC, src % NPC
        half = (l >= QSR[1]).astype(np.int64)
        src_row = np.where(half == 1,
                           r * (NPC - QSR[1]) + (l - QSR[1]),
                           r * QSR[1] + l)
    order = np.lexsort((src, dst, half, tile_g))
    src_row, dst, tile_g, half = (src_row[order], dst[order], tile_g[order],
                                  half[order])

    NTILES_G = NPAD // NT            # 392
    cnt = np.zeros((NTILES_G, 2), dtype=np.int64)
    np.add.at(cnt, (tile_g, half), 1)
    starts = np.zeros((NTILES_G, 2), dtype=np.int64)
    starts.reshape(-1)[1:] = np.cumsum(cnt.reshape(-1))[:-1]

    # chunks per (slot, half): max over cores
    cores = np.arange(NCORES)
    cpt = np.zeros((TPC, 2), dtype=np.int64)
    for s in range(TPC):
        t_ids = cores * TPC + s
        for hf in range(2):
            cpt[s, hf] = max(1, int(np.ceil(cnt[t_ids, hf].max() / NT)))
    TOTC = int(cpt.sum())
    TOTIDX = TOTC * NT

    src_rel = np.zeros((NCORES, TOTC, NT), dtype=np.int64)
    ad_idx = np.zeros((NCORES, TOTC, NT), dtype=np.int64)
    dst_rel = np.full((NCORES, TOTC, NT), 999.0, dtype=np.float32)
    dloc_all = np.zeros((NCORES, TOTC, NT), dtype=np.int64)
    valid = np.zeros((NCORES, TOTC, NT), dtype=bool)

    for k in range(NCORES):
        coff = 0
        for s in range(TPC):
            t = k * TPC + s
            for hf in range(2):
                nch = int(cpt[s, hf])
                st, cn = starts[t, hf], int(cnt[t, hf])
                src_rel[k, coff:coff + nch].reshape(-1)[:cn] = src_row[st:st + cn]
                ad_idx[k, coff:coff + nch].reshape(-1)[:cn] = (
                    dst[st:st + cn] % NPC - (s // 2) * 2 * NT)
                dloc_all[k, coff:coff + nch].reshape(-1)[:cn] = dst[st:st + cn] % NT
                valid[k, coff:coff + nch].reshape(-1)[:cn] = True
                coff += nch
        assert coff == TOTC

    assert src_rel.min() >= 0 and src_rel.max() <= 32767

    # per-chunk dst windows: 64-wide when the cross-core span fits, else 128
    woff = np.zeros(TOTC, dtype=np.int64)
    wlen = np.full(TOTC, 128, dtype=np.int64)
    for c in range(TOTC):
        v = valid[:, c, :]
        if v.any():
            dl = dloc_all[:, c, :][v]
            lo, hi = int(dl.min()), int(dl.max())
            wo = 0 if lo < 64 else 64
            if hi < wo + 64:
                woff[c] = wo
                wlen[c] = 64

    for k in range(NCORES):
        dr = dloc_all[k] - woff[:, None]
        dst_rel[k][valid[k]] = dr[valid[k]].astype(np.float32)

    # ---- regroup chunks: G slots per gather group, per (group, half) with
    # wl=64 chunks first so the indicator op can run width-aware ----
    G = 2
    coffs0 = np.concatenate([[0], np.cumsum(cpt.sum(axis=1))]).astype(int)
    perm = []          # new order -> original chunk index
    groups = []        # per group: dict
    for g0 in range(0, TPC, G):
        sl = [s for s in range(g0, min(g0 + G, TPC))]
        ginfo = {"slots": sl, "halves": []}
        for hf in range(2):
            idxs = []
            for s in sl:
                base = coffs0[s] + (0 if hf == 0 else int(cpt[s, 0]))
                idxs += [(base + j, s) for j in range(int(cpt[s, hf]))]
            idxs.sort(key=lambda t: 0 if wlen[t[0]] == 64 else 1)
            n64 = sum(1 for (c, _) in idxs if wlen[c] == 64)
            ginfo["halves"].append({
                "n": len(idxs), "n64": n64,
                "slot_of": [s for (_, s) in idxs],
            })
            perm += [c for (c, _) in idxs]
        groups.append(ginfo)
    perm = np.array(perm, dtype=np.int64)
    assert len(perm) == TOTC and len(set(perm.tolist())) == TOTC

    src_rel = src_rel[:, perm]
    ad_idx = ad_idx[:, perm]
    dst_rel = dst_rel[:, perm]
    woff = woff[perm]
    wlen = wlen[perm]

    # wrapped int16 index layout: [128, TOTIDX//16]
    def wrap(a):
        fl = a.reshape(NCORES, TOTIDX)
        w = fl.reshape(NCORES, TOTIDX // 16, 16).transpose(0, 2, 1)
        return np.tile(w, (1, 8, 1)).astype(np.int16)

    src_w = wrap(src_rel)
    ad_w = wrap(ad_idx)
    dst_col = dst_rel.transpose(0, 2, 1).astype(BF16)  # [NCORES, 128, TOTC]

    meta = {
        "cpt": cpt, "woff": woff, "wlen": wlen, "TOTC": TOTC,
        "TOTIDX": TOTIDX, "groups": groups, "G": G,
    }
    return src_w, ad_w, dst_col, meta


def _build(meta):
    import concourse.bass as bass
    import concourse.bacc as bacc
    import concourse.mybir as mybir
    import concourse.tile as tile

    f32 = mybir.dt.float32
    bf16 = mybir.dt.bfloat16
    i16 = mybir.dt.int16
    AF = mybir.ActivationFunctionType
    OP = mybir.AluOpType

    cpt, woff, wlen = meta["cpt"], meta["woff"], meta["wlen"]
    TOTC, TOTIDX = meta["TOTC"], meta["TOTIDX"]

    nc = bacc.Bacc("TRN2", target_bir_lowering=False, debug=False)

    # ---- I/O ----
    xT_sl = nc.dram_tensor("xT_sl", [NFP, NPC], bf16, kind="ExternalInput")
    idx_src = nc.dram_tensor("idx_src", [128, TOTIDX // 16], i16, kind="ExternalInput")
    idx_ad = nc.dram_tensor("idx_ad", [128, TOTIDX // 16], i16, kind="ExternalInput")
    dst_col = nc.dram_tensor("dst_col", [128, TOTC], bf16, kind="ExternalInput")
    fb_p = nc.dram_tensor("fb_p", [NFP, NMEL], bf16, kind="ExternalInput")
    Wg_d = nc.dram_tensor("Wg", [NMEL, HC], f32, kind="ExternalInput")
    attb_s = nc.dram_tensor("attb_s", [HC, 4], f32, kind="ExternalInput")
    attb_d = nc.dram_tensor("attb_d", [HC, 4], f32, kind="ExternalInput")
    bias_bc = nc.dram_tensor("bias_bc", [128, HC], bf16, kind="ExternalInput")
    W1_d = nc.dram_tensor("W1", [HC, 256], bf16, kind="ExternalInput")
    b1_d = nc.dram_tensor("b1", [128, 4], f32, kind="ExternalInput")   # [b1 | -a*b1]
    W2_d = nc.dram_tensor("W2", [256, HC], bf16, kind="ExternalInput")
    b2_d = nc.dram_tensor("b2", [128, 2], f32, kind="ExternalInput")   # [b2 | -a*b2]
    W3_d = nc.dram_tensor("W3", [HC, 10], bf16, kind="ExternalInput")
    b3_d = nc.dram_tensor("b3", [128, 2], f32, kind="ExternalInput")   # [b3 | -a*b3]
    eye_f = nc.dram_tensor("eye_f", [128, 128], f32, kind="ExternalInput")
    eye_b = nc.dram_tensor("eye_b", [128, 128], bf16, kind="ExternalInput")
    iota_d = nc.dram_tensor("iota", [128, 128], bf16, kind="ExternalInput")
    ones_d = nc.dram_tensor("ones", [128, 16], bf16, kind="ExternalInput")
    flag_d = nc.dram_tensor("flagz", [1, 16], bf16, kind="ExternalInput")
    outT = nc.dram_tensor("outT", [10, NPC], f32, kind="ExternalOutput")

    core_ids = list(range(NCORES))

    with tile.TileContext(nc) as tc:
        with (
            tc.tile_pool(name="dram", bufs=1, space="DRAM") as dpool,
            tc.tile_pool(name="const", bufs=1) as cpool,
        ):
            if _KDW == 1:
                # one shared table in plain node order; barrier flag separate
                Hfull = dpool.tile([NPAD, RDX], bf16, addr_space="Shared")
                Bar = dpool.tile([8, 16], bf16, addr_space="Shared")
                flag_loc = dpool.tile([1, 16], bf16)
            elif _KDW == 2:
                Hext_loc = dpool.tile([NPC, RDX], bf16)
                Hfull = dpool.tile([NPAD, RDX], bf16, addr_space="Shared")
            else:
                Hext_loc = dpool.tile([NPC, RDX], bf16)
                Hfull_a = dpool.tile([8 * QSR[1], RDX], bf16, addr_space="Shared")
                Hfull_b = dpool.tile([8 * (NPC - QSR[1]), RDX], bf16,
                                     addr_space="Shared")
            adrep = dpool.tile([NPC, 128], bf16)

            # ---- constants to SBUF ----
            fb_t = cpool.tile([128, 5, NMEL], bf16)
            nc.sync.dma_start(fb_t[:], fb_p.rearrange("(b p) m -> p b m", p=128))
            Wg_t = cpool.tile([128, HC], f32)
            nc.sync.dma_start(Wg_t[:], Wg_d[:])
            atts_t = cpool.tile([128, 4], f32)
            nc.sync.dma_start(atts_t[:], attb_s[:])
            attd_t = cpool.tile([128, 4], f32)
            nc.sync.dma_start(attd_t[:], attb_d[:])
            bias_t = cpool.tile([128, HC], bf16)
            nc.sync.dma_start(bias_t[:], bias_bc[:])
            W1_t = cpool.tile([128, 256], bf16)
            nc.sync.dma_start(W1_t[:], W1_d[:])
            b1_t = cpool.tile([128, 4], f32)
            nc.sync.dma_start(b1_t[:], b1_d[:])
            W2_t = cpool.tile([128, 2, HC], bf16)
            nc.sync.dma_start(W2_t[:], W2_d.rearrange("(b p) m -> p b m", p=128))
            b2_t = cpool.tile([128, 2], f32)
            nc.sync.dma_start(b2_t[:], b2_d[:])
            W3_t = cpool.tile([128, 10], bf16)
            nc.sync.dma_start(W3_t[:], W3_d[:])
            b3_t = cpool.tile([128, 2], f32)
            nc.sync.dma_start(b3_t[:], b3_d[:])
            eyef_t = cpool.tile([128, 128], f32)
            nc.sync.dma_start(eyef_t[:], eye_f[:])
            eyeb_t = cpool.tile([128, 128], bf16)
            nc.sync.dma_start(eyeb_t[:], eye_b[:])
            iota_t = cpool.tile([128, 128], bf16)
            nc.sync.dma_start(iota_t[:], iota_d[:])
            ones_t = cpool.tile([128, 16], bf16)
            nc.sync.dma_start(ones_t[:], ones_d[:])
            isrc_t = cpool.tile([128, TOTIDX // 16], i16)
            nc.sync.dma_start(isrc_t[:], idx_src[:])
            iad_t = cpool.tile([128, TOTIDX // 16], i16)
            nc.sync.dma_start(iad_t[:], idx_ad[:])
            dcol_t = cpool.tile([128, TOTC], bf16)
            nc.sync.dma_start(dcol_t[:], dst_col[:])

            # Wgaug [mel 128, 136] bf16 = [Wg | Wg@att_s | Wg@att_d]
            Wgaug_t = cpool.tile([128, 136], bf16)
            with tc.tile_pool(name="cpsum", bufs=1, space="PSUM") as cpsum:
                WgT_ps = cpsum.tile([128, 128], f32)
                nc.tensor.transpose(WgT_ps[:], Wg_t[:], eyef_t[:])
                WgT_t = cpool.tile([128, 128], f32)
                nc.vector.tensor_copy(WgT_t[:], WgT_ps[:])
                Wgatt_ps = cpsum.tile([128, 8], f32)
                nc.tensor.matmul(Wgatt_ps[:, 0:4], WgT_t[:], atts_t[:])
                nc.tensor.matmul(Wgatt_ps[:, 4:8], WgT_t[:], attd_t[:])
                nc.vector.tensor_copy(Wgaug_t[:, 0:128], Wg_t[:])
                nc.vector.tensor_copy(Wgaug_t[:, 128:136], Wgatt_ps[:])

            # ================= stage A =================
            bar = None
            rows_sb = cpool.tile([128, TPC, 136], bf16, name="rows_sb") if _KDW == 1 else None
            with (
                tc.tile_pool(name="sa_sb", bufs=2) as sa,
                tc.tile_pool(name="sa_ps", bufs=2, space="PSUM") as saps,
                tc.tile_pool(name="sa_ps1", bufs=2, space="PSUM") as saps1,
            ):
                QEND = {24: 0, 49: 1}
                for g0 in range(0, TPC, 4):
                    gsz = min(4, TPC - g0)
                    gn = gsz * NT
                    h1T_ps = saps.tile([128, 512], f32, tag="h1T")
                    for b in range(5):
                        xtb = sa.tile([128, 512], bf16, tag="xtb", bufs=6)
                        nc.sync.dma_start(
                            xtb[:, 0:gn],
                            xT_sl[b * 128:(b + 1) * 128,
                                  g0 * NT:g0 * NT + gn])
                        nc.tensor.matmul(
                            h1T_ps[:, 0:gn], fb_t[:, b, :], xtb[:, 0:gn],
                            start=(b == 0), stop=(b == 4))
                    h1T = sa.tile([128, 512], bf16, tag="h1Ts")
                    nc.scalar.activation(h1T[:, 0:gn], h1T_ps[:, 0:gn], AF.Copy)
                    for u in range(gsz):
                        s = g0 + u
                        h_ps = saps1.tile([128, 136], f32, tag="hps")
                        nc.tensor.matmul(
                            h_ps[:], h1T[:, u * NT:(u + 1) * NT], Wgaug_t[:])
                        if _KDW == 1:
                            hrow = rows_sb[:, s, :]
                            nc.scalar.activation(hrow, h_ps[:], AF.Copy)
                            hoff = rows_sb.offset + s * 136
                        else:
                            hrow_t = sa.tile([128, 136], bf16, tag="hrow")
                            hrow = hrow_t[:]
                            nc.scalar.activation(hrow, h_ps[:], AF.Copy)
                            hoff = hrow_t.offset
                            nc.sync.dma_start(
                                Hext_loc[s * NT:(s + 1) * NT, 0:132],
                                hrow_t[:, 0:132])
                        adr = sa.tile([128, 128], bf16, tag="adr")
                        nc.vector.tensor_copy(
                            adr[:].rearrange("p (a b) -> p a b", a=32, b=4),
                            bass.AP(hrow.tensor, hoff + 132,
                                    [hrow.ap[0], [0, 32], [1, 4]]))
                        nc.sync.dma_start(adrep[s * NT:(s + 1) * NT, :], adr[:])
                    if _KDW == 0 and (g0 + gsz) in QEND:
                        q = QEND[g0 + gsz]
                        hf_out = Hfull_a if q == 0 else Hfull_b
                        nc.gpsimd.collective_compute(
                            "AllGather", mybir.AluOpType.bypass,
                            ins=[Hext_loc[QSR[q]:QSR[q + 1], :]],
                            outs=[hf_out[:]],
                            replica_groups=[core_ids])
                if _KDW == 2:
                    nc.gpsimd.collective_compute(
                        "AllGather", mybir.AluOpType.bypass,
                        ins=[Hext_loc[:]],
                        outs=[Hfull[:]],
                        replica_groups=[core_ids])
                if _KDW == 1:
                    import concourse.bass as _b
                    fz = sa.tile([1, 16], bf16, tag="fz")
                    nc.vector.memset(fz[:], 0.0)
                    nc.sync.dma_start(flag_loc[:, :], fz[:])
                    rk = nc.sync.partition_id()
                    rk_off = rk * (NPC * RDX)
                    # single write of the whole slice into the shared table
                    w = nc.sync.dma_start(
                        bass.AP(Hfull.tensor, rk_off + Hfull.offset,
                                [[RDX, NT], [NT * RDX, TPC], [1, 132]]),
                        rows_sb[:, :, 0:132])
                    bar = nc.gpsimd.collective_compute(
                        "AllGather", mybir.AluOpType.bypass,
                        ins=[flag_loc[0:1, 0:16]],
                        outs=[Bar[:, :]],
                        replica_groups=[core_ids])
                    _b._add_dep_helper(bar.ins, w.ins, sync=True,
                                       reason="barrier after shared write")

            # compact per-edge a_d staging: [128, TOTC, 4] bf16 (~8KB/prt)
            adall = cpool.tile([128, TOTC, 4], bf16, name="adall")

            # gather table views
            if _KDW >= 1:
                TA = Hfull[0:NPAD, :]
                TB = Hfull[SPLIT:NPAD, :]
            else:
                TA = Hfull_a[:]
                TB = Hfull_b[:]

            # ================= edge phase + MLP =================
            groups = meta["groups"]
            gstarts = []
            p = 0
            for gi in groups:
                gstarts.append(p)
                p += gi["halves"][0]["n"] + gi["halves"][1]["n"]
            assert p == TOTC
            GA_MAX = max(gi["halves"][0]["n"] for gi in groups)
            GB_MAX = max(gi["halves"][1]["n"] for gi in groups)
            TOTG_MAX = max(gi["halves"][0]["n"] + gi["halves"][1]["n"]
                           for gi in groups)

            with (
                tc.tile_pool(name="eg_g", bufs=3) as egg,
                tc.tile_pool(name="eg_sb", bufs=3) as egs,
                tc.tile_pool(name="eg_acc", bufs=3, space="PSUM") as egacc,
                tc.tile_pool(name="eg_tp", bufs=2, space="PSUM") as egtp,
                tc.tile_pool(name="mlp_sb", bufs=2) as msb,
                tc.tile_pool(name="mlp_ps", bufs=1, space="PSUM") as mps,
            ):
                # --- a_d prefetch: runs on DMA engines during the AllGather
                # (adrep slices are ready as soon as stage A passes the slot;
                # compact values land in the persistent adall tile) ---
                for ginfo, gstart in zip(groups, gstarts):
                    g0 = ginfo["slots"][0]
                    gext = len(ginfo["slots"]) * NT
                    tot_g = ginfo["halves"][0]["n"] + ginfo["halves"][1]["n"]
                    adp = egg.tile([128, TOTG_MAX, 128], bf16, tag="adp",
                                   bufs=3)
                    nc.gpsimd.dma_gather(
                        adp[:, 0:tot_g, :], adrep[g0 * NT:g0 * NT + gext, :],
                        iad_t[:, gstart * 8:(gstart + tot_g) * 8],
                        num_idxs=tot_g * NT, num_idxs_reg=tot_g * NT,
                        elem_size=128, single_packet=False)
                    nc.scalar.activation(
                        adall[:, gstart:gstart + tot_g, :],
                        adp[:, 0:tot_g, 0:4], AF.Copy)

                actT4 = None
                gsz = 4
                for ginfo, gstart in zip(groups, gstarts):
                    sl = ginfo["slots"]
                    hA, hB = ginfo["halves"]
                    nA, nB = hA["n"], hB["n"]
                    tot_g = nA + nB
                    # last (half, chunk-in-half) per slot for the stop flag
                    last_of = {}
                    for hf, hh in ((0, hA), (1, hB)):
                        for j, s in enumerate(hh["slot_of"]):
                            last_of[s] = (hf, j)

                    accs = {}
                    for s in sl:
                        acc = egacc.tile([128, 132], f32, tag="acc",
                                         name=f"acc_s{s % 2}")
                        nc.vector.memset(acc[:], 0.0)
                        accs[s] = acc

                    ad = adall[:, gstart:gstart + tot_g, :]

                    gA = egg.tile([128, GA_MAX, RDX], bf16, tag="gA")
                    giA = nc.gpsimd.dma_gather(
                        gA[:, 0:nA, :], TA,
                        isrc_t[:, gstart * 8:(gstart + nA) * 8],
                        num_idxs=nA * NT, num_idxs_reg=nA * NT,
                        elem_size=RDX, single_packet=False)
                    gB = egg.tile([128, GB_MAX, RDX], bf16, tag="gB")
                    giB = nc.gpsimd.dma_gather(
                        gB[:, 0:nB, :], TB,
                        isrc_t[:, (gstart + nA) * 8:(gstart + tot_g) * 8],
                        num_idxs=nB * NT, num_idxs_reg=nB * NT,
                        elem_size=RDX, single_packet=False)
                    if bar is not None:
                        import concourse.bass as _b
                        for gi_ in (giA, giB):
                            if gi_ is not None:
                                _b._add_dep_helper(
                                    gi_.ins, bar.ins, sync=True,
                                    reason="gather after shared-table barrier")

                    ind = egs.tile([128, TOTG_MAX, 128], bf16, tag="ind")
                    for (gt, hh, c0) in ((gA, hA, 0), (gB, hB, nA)):
                        nh = hh["n"]
                        if nh == 0:
                            continue
                        # t = a_s + a_d ; lrelu = max(t, 0.2t) ; ex = exp
                        tt = egs.tile([128, TOTG_MAX, 4], bf16, tag="tt",
                                      bufs=2)
                        nc.vector.tensor_tensor(
                            tt[:, 0:nh, :], gt[:, 0:nh, 128:132],
                            ad[:, c0:c0 + nh, :], OP.add)
                        t2 = egs.tile([128, TOTG_MAX, 4], bf16, tag="t2",
                                      bufs=2)
                        nc.vector.tensor_scalar(
                            t2[:, 0:nh, :], tt[:, 0:nh, :], NEG_ATT, None,
                            OP.mult)
                        nc.vector.tensor_tensor(
                            tt[:, 0:nh, :], tt[:, 0:nh, :], t2[:, 0:nh, :],
                            OP.max)
                        nc.scalar.activation(
                            tt[:, 0:nh, :], tt[:, 0:nh, :], AF.Exp)
                        # msg *= ex; rows are head-interleaved [32ch x 4h]
                        # so every operand's last dim is packed (DVE 2x mode)
                        g4 = bass.AP(
                            gt.tensor, gt.offset,
                            [gt.ap[0], [RDX, nh], [4, 32], [1, 4]])
                        exb = bass.AP(
                            tt.tensor, tt.offset,
                            [tt.ap[0], [4, nh], [0, 32], [1, 4]])
                        nc.vector.tensor_tensor(g4, g4, exb, OP.mult)
                        # ex -> row cols 128:132 (Activation engine copy)
                        nc.scalar.activation(
                            gt[:, 0:nh, 128:132], tt[:, 0:nh, :], AF.Copy)
                        # indicator, width-aware (wl=64 chunks ordered first)
                        n64 = hh["n64"]
                        for lo, ncnt, w in ((0, n64, 64), (n64, nh - n64, 128)):
                            if ncnt == 0:
                                continue
                            iob = bass.AP(
                                iota_t.tensor, iota_t.offset,
                                [iota_t.ap[0], [0, ncnt], [1, w]])
                            dcb = bass.AP(
                                dcol_t.tensor,
                                dcol_t.offset + gstart + c0 + lo,
                                [dcol_t.ap[0], [1, ncnt], [0, w]])
                            io = bass.AP(
                                ind.tensor, ind.offset + (c0 + lo) * 128,
                                [ind.ap[0], [128, ncnt], [1, w]])
                            nc.vector.tensor_tensor(io, iob, dcb, OP.is_equal)
                        for j in range(nh):
                            s = hh["slot_of"][j]
                            cglob = gstart + c0 + j
                            wo = int(woff[cglob])
                            wl = int(wlen[cglob])
                            nc.tensor.matmul(
                                accs[s][wo:wo + wl, :],
                                ind[:, c0 + j, 0:wl], gt[:, j, 0:132],
                                start=False,
                                stop=(last_of[s] == ((0 if c0 == 0 else 1), j)),
                                skip_group_check=True)

                    for s in sl:
                        acc = accs[s]
                        # normalize + bias + ELU (node-major)
                        dinv = egs.tile([128, 4], f32, tag="dinv")
                        nc.vector.tensor_scalar(
                            dinv[:], acc[:, 128:132], 1e-12, None, OP.add)
                        nc.vector.reciprocal(dinv[:], dinv[:])
                        gat = egs.tile([128, 128], bf16, tag="gat")
                        ga = bass.AP(gat.tensor, gat.offset,
                                     [gat.ap[0], [4, 32], [1, 4]])
                        aa = bass.AP(acc.tensor, acc.offset,
                                     [acc.ap[0], [4, 32], [1, 4]])
                        db = bass.AP(dinv.tensor, dinv.offset,
                                     [dinv.ap[0], [0, 32], [1, 4]])
                        nc.vector.tensor_tensor(ga, aa, db, OP.mult)
                        nc.vector.tensor_tensor(gat[:], gat[:], bias_t[:],
                                                OP.add)
                        # ELU = relu(x) - relu(1 - exp(x))
                        t1 = egs.tile([128, 128], bf16, tag="t1")
                        nc.scalar.activation(t1[:], gat[:], AF.Exp)
                        nc.scalar.activation(t1[:], t1[:], AF.Relu, scale=-1.0,
                                             bias=1.0)
                        nc.scalar.activation(gat[:], gat[:], AF.Relu)
                        nc.vector.tensor_sub(gat[:], gat[:], t1[:])
                        # transpose -> actT4
                        sub = s % 4
                        if sub == 0:
                            gsz = min(4, TPC - s)
                            actT4 = msb.tile([128, 4 * NT], bf16, tag="actT4")
                        tp = egtp.tile([128, 128], bf16, tag="tp2")
                        nc.tensor.transpose(tp[:], gat[:], eyeb_t[:])
                        nc.vector.tensor_copy(
                            actT4[:, sub * NT:(sub + 1) * NT], tp[:])
                        self_mlp = (sub == gsz - 1)
                        if self_mlp:
                        g0 = s - sub
                        gn = gsz * NT
                        # L1: lrelu(x+b) = relu(x+b) - relu(-a*x - a*b)
                        a1 = msb.tile([128, 2, 512], bf16, tag="a1")
                        r2 = msb.tile([128, 512], bf16, tag="r2")
                        for j in range(2):
                            o1 = mps.tile([128, 512], f32, tag="o1")
                            nc.tensor.matmul(
                                o1[:, 0:gn], W1_t[:, j * 128:(j + 1) * 128],
                                actT4[:, 0:gn])
                            nc.scalar.activation(
                                a1[:, j, 0:gn], o1[:, 0:gn], AF.Relu,
                                bias=b1_t[:, j:j + 1])
                            nc.scalar.activation(
                                r2[:, 0:gn], o1[:, 0:gn], AF.Relu,
                                scale=-NEG_MLP, bias=b1_t[:, 2 + j:3 + j])
                            nc.vector.tensor_sub(
                                a1[:, j, 0:gn], a1[:, j, 0:gn], r2[:, 0:gn])
                        o2 = mps.tile([128, 512], f32, tag="o2")
                        for j in range(2):
                            nc.tensor.matmul(
                                o2[:, 0:gn], W2_t[:, j, :], a1[:, j, 0:gn],
                                start=(j == 0), stop=(j == 1))
                        a2 = msb.tile([128, 512], bf16, tag="a2")
                        r2b = msb.tile([128, 512], bf16, tag="r2b")
                        nc.scalar.activation(
                            a2[:, 0:gn], o2[:, 0:gn], AF.Relu,
                            bias=b2_t[:, 0:1])
                        nc.scalar.activation(
                            r2b[:, 0:gn], o2[:, 0:gn], AF.Relu,
                            scale=-NEG_MLP, bias=b2_t[:, 1:2])
                        nc.vector.tensor_sub(
                            a2[:, 0:gn], a2[:, 0:gn], r2b[:, 0:gn])
                        o3 = mps.tile([16, 512], f32, tag="sm", name="o3_t")
                        nc.tensor.matmul(o3[0:10, 0:gn], W3_t[:], a2[:, 0:gn])
                        z = msb.tile([16, 512], bf16, tag="z")
                        zr = msb.tile([16, 512], bf16, tag="zr")
                        nc.scalar.activation(
                            z[0:10, 0:gn], o3[0:10, 0:gn], AF.Relu,
                            bias=b3_t[0:10, 0:1])
                        nc.scalar.activation(
                            zr[0:10, 0:gn], o3[0:10, 0:gn], AF.Relu,
                            scale=-NEG_MLP, bias=b3_t[0:10, 1:2])
                        nc.vector.tensor_sub(
                            z[0:10, 0:gn], z[0:10, 0:gn], zr[0:10, 0:gn])
                        nc.scalar.activation(z[0:10, 0:gn], z[0:10, 0:gn],
                                             AF.Exp)
                        ssum = mps.tile([16, 512], f32, tag="sm",
                                        name="ssum_t")[0:1, :]
                        nc.tensor.matmul(
                            ssum[:, 0:gn], ones_t[0:10, 0:1], z[0:10, 0:gn])
                        sinv = msb.tile([1, 512], bf16, tag="sinv")
                        with nc.allow_low_precision(reason="softmax recip"):
                            nc.vector.reciprocal(sinv[:, 0:gn], ssum[:, 0:gn])
                        sx = mps.tile([16, 512], f32, tag="sm", name="sx_t")
                        nc.tensor.matmul(
                            sx[0:10, 0:gn], ones_t[0:1, 0:10], sinv[:, 0:gn])
                        res = msb.tile([16, 512], f32, tag="res")
                        nc.vector.tensor_mul(
                            res[0:10, 0:gn], z[0:10, 0:gn], sx[0:10, 0:gn])
                        nc.sync.dma_start(
                            outT[:, g0 * NT:g0 * NT + gn], res[0:10, 0:gn])

    nc.compile()
    return nc


def _inputs_per_core(inputs, src_w, ad_w, dst_col, meta):
    x = np.asarray(inputs["x"], dtype=np.float32)
    fb = np.asarray(inputs["fb"], dtype=np.float32)
    Wg = np.asarray(inputs["Wg"], dtype=np.float32)
    bias_g = np.asarray(inputs["bias_g"], dtype=np.float32)
    att_src = np.asarray(inputs["att_src"], dtype=np.float32)
    att_dst = np.asarray(inputs["att_dst"], dtype=np.float32)
    W1 = np.asarray(inputs["W1"], dtype=np.float32)
    b1 = np.asarray(inputs["b1"], dtype=np.float32)
    W2 = np.asarray(inputs["W2"], dtype=np.float32)
    b2 = np.asarray(inputs["b2"], dtype=np.float32)
    W3 = np.asarray(inputs["W3"], dtype=np.float32)
    b3 = np.asarray(inputs["b3"], dtype=np.float32)

    x_pad = np.zeros((NPAD, NFP), dtype=np.float32)
    x_pad[:N, :NF] = x
    fb_pad = np.zeros((NFP, NMEL), dtype=np.float32)
    fb_pad[:NF] = fb

    att_blk_s = np.zeros((HC, 4), dtype=np.float32)
    att_blk_d = np.zeros((HC, 4), dtype=np.float32)
    for h in range(H):
        att_blk_s[h * C:(h + 1) * C, h] = att_src[h]
        att_blk_d[h * C:(h + 1) * C, h] = att_dst[h]

    # head-interleaved feature order: new col j = old col (j%4)*32 + j//4
    perm_il = np.array([(j % 4) * 32 + j // 4 for j in range(HC)])
    Wg = np.ascontiguousarray(Wg[:, perm_il])
    att_blk_s = np.ascontiguousarray(att_blk_s[perm_il])
    att_blk_d = np.ascontiguousarray(att_blk_d[perm_il])
    bias_g = bias_g[perm_il]
    W1 = np.ascontiguousarray(W1[perm_il, :])

    b1p = np.zeros((128, 4), dtype=np.float32)
    b1p[:, 0] = b1[:128]
    b1p[:, 1] = b1[128:]
    b1p[:, 2:4] = -NEG_MLP * b1p[:, 0:2]
    b2p = np.zeros((128, 2), dtype=np.float32)
    b2p[:, 0] = b2
    b2p[:, 1] = -NEG_MLP * b2
    b3p = np.zeros((128, 2), dtype=np.float32)
    b3p[:10, 0] = b3
    b3p[:10, 1] = -NEG_MLP * b3

    common = {
        "fb_p": fb_pad.astype(BF16), "Wg": Wg,
        "attb_s": att_blk_s, "attb_d": att_blk_d,
        "bias_bc": np.tile(bias_g[None, :], (128, 1)).astype(BF16),
        "W1": W1.astype(BF16), "b1": b1p,
        "W2": W2.astype(BF16), "b2": b2p,
        "W3": W3.astype(BF16), "b3": b3p,
        "eye_f": np.eye(128, dtype=np.float32),
        "eye_b": np.eye(128).astype(BF16),
        "iota": np.tile(np.arange(128, dtype=np.float32)[None, :],
                        (128, 1)).astype(BF16),
        "ones": np.ones((128, 16)).astype(BF16),
        "flagz": np.zeros((1, 16)).astype(BF16),
    }
    xT_pad = np.ascontiguousarray(x_pad.T.astype(BF16))  # [640, NPAD]
    maps = []
    for k in range(NCORES):
        m = dict(common)
        m["xT_sl"] = np.ascontiguousarray(xT_pad[:, k * NPC:(k + 1) * NPC])
        m["idx_src"] = src_w[k]
        m["idx_ad"] = ad_w[k]
        m["dst_col"] = dst_col[k]
        maps.append(m)
    return maps


def kernel(**inputs):
    from concourse.bass_utils import run_bass_kernel_spmd

    src_w, ad_w, dst_col, meta = _prep(inputs["edge_index"])
    key = ("nc", meta["TOTC"], tuple(meta["cpt"].reshape(-1)),
           tuple(meta["woff"]))
    if key not in _CACHE:
        _CACHE.clear()
        _CACHE[key] = _build(meta)
    nc = _CACHE[key]
    maps = _inputs_per_core(inputs, src_w, ad_w, dst_col, meta)
    res = run_bass_kernel_spmd(nc, maps, core_ids=list(range(NCORES)))
    out = np.zeros((NPAD, 10), dtype=np.float32)
    for k in range(NCORES):
        out[k * NPC:(k + 1) * NPC] = res.results[k]["outT"].T
    return out[:N]


# revision 6
# speedup vs baseline: 1.2180x; 1.1110x over previous
"""GAT (gnn_message_passing) Trainium2 Bass kernel — 8-core SPMD, v2.

Contract: kernel(**inputs) -> np.ndarray with FULL inputs / FULL output.
Self-contained: hardcodes shapes; only imports the container's concourse stack.

v2 design vs v1:
  - bf16 edge path: shared table rows are 256x bf16 (512B), scatter matmuls,
    indicator and row-scaling all bf16.
  - No Lrelu on the Activation engine (no act-table reloads): attention
    leaky-relu is max(t, 0.2t) on DVE; MLP leaky-relu is
    relu(x+b) - relu(-a*x - a*b) via two Relu activations + one DVE subtract.
  - Stage A consumes host-transposed x (no PE transposes) and emits node-major
    rows [h | a_s | a_d] with one matmul against an augmented [Wg|Wg@as|Wg@ad].
  - KDW=1: each core writes its row slice straight into the shared DRAM table
    at a partition_id()-based dynamic offset, then a tiny AllGather acts as a
    barrier. KDW=0 falls back to two real bf16 AllGathers.
"""
import sys

for _p in ("/opt/trn_rl_repo", "/root/.axon_site/_ro/trn_rl_repo"):
    if _p not in sys.path:
        sys.path.append(_p)

import os
import numpy as np
import ml_dtypes

BF16 = ml_dtypes.bfloat16
# KDW modes: 0 = two half-table AllGathers (quarter row scheme),
#            1 = direct shared write + barrier (broken: scratchpad is only
#                pair-shared, kept for reference),
#            2 = ONE fat AllGather of the full 512B-pitch table, plain order
_KDW = int(os.environ.get("KDW", "2"))

# ---------------- problem constants (hardcoded per contract) ----------------
N = 50000
NF = 513
NFP = 640            # padded feature dim (5 * 128)
NMEL = 128
H, C = 4, 32
HC = H * C           # 128
E = 800000
NEG_ATT = 0.2
NEG_MLP = 0.01

NCORES = 8
TPC = 49             # tiles per core
NT = 128             # nodes per tile
NPC = TPC * NT       # 6272 nodes per core
NPAD = NCORES * NPC  # 50176
RDX = 256            # table row pitch in bf16 elems (512 B)
SPLIT = 4 * NPC      # 25088: table A/B row split (int16 idx headroom)
QSR = (0, 3072, NPC)  # KDW=0 quarter split (rows per AllGather region)

_CACHE = {}


def _prep(edge_index):
    """Host-side edge preprocessing. Returns per-core index/metadata arrays."""
    src = np.asarray(edge_index[0], dtype=np.int64)
    dst = np.asarray(edge_index[1], dtype=np.int64)
    loop = np.arange(N, dtype=np.int64)
    src = np.concatenate([src, loop])
    dst = np.concatenate([dst, loop])

    tile_g = dst // NT                # global tile id 0..391
    if _KDW >= 1:
        half = (src >= SPLIT).astype(np.int64)
        src_row = src - half * SPLIT
    else:
        r, l = src // NPC, src % NPC
        half = (l >= QSR[1]).astype(np.int64)
        src_row = np.where(half == 1,
                           r * (NPC - QSR[1]) + (l - QSR[1]),
                           r * QSR[1] + l)
    order = np.lexsort((src, dst, half, tile_g))
    src_row, dst, tile_g, half = (src_row[order], dst[order], tile_g[order],
                                  half[order])

    NTILES_G = NPAD // NT            # 392
    cnt = np.zeros((NTILES_G, 2), dtype=np.int64)
    np.add.at(cnt, (tile_g, half), 1)
    starts = np.zeros((NTILES_G, 2), dtype=np.int64)
    starts.reshape(-1)[1:] = np.cumsum(cnt.reshape(-1))[:-1]

    # chunks per (slot, half): max over cores
    cores = np.arange(NCORES)
    cpt = np.zeros((TPC, 2), dtype=np.int64)
    for s in range(TPC):
        t_ids = cores * TPC + s
        for hf in range(2):
            cpt[s, hf] = max(1, int(np.ceil(cnt[t_ids, hf].max() / NT)))
    TOTC = int(cpt.sum())
    TOTIDX = TOTC * NT

    src_rel = np.zeros((NCORES, TOTC, NT), dtype=np.int64)
    ad_idx = np.zeros((NCORES, TOTC, NT), dtype=np.int64)
    dst_rel = np.full((NCORES, TOTC, NT), 999.0, dtype=np.float32)
    dloc_all = np.zeros((NCORES, TOTC, NT), dtype=np.int64)
    valid = np.zeros((NCORES, TOTC, NT), dtype=bool)

    for k in range(NCORES):
        coff = 0
        for s in range(TPC):
            t = k * TPC + s
            for hf in range(2):
                nch = int(cpt[s, hf])
                st, cn = starts[t, hf], int(cnt[t, hf])
                src_rel[k, coff:coff + nch].reshape(-1)[:cn] = src_row[st:st + cn]
                ad_idx[k, coff:coff + nch].reshape(-1)[:cn] = (
                    dst[st:st + cn] % NPC - (s // 2) * 2 * NT)
                dloc_all[k, coff:coff + nch].reshape(-1)[:cn] = dst[st:st + cn] % NT
                valid[k, coff:coff + nch].reshape(-1)[:cn] = True
                coff += nch
        assert coff == TOTC

    assert src_rel.min() >= 0 and src_rel.max() <= 32767

    # per-chunk dst windows: 64-wide when the cross-core span fits, else 128
    woff = np.zeros(TOTC, dtype=np.int64)
    wlen = np.full(TOTC, 128, dtype=np.int64)
    for c in range(TOTC):
        v = valid[:, c, :]
        if v.any():
            dl = dloc_all[:, c, :][v]
            lo, hi = int(dl.min()), int(dl.max())
            wo = 0 if lo < 64 else 64
            if hi < wo + 64:
                woff[c] = wo
                wlen[c] = 64

    for k in range(NCORES):
        dr = dloc_all[k] - woff[:, None]
        dst_rel[k][valid[k]] = dr[valid[k]].astype(np.float32)

    # ---- regroup chunks: G slots per gather group, per (group, half) with
    # wl=64 chunks first so the indicator op can run width-aware ----
    G = 2
    coffs0 = np.concatenate([[0], np.cumsum(cpt.sum(axis=1))]).astype(int)
    perm = []          # new order -> original chunk index
    groups = []        # per group: dict
    for g0 in range(0, TPC, G):
        sl = [s for s in range(g0, min(g0 + G, TPC))]
        ginfo = {"slots": sl, "halves": []}
        for hf in range(2):
            idxs = []
            for s in sl:
                base = coffs0[s] + (0 if hf == 0 else int(cpt[s, 0]))
                idxs += [(base + j, s) for j in range(int(cpt[s, hf]))]
            idxs.sort(key=lambda t: 0 if wlen[t[0]] == 64 else 1)
            n64 = sum(1 for (c, _) in idxs if wlen[c] == 64)
            ginfo["halves"].append({
                "n": len(idxs), "n64": n64,
                "slot_of": [s for (_, s) in idxs],
            })
            perm += [c for (c, _) in idxs]
        groups.append(ginfo)
    perm = np.array(perm, dtype=np.int64)
    assert len(perm) == TOTC and len(set(perm.tolist())) == TOTC

    src_rel = src_rel[:, perm]
    ad_idx = ad_idx[:, perm]
    dst_rel = dst_rel[:, perm]
    woff = woff[perm]
    wlen = wlen[perm]

    # wrapped int16 index layout: [128, TOTIDX//16]
    def wrap(a):
        fl = a.reshape(NCORES, TOTIDX)
        w = fl.reshape(NCORES, TOTIDX // 16, 16).transpose(0, 2, 1)
        return np.tile(w, (1, 8, 1)).astype(np.int16)

    src_w = wrap(src_rel)
    ad_w = wrap(ad_idx)
    dst_col = dst_rel.transpose(0, 2, 1).astype(BF16)  # [NCORES, 128, TOTC]

    meta = {
        "cpt": cpt, "woff": woff, "wlen": wlen, "TOTC": TOTC,
        "TOTIDX": TOTIDX, "groups": groups, "G": G,
    }
    return src_w, ad_w, dst_col, meta


def _build(meta):
    import concourse.bass as bass
    import concourse.bacc as bacc
    import concourse.mybir as mybir
    import concourse.tile as tile

    f32 = mybir.dt.float32
    bf16 = mybir.dt.bfloat16
    i16 = mybir.dt.int16
    AF = mybir.ActivationFunctionType
    OP = mybir.AluOpType

    cpt, woff, wlen = meta["cpt"], meta["woff"], meta["wlen"]
    TOTC, TOTIDX = meta["TOTC"], meta["TOTIDX"]

    nc = bacc.Bacc("TRN2", target_bir_lowering=False, debug=False)

    # ---- I/O ----
    xT_sl = nc.dram_tensor("xT_sl", [NFP, NPC], bf16, kind="ExternalInput")
    idx_src = nc.dram_tensor("idx_src", [128, TOTIDX // 16], i16, kind="ExternalInput")
    idx_ad = nc.dram_tensor("idx_ad", [128, TOTIDX // 16], i16, kind="ExternalInput")
    dst_col = nc.dram_tensor("dst_col", [128, TOTC], bf16, kind="ExternalInput")
    fb_p = nc.dram_tensor("fb_p", [NFP, NMEL], bf16, kind="ExternalInput")
    Wg_d = nc.dram_tensor("Wg", [NMEL, HC], f32, kind="ExternalInput")
    attb_s = nc.dram_tensor("attb_s", [HC, 4], f32, kind="ExternalInput")
    attb_d = nc.dram_tensor("attb_d", [HC, 4], f32, kind="ExternalInput")
    bias_bc = nc.dram_tensor("bias_bc", [128, HC], bf16, kind="ExternalInput")
    W1_d = nc.dram_tensor("W1", [HC, 256], bf16, kind="ExternalInput")
    b1_d = nc.dram_tensor("b1", [128, 4], f32, kind="ExternalInput")   # [b1 | -a*b1]
    W2_d = nc.dram_tensor("W2", [256, HC], bf16, kind="ExternalInput")
    b2_d = nc.dram_tensor("b2", [128, 2], f32, kind="ExternalInput")   # [b2 | -a*b2]
    W3_d = nc.dram_tensor("W3", [HC, 10], bf16, kind="ExternalInput")
    b3_d = nc.dram_tensor("b3", [128, 2], f32, kind="ExternalInput")   # [b3 | -a*b3]
    eye_f = nc.dram_tensor("eye_f", [128, 128], f32, kind="ExternalInput")
    eye_b = nc.dram_tensor("eye_b", [128, 128], bf16, kind="ExternalInput")
    iota_d = nc.dram_tensor("iota", [128, 128], bf16, kind="ExternalInput")
    ones_d = nc.dram_tensor("ones", [128, 16], bf16, kind="ExternalInput")
    flag_d = nc.dram_tensor("flagz", [1, 16], bf16, kind="ExternalInput")
    outT = nc.dram_tensor("outT", [10, NPC], f32, kind="ExternalOutput")

    core_ids = list(range(NCORES))

    with tile.TileContext(nc) as tc:
        with (
            tc.tile_pool(name="dram", bufs=1, space="DRAM") as dpool,
            tc.tile_pool(name="const", bufs=1) as cpool,
        ):
            if _KDW == 1:
                # one shared table in plain node order; barrier flag separate
                Hfull = dpool.tile([NPAD, RDX], bf16, addr_space="Shared")
                Bar = dpool.tile([8, 16], bf16, addr_space="Shared")
                flag_loc = dpool.tile([1, 16], bf16)
            elif _KDW == 2:
                Hext_loc = dpool.tile([NPC, RDX], bf16)
                Hfull = dpool.tile([NPAD, RDX], bf16, addr_space="Shared")
            else:
                Hext_loc = dpool.tile([NPC, RDX], bf16)
                Hfull_a = dpool.tile([8 * QSR[1], RDX], bf16, addr_space="Shared")
                Hfull_b = dpool.tile([8 * (NPC - QSR[1]), RDX], bf16,
                                     addr_space="Shared")
            adrep = dpool.tile([NPC, 128], bf16)

            # ---- constants to SBUF ----
            fb_t = cpool.tile([128, 5, NMEL], bf16)
            nc.sync.dma_start(fb_t[:], fb_p.rearrange("(b p) m -> p b m", p=128))
            Wg_t = cpool.tile([128, HC], f32)
            nc.sync.dma_start(Wg_t[:], Wg_d[:])
            atts_t = cpool.tile([128, 4], f32)
            nc.sync.dma_start(atts_t[:], attb_s[:])
            attd_t = cpool.tile([128, 4], f32)
            nc.sync.dma_start(attd_t[:], attb_d[:])
            bias_t = cpool.tile([128, HC], bf16)
            nc.sync.dma_start(bias_t[:], bias_bc[:])
            W1_t = cpool.tile([128, 256], bf16)
            nc.sync.dma_start(W1_t[:], W1_d[:])
            b1_t = cpool.tile([128, 4], f32)
            nc.sync.dma_start(b1_t[:], b1_d[:])
            W2_t = cpool.tile([128, 2, HC], bf16)
            nc.sync.dma_start(W2_t[:], W2_d.rearrange("(b p) m -> p b m", p=128))
            b2_t = cpool.tile([128, 2], f32)
            nc.sync.dma_start(b2_t[:], b2_d[:])
            W3_t = cpool.tile([128, 10], bf16)
            nc.sync.dma_start(W3_t[:], W3_d[:])
            b3_t = cpool.tile([128, 2], f32)
            nc.sync.dma_start(b3_t[:], b3_d[:])
            eyef_t = cpool.tile([128, 128], f32)
            nc.sync.dma_start(eyef_t[:], eye_f[:])
            eyeb_t = cpool.tile([128, 128], bf16)
            nc.sync.dma_start(eyeb_t[:], eye_b[:])
            iota_t = cpool.tile([128, 128], bf16)
            nc.sync.dma_start(iota_t[:], iota_d[:])
            ones_t = cpool.tile([128, 16], bf16)
            nc.sync.dma_start(ones_t[:], ones_d[:])
            isrc_t = cpool.tile([128, TOTIDX // 16], i16)
            iad_t = cpool.tile([128, TOTIDX // 16], i16)
            dcol_t = cpool.tile([128, TOTC], bf16)

            # Wgaug [mel 128, 136] bf16 = [Wg | Wg@att_s | Wg@att_d]
            Wgaug_t = cpool.tile([128, 136], bf16)
            with tc.tile_pool(name="cpsum", bufs=1, space="PSUM") as cpsum:
                WgT_ps = cpsum.tile([128, 128], f32)
                nc.tensor.transpose(WgT_ps[:], Wg_t[:], eyef_t[:])
                WgT_t = cpool.tile([128, 128], f32)
                nc.vector.tensor_copy(WgT_t[:], WgT_ps[:])
                Wgatt_ps = cpsum.tile([128, 8], f32)
                nc.tensor.matmul(Wgatt_ps[:, 0:4], WgT_t[:], atts_t[:])
                nc.tensor.matmul(Wgatt_ps[:, 4:8], WgT_t[:], attd_t[:])
                nc.vector.tensor_copy(Wgaug_t[:, 0:128], Wg_t[:])
                nc.vector.tensor_copy(Wgaug_t[:, 128:136], Wgatt_ps[:])

            # ================= stage A =================
            bar = None
            rows_sb = cpool.tile([128, TPC, 136], bf16, name="rows_sb") if _KDW == 1 else None
            with (
                tc.tile_pool(name="sa_sb", bufs=2) as sa,
                tc.tile_pool(name="sa_ps", bufs=2, space="PSUM") as saps,
                tc.tile_pool(name="sa_ps1", bufs=2, space="PSUM") as saps1,
            ):
                QEND = {24: 0, 49: 1}
                for g0 in range(0, TPC, 4):
                    gsz = min(4, TPC - g0)
                    gn = gsz * NT
                    h1T_ps = saps.tile([128, 512], f32, tag="h1T")
                    for b in range(5):
                        xtb = sa.tile([128, 512], bf16, tag="xtb", bufs=6)
                        (nc.sync if b % 2 == 0 else nc.scalar).dma_start(
                            xtb[:, 0:gn],
                            xT_sl[b * 128:(b + 1) * 128,
                                  g0 * NT:g0 * NT + gn])
                        nc.tensor.matmul(
                            h1T_ps[:, 0:gn], fb_t[:, b, :], xtb[:, 0:gn],
                            start=(b == 0), stop=(b == 4))
                    h1T = sa.tile([128, 512], bf16, tag="h1Ts")
                    nc.scalar.activation(h1T[:, 0:gn], h1T_ps[:, 0:gn], AF.Copy)
                    for u in range(gsz):
                        s = g0 + u
                        h_ps = saps1.tile([128, 136], f32, tag="hps")
                        nc.tensor.matmul(
                            h_ps[:], h1T[:, u * NT:(u + 1) * NT], Wgaug_t[:])
                        if _KDW == 1:
                            hrow = rows_sb[:, s, :]
                            nc.scalar.activation(hrow, h_ps[:], AF.Copy)
                            hoff = rows_sb.offset + s * 136
                        else:
                            hrow_t = sa.tile([128, 136], bf16, tag="hrow")
                            hrow = hrow_t[:]
                            nc.scalar.activation(hrow, h_ps[:], AF.Copy)
                            hoff = hrow_t.offset
                            nc.sync.dma_start(
                                Hext_loc[s * NT:(s + 1) * NT, 0:132],
                                hrow_t[:, 0:132])
                        adr = sa.tile([128, 128], bf16, tag="adr")
                        nc.vector.tensor_copy(
                            adr[:].rearrange("p (a b) -> p a b", a=32, b=4),
                            bass.AP(hrow.tensor, hoff + 132,
                                    [hrow.ap[0], [0, 32], [1, 4]]))
                        nc.scalar.dma_start(adrep[s * NT:(s + 1) * NT, :],
                                            adr[:])
                    if _KDW == 0 and (g0 + gsz) in QEND:
                        q = QEND[g0 + gsz]
                        hf_out = Hfull_a if q == 0 else Hfull_b
                        nc.gpsimd.collective_compute(
                            "AllGather", mybir.AluOpType.bypass,
                            ins=[Hext_loc[QSR[q]:QSR[q + 1], :]],
                            outs=[hf_out[:]],
                            replica_groups=[core_ids])
                if _KDW == 2:
                    nc.gpsimd.collective_compute(
                        "AllGather", mybir.AluOpType.bypass,
                        ins=[Hext_loc[:]],
                        outs=[Hfull[:]],
                        replica_groups=[core_ids])
                if _KDW == 1:
                    import concourse.bass as _b
                    fz = sa.tile([1, 16], bf16, tag="fz")
                    nc.vector.memset(fz[:], 0.0)
                    nc.sync.dma_start(flag_loc[:, :], fz[:])
                    rk = nc.sync.partition_id()
                    rk_off = rk * (NPC * RDX)
                    # single write of the whole slice into the shared table
                    w = nc.sync.dma_start(
                        bass.AP(Hfull.tensor, rk_off + Hfull.offset,
                                [[RDX, NT], [NT * RDX, TPC], [1, 132]]),
                        rows_sb[:, :, 0:132])
                    bar = nc.gpsimd.collective_compute(
                        "AllGather", mybir.AluOpType.bypass,
                        ins=[flag_loc[0:1, 0:16]],
                        outs=[Bar[:, :]],
                        replica_groups=[core_ids])
                    _b._add_dep_helper(bar.ins, w.ins, sync=True,
                                       reason="barrier after shared write")

            nc.sync.dma_start(isrc_t[:], idx_src[:])
            nc.sync.dma_start(iad_t[:], idx_ad[:])
            nc.sync.dma_start(dcol_t[:], dst_col[:])

            # compact per-edge a_d staging: [128, TOTC, 4] bf16 (~8KB/prt)
            adall = cpool.tile([128, TOTC, 4], bf16, name="adall")

            # gather table views
            if _KDW >= 1:
                TA = Hfull[0:NPAD, :]
                TB = Hfull[SPLIT:NPAD, :]
            else:
                TA = Hfull_a[:]
                TB = Hfull_b[:]

            # ================= edge phase + MLP =================
            groups = meta["groups"]
            gstarts = []
            p = 0
            for gi in groups:
                gstarts.append(p)
                p += gi["halves"][0]["n"] + gi["halves"][1]["n"]
            assert p == TOTC
            GA_MAX = max(gi["halves"][0]["n"] for gi in groups)
            GB_MAX = max(gi["halves"][1]["n"] for gi in groups)
            TOTG_MAX = max(gi["halves"][0]["n"] + gi["halves"][1]["n"]
                           for gi in groups)

            with (
                tc.tile_pool(name="eg_g", bufs=3) as egg,
                tc.tile_pool(name="eg_sb", bufs=3) as egs,
                tc.tile_pool(name="eg_acc", bufs=4, space="PSUM") as egacc,
                tc.tile_pool(name="eg_tp", bufs=1, space="PSUM") as egtp,
                tc.tile_pool(name="mlp_sb", bufs=2) as msb,
                tc.tile_pool(name="mlp_ps", bufs=1, space="PSUM") as mps,
            ):
                # --- a_d prefetch: runs on DMA engines during the AllGather
                # (adrep slices are ready as soon as stage A passes the slot;
                # compact values land in the persistent adall tile) ---
                for ginfo, gstart in zip(groups, gstarts):
                    g0 = ginfo["slots"][0]
                    gext = len(ginfo["slots"]) * NT
                    tot_g = ginfo["halves"][0]["n"] + ginfo["halves"][1]["n"]
                    adp = egg.tile([128, TOTG_MAX, 128], bf16, tag="adp",
                                   bufs=3)
                    nc.gpsimd.dma_gather(
                        adp[:, 0:tot_g, :], adrep[g0 * NT:g0 * NT + gext, :],
                        iad_t[:, gstart * 8:(gstart + tot_g) * 8],
                        num_idxs=tot_g * NT, num_idxs_reg=tot_g * NT,
                        elem_size=128, single_packet=False)
                    nc.scalar.activation(
                        adall[:, gstart:gstart + tot_g, :],
                        adp[:, 0:tot_g, 0:4], AF.Copy)

                actT4 = None
                gsz = 4
                for ginfo, gstart in zip(groups, gstarts):
                    sl = ginfo["slots"]
                    hA, hB = ginfo["halves"]
                    nA, nB = hA["n"], hB["n"]
                    tot_g = nA + nB
                    # last (half, chunk-in-half) per slot for the stop flag
                    last_of = {}
                    for hf, hh in ((0, hA), (1, hB)):
                        for j, s in enumerate(hh["slot_of"]):
                            last_of[s] = (hf, j)

                    accs = {}
                    for s in sl:
                        acc = egacc.tile([128, 132], f32, tag="acc",
                                         name=f"acc_s{s % 2}")
                        nc.vector.memset(acc[:], 0.0)
                        accs[s] = acc

                    ad = adall[:, gstart:gstart + tot_g, :]

                    gA = egg.tile([128, GA_MAX, RDX], bf16, tag="gA")
                    giA = nc.gpsimd.dma_gather(
                        gA[:, 0:nA, :], TA,
                        isrc_t[:, gstart * 8:(gstart + nA) * 8],
                        num_idxs=nA * NT, num_idxs_reg=nA * NT,
                        elem_size=RDX, single_packet=False)
                    gB = egg.tile([128, GB_MAX, RDX], bf16, tag="gB")
                    giB = nc.gpsimd.dma_gather(
                        gB[:, 0:nB, :], TB,
                        isrc_t[:, (gstart + nA) * 8:(gstart + tot_g) * 8],
                        num_idxs=nB * NT, num_idxs_reg=nB * NT,
                        elem_size=RDX, single_packet=False)
                    if bar is not None:
                        import concourse.bass as _b
                        for gi_ in (giA, giB):
                            if gi_ is not None:
                                _b._add_dep_helper(
                                    gi_.ins, bar.ins, sync=True,
                                    reason="gather after shared-table barrier")

                    ind = egs.tile([128, TOTG_MAX, 128], bf16, tag="ind")
                    for (gt, hh, c0) in ((gA, hA, 0), (gB, hB, nA)):
                        nh = hh["n"]
                        if nh == 0:
                            continue
                        # t = a_s + a_d ; lrelu = max(t, 0.2t) ; ex = exp
                        tt = egs.tile([128, TOTG_MAX, 4], bf16, tag="tt",
                                      bufs=2)
                        nc.vector.tensor_tensor(
                            tt[:, 0:nh, :], gt[:, 0:nh, 128:132],
                            ad[:, c0:c0 + nh, :], OP.add)
                        t2 = egs.tile([128, TOTG_MAX, 4], bf16, tag="t2",
                                      bufs=2)
                        nc.vector.tensor_scalar(
                            t2[:, 0:nh, :], tt[:, 0:nh, :], NEG_ATT, None,
                            OP.mult)
                        nc.vector.tensor_tensor(
                            tt[:, 0:nh, :], tt[:, 0:nh, :], t2[:, 0:nh, :],
                            OP.max)
                        nc.scalar.activation(
                            tt[:, 0:nh, :], tt[:, 0:nh, :], AF.Exp)
                        # msg *= ex; rows are head-interleaved [32ch x 4h]
                        # so every operand's last dim is packed (DVE 2x mode)
                        g4 = bass.AP(
                            gt.tensor, gt.offset,
                            [gt.ap[0], [RDX, nh], [4, 32], [1, 4]])
                        exb = bass.AP(
                            tt.tensor, tt.offset,
                            [tt.ap[0], [4, nh], [0, 32], [1, 4]])
                        nc.vector.tensor_tensor(g4, g4, exb, OP.mult)
                        # ex -> row cols 128:132 (Activation engine copy)
                        nc.scalar.activation(
                            gt[:, 0:nh, 128:132], tt[:, 0:nh, :], AF.Copy)
                        # indicator, width-aware (wl=64 chunks ordered first)
                        n64 = hh["n64"]
                        for lo, ncnt, w in ((0, n64, 64), (n64, nh - n64, 128)):
                            if ncnt == 0:
                                continue
                            iob = bass.AP(
                                iota_t.tensor, iota_t.offset,
                                [iota_t.ap[0], [0, ncnt], [1, w]])
                            dcb = bass.AP(
                                dcol_t.tensor,
                                dcol_t.offset + gstart + c0 + lo,
                                [dcol_t.ap[0], [1, ncnt], [0, w]])
                            io = bass.AP(
                                ind.tensor, ind.offset + (c0 + lo) * 128,
                                [ind.ap[0], [128, ncnt], [1, w]])
                            nc.vector.tensor_tensor(io, iob, dcb, OP.is_equal)
                        for j in range(nh):
                            s = hh["slot_of"][j]
                            cglob = gstart + c0 + j
                            wo = int(woff[cglob])
                            wl = int(wlen[cglob])
                            nc.tensor.matmul(
                                accs[s][wo:wo + wl, :],
                                ind[:, c0 + j, 0:wl], gt[:, j, 0:132],
                                start=False,
                                stop=(last_of[s] == ((0 if c0 == 0 else 1), j)),
                                skip_group_check=True)

                    for s in sl:
                        acc = accs[s]
                        # normalize + bias + ELU (node-major)
                        dinv = egs.tile([128, 4], f32, tag="dinv")
                        nc.vector.tensor_scalar(
                            dinv[:], acc[:, 128:132], 1e-12, None, OP.add)
                        nc.vector.reciprocal(dinv[:], dinv[:])
                        gat = egs.tile([128, 128], bf16, tag="gat")
                        ga = bass.AP(gat.tensor, gat.offset,
                                     [gat.ap[0], [4, 32], [1, 4]])
                        aa = bass.AP(acc.tensor, acc.offset,
                                     [acc.ap[0], [4, 32], [1, 4]])
                        db = bass.AP(dinv.tensor, dinv.offset,
                                     [dinv.ap[0], [0, 32], [1, 4]])
                        nc.vector.tensor_tensor(ga, aa, db, OP.mult)
                        nc.vector.tensor_tensor(gat[:], gat[:], bias_t[:],
                                                OP.add)
                        # ELU = relu(x) - relu(1 - exp(x))
                        t1 = egs.tile([128, 128], bf16, tag="t1")
                        nc.scalar.activation(t1[:], gat[:], AF.Exp)
                        nc.scalar.activation(t1[:], t1[:], AF.Relu, scale=-1.0,
                                             bias=1.0)
                        nc.scalar.activation(gat[:], gat[:], AF.Relu)
                        nc.vector.tensor_sub(gat[:], gat[:], t1[:])
                        # transpose -> actT4
                        sub = s % 4
                        if sub == 0:
                            gsz = min(4, TPC - s)
                            actT4 = msb.tile([128, 4 * NT], bf16, tag="actT4")
                        tp = egtp.tile([128, 128], bf16, tag="tp2")
                        nc.tensor.transpose(tp[:], gat[:], eyeb_t[:])
                        nc.vector.tensor_copy(
                            actT4[:, sub * NT:(sub + 1) * NT], tp[:])
                        self_mlp = (sub == gsz - 1)
                        if self_mlp:
                        g0 = s - sub
                        gn = gsz * NT
                        # L1: lrelu(x+b) = relu(x+b) - relu(-a*x - a*b)
                        a1 = msb.tile([128, 2, 512], bf16, tag="a1")
                        r2 = msb.tile([128, 512], bf16, tag="r2")
                        for j in range(2):
                            o1 = mps.tile([128, 512], f32, tag="o1")
                            nc.tensor.matmul(
                                o1[:, 0:gn], W1_t[:, j * 128:(j + 1) * 128],
                                actT4[:, 0:gn])
                            nc.scalar.activation(
                                a1[:, j, 0:gn], o1[:, 0:gn], AF.Relu,
                                bias=b1_t[:, j:j + 1])
                            nc.scalar.activation(
                                r2[:, 0:gn], o1[:, 0:gn], AF.Relu,
                                scale=-NEG_MLP, bias=b1_t[:, 2 + j:3 + j])
                            nc.vector.tensor_sub(
                                a1[:, j, 0:gn], a1[:, j, 0:gn], r2[:, 0:gn])
                        o2 = mps.tile([128, 512], f32, tag="o2")
                        for j in range(2):
                            nc.tensor.matmul(
                                o2[:, 0:gn], W2_t[:, j, :], a1[:, j, 0:gn],
                                start=(j == 0), stop=(j == 1))
                        a2 = msb.tile([128, 512], bf16, tag="a2")
                        r2b = msb.tile([128, 512], bf16, tag="r2b")
                        nc.scalar.activation(
                            a2[:, 0:gn], o2[:, 0:gn], AF.Relu,
                            bias=b2_t[:, 0:1])
                        nc.scalar.activation(
                            r2b[:, 0:gn], o2[:, 0:gn], AF.Relu,
                            scale=-NEG_MLP, bias=b2_t[:, 1:2])
                        nc.vector.tensor_sub(
                            a2[:, 0:gn], a2[:, 0:gn], r2b[:, 0:gn])
                        o3 = mps.tile([16, 512], f32, tag="sm", name="o3_t")
                        nc.tensor.matmul(o3[0:10, 0:gn], W3_t[:], a2[:, 0:gn])
                        z = msb.tile([16, 512], bf16, tag="z")
                        zr = msb.tile([16, 512], bf16, tag="zr")
                        nc.scalar.activation(
                            z[0:10, 0:gn], o3[0:10, 0:gn], AF.Relu,
                            bias=b3_t[0:10, 0:1])
                        nc.scalar.activation(
                            zr[0:10, 0:gn], o3[0:10, 0:gn], AF.Relu,
                            scale=-NEG_MLP, bias=b3_t[0:10, 1:2])
                        nc.vector.tensor_sub(
                            z[0:10, 0:gn], z[0:10, 0:gn], zr[0:10, 0:gn])
                        nc.scalar.activation(z[0:10, 0:gn], z[0:10, 0:gn],
                                             AF.Exp)
                        ssum = mps.tile([16, 512], f32, tag="sm",
                                        name="ssum_t")[0:1, :]
                        nc.tensor.matmul(
                            ssum[:, 0:gn], ones_t[0:10, 0:1], z[0:10, 0:gn])
                        sinv = msb.tile([1, 512], bf16, tag="sinv")
                        with nc.allow_low_precision(reason="softmax recip"):
                            nc.vector.reciprocal(sinv[:, 0:gn], ssum[:, 0:gn])
                        sx = mps.tile([16, 512], f32, tag="sm", name="sx_t")
                        nc.tensor.matmul(
                            sx[0:10, 0:gn], ones_t[0:1, 0:10], sinv[:, 0:gn])
                        res = msb.tile([16, 512], f32, tag="res")
                        nc.vector.tensor_mul(
                            res[0:10, 0:gn], z[0:10, 0:gn], sx[0:10, 0:gn])
                        nc.sync.dma_start(
                            outT[:, g0 * NT:g0 * NT + gn], res[0:10, 0:gn])

    nc.compile()
    return nc


def _inputs_per_core(inputs, src_w, ad_w, dst_col, meta):
    x = np.asarray(inputs["x"], dtype=np.float32)
    fb = np.asarray(inputs["fb"], dtype=np.float32)
    Wg = np.asarray(inputs["Wg"], dtype=np.float32)
    bias_g = np.asarray(inputs["bias_g"], dtype=np.float32)
    att_src = np.asarray(inputs["att_src"], dtype=np.float32)
    att_dst = np.asarray(inputs["att_dst"], dtype=np.float32)
    W1 = np.asarray(inputs["W1"], dtype=np.float32)
    b1 = np.asarray(inputs["b1"], dtype=np.float32)
    W2 = np.asarray(inputs["W2"], dtype=np.float32)
    b2 = np.asarray(inputs["b2"], dtype=np.float32)
    W3 = np.asarray(inputs["W3"], dtype=np.float32)
    b3 = np.asarray(inputs["b3"], dtype=np.float32)

    x_pad = np.zeros((NPAD, NFP), dtype=np.float32)
    x_pad[:N, :NF] = x
    fb_pad = np.zeros((NFP, NMEL), dtype=np.float32)
    fb_pad[:NF] = fb

    att_blk_s = np.zeros((HC, 4), dtype=np.float32)
    att_blk_d = np.zeros((HC, 4), dtype=np.float32)
    for h in range(H):
        att_blk_s[h * C:(h + 1) * C, h] = att_src[h]
        att_blk_d[h * C:(h + 1) * C, h] = att_dst[h]

    # head-interleaved feature order: new col j = old col (j%4)*32 + j//4
    perm_il = np.array([(j % 4) * 32 + j // 4 for j in range(HC)])
    Wg = np.ascontiguousarray(Wg[:, perm_il])
    att_blk_s = np.ascontiguousarray(att_blk_s[perm_il])
    att_blk_d = np.ascontiguousarray(att_blk_d[perm_il])
    bias_g = bias_g[perm_il]
    W1 = np.ascontiguousarray(W1[perm_il, :])

    b1p = np.zeros((128, 4), dtype=np.float32)
    b1p[:, 0] = b1[:128]
    b1p[:, 1] = b1[128:]
    b1p[:, 2:4] = -NEG_MLP * b1p[:, 0:2]
    b2p = np.zeros((128, 2), dtype=np.float32)
    b2p[:, 0] = b2
    b2p[:, 1] = -NEG_MLP * b2
    b3p = np.zeros((128, 2), dtype=np.float32)
    b3p[:10, 0] = b3
    b3p[:10, 1] = -NEG_MLP * b3

    common = {
        "fb_p": fb_pad.astype(BF16), "Wg": Wg,
        "attb_s": att_blk_s, "attb_d": att_blk_d,
        "bias_bc": np.tile(bias_g[None, :], (128, 1)).astype(BF16),
        "W1": W1.astype(BF16), "b1": b1p,
        "W2": W2.astype(BF16), "b2": b2p,
        "W3": W3.astype(BF16), "b3": b3p,
        "eye_f": np.eye(128, dtype=np.float32),
        "eye_b": np.eye(128).astype(BF16),
        "iota": np.tile(np.arange(128, dtype=np.float32)[None, :],
                        (128, 1)).astype(BF16),
        "ones": np.ones((128, 16)).astype(BF16),
        "flagz": np.zeros((1, 16)).astype(BF16),
    }
    xT_pad = np.ascontiguousarray(x_pad.T.astype(BF16))  # [640, NPAD]
    maps = []
    for k in range(NCORES):
        m = dict(common)
        m["xT_sl"] = np.ascontiguousarray(xT_pad[:, k * NPC:(k + 1) * NPC])
        m["idx_src"] = src_w[k]
        m["idx_ad"] = ad_w[k]
        m["dst_col"] = dst_col[k]
        maps.append(m)
    return maps


def kernel(**inputs):
    from concourse.bass_utils import run_bass_kernel_spmd

    src_w, ad_w, dst_col, meta = _prep(inputs["edge_index"])
    key = ("nc", meta["TOTC"], tuple(meta["cpt"].reshape(-1)),
           tuple(meta["woff"]))
    if key not in _CACHE:
        _CACHE.clear()
        _CACHE[key] = _build(meta)
    nc = _CACHE[key]
    maps = _inputs_per_core(inputs, src_w, ad_w, dst_col, meta)
    res = run_bass_kernel_spmd(nc, maps, core_ids=list(range(NCORES)))
    out = np.zeros((NPAD, 10), dtype=np.float32)
    for k in range(NCORES):
        out[k * NPC:(k + 1) * NPC] = res.results[k]["outT"].T
    return out[:N]


# revision 7
# speedup vs baseline: 1.2289x; 1.0089x over previous
"""GAT (gnn_message_passing) Trainium2 Bass kernel — 8-core SPMD, v2.

Contract: kernel(**inputs) -> np.ndarray with FULL inputs / FULL output.
Self-contained: hardcodes shapes; only imports the container's concourse stack.

v2 design vs v1:
  - bf16 edge path: shared table rows are 256x bf16 (512B), scatter matmuls,
    indicator and row-scaling all bf16.
  - No Lrelu on the Activation engine (no act-table reloads): attention
    leaky-relu is max(t, 0.2t) on DVE; MLP leaky-relu is
    relu(x+b) - relu(-a*x - a*b) via two Relu activations + one DVE subtract.
  - Stage A consumes host-transposed x (no PE transposes) and emits node-major
    rows [h | a_s | a_d] with one matmul against an augmented [Wg|Wg@as|Wg@ad].
  - KDW=1: each core writes its row slice straight into the shared DRAM table
    at a partition_id()-based dynamic offset, then a tiny AllGather acts as a
    barrier. KDW=0 falls back to two real bf16 AllGathers.
"""
import sys

for _p in ("/opt/trn_rl_repo", "/root/.axon_site/_ro/trn_rl_repo"):
    if _p not in sys.path:
        sys.path.append(_p)

import os
import numpy as np
import ml_dtypes

BF16 = ml_dtypes.bfloat16
# KDW modes: 0 = two half-table AllGathers (quarter row scheme),
#            1 = direct shared write + barrier (broken: scratchpad is only
#                pair-shared, kept for reference),
#            2 = ONE fat AllGather of the full 512B-pitch table, plain order
_KDW = int(os.environ.get("KDW", "2"))

# ---------------- problem constants (hardcoded per contract) ----------------
N = 50000
NF = 513
NFP = 640            # padded feature dim (5 * 128)
NMEL = 128
H, C = 4, 32
HC = H * C           # 128
E = 800000
NEG_ATT = 0.2
NEG_MLP = 0.01

NCORES = 8
TPC = 49             # tiles per core
NT = 128             # nodes per tile
NPC = TPC * NT       # 6272 nodes per core
NPAD = NCORES * NPC  # 50176
RDX = 256            # table row pitch in bf16 elems (512 B)
SPLIT = 4 * NPC      # 25088: table A/B row split (int16 idx headroom)
QSR = (0, 3072, NPC)  # KDW=0 quarter split (rows per AllGather region)

_CACHE = {}


def _prep(edge_index):
    """Host-side edge preprocessing. Returns per-core index/metadata arrays."""
    src = np.asarray(edge_index[0], dtype=np.int64)
    dst = np.asarray(edge_index[1], dtype=np.int64)
    loop = np.arange(N, dtype=np.int64)
    src = np.concatenate([src, loop])
    dst = np.concatenate([dst, loop])

    tile_g = dst // NT                # global tile id 0..391
    if _KDW >= 1:
        half = (src >= SPLIT).astype(np.int64)
        src_row = src - half * SPLIT
    else:
        r, l = src // NPC, src % NPC
        half = (l >= QSR[1]).astype(np.int64)
        src_row = np.where(half == 1,
                           r * (NPC - QSR[1]) + (l - QSR[1]),
                           r * QSR[1] + l)
    order = np.lexsort((src, dst, half, tile_g))
    src_row, dst, tile_g, half = (src_row[order], dst[order], tile_g[order],
                                  half[order])

    NTILES_G = NPAD // NT            # 392
    cnt = np.zeros((NTILES_G, 2), dtype=np.int64)
    np.add.at(cnt, (tile_g, half), 1)
    starts = np.zeros((NTILES_G, 2), dtype=np.int64)
    starts.reshape(-1)[1:] = np.cumsum(cnt.reshape(-1))[:-1]

    # chunks per (slot, half): max over cores
    cores = np.arange(NCORES)
    cpt = np.zeros((TPC, 2), dtype=np.int64)
    for s in range(TPC):
        t_ids = cores * TPC + s
        for hf in range(2):
            cpt[s, hf] = max(1, int(np.ceil(cnt[t_ids, hf].max() / NT)))
    TOTC = int(cpt.sum())
    TOTIDX = TOTC * NT

    src_rel = np.zeros((NCORES, TOTC, NT), dtype=np.int64)
    ad_idx = np.zeros((NCORES, TOTC, NT), dtype=np.int64)
    dst_rel = np.full((NCORES, TOTC, NT), 999.0, dtype=np.float32)
    dloc_all = np.zeros((NCORES, TOTC, NT), dtype=np.int64)
    valid = np.zeros((NCORES, TOTC, NT), dtype=bool)

    for k in range(NCORES):
        coff = 0
        for s in range(TPC):
            t = k * TPC + s
            for hf in range(2):
                nch = int(cpt[s, hf])
                st, cn = starts[t, hf], int(cnt[t, hf])
                src_rel[k, coff:coff + nch].reshape(-1)[:cn] = src_row[st:st + cn]
                ad_idx[k, coff:coff + nch].reshape(-1)[:cn] = (
                    dst[st:st + cn] % NPC - (s // 2) * 2 * NT)
                dloc_all[k, coff:coff + nch].reshape(-1)[:cn] = dst[st:st + cn] % NT
                valid[k, coff:coff + nch].reshape(-1)[:cn] = True
                coff += nch
        assert coff == TOTC

    assert src_rel.min() >= 0 and src_rel.max() <= 32767

    # per-chunk dst windows: 64-wide when the cross-core span fits, else 128
    woff = np.zeros(TOTC, dtype=np.int64)
    wlen = np.full(TOTC, 128, dtype=np.int64)
    for c in range(TOTC):
        v = valid[:, c, :]
        if v.any():
            dl = dloc_all[:, c, :][v]
            lo, hi = int(dl.min()), int(dl.max())
            wo = 0 if lo < 64 else 64
            if hi < wo + 64:
                woff[c] = wo
                wlen[c] = 64

    for k in range(NCORES):
        dr = dloc_all[k] - woff[:, None]
        dst_rel[k][valid[k]] = dr[valid[k]].astype(np.float32)

    # ---- regroup chunks: G slots per gather group, per (group, half) with
    # wl=64 chunks first so the indicator op can run width-aware ----
    G = 2
    coffs0 = np.concatenate([[0], np.cumsum(cpt.sum(axis=1))]).astype(int)
    perm = []          # new order -> original chunk index
    groups = []        # per group: dict
    for g0 in range(0, TPC, G):
        sl = [s for s in range(g0, min(g0 + G, TPC))]
        ginfo = {"slots": sl, "halves": []}
        for hf in range(2):
            idxs = []
            for s in sl:
                base = coffs0[s] + (0 if hf == 0 else int(cpt[s, 0]))
                idxs += [(base + j, s) for j in range(int(cpt[s, hf]))]
            idxs.sort(key=lambda t: 0 if wlen[t[0]] == 64 else 1)
            n64 = sum(1 for (c, _) in idxs if wlen[c] == 64)
            ginfo["halves"].append({
                "n": len(idxs), "n64": n64,
                "slot_of": [s for (_, s) in idxs],
            })
            perm += [c for (c, _) in idxs]
        groups.append(ginfo)
    perm = np.array(perm, dtype=np.int64)
    assert len(perm) == TOTC and len(set(perm.tolist())) == TOTC

    src_rel = src_rel[:, perm]
    ad_idx = ad_idx[:, perm]
    dst_rel = dst_rel[:, perm]
    woff = woff[perm]
    wlen = wlen[perm]

    # wrapped int16 index layout: [128, TOTIDX//16]
    def wrap(a):
        fl = a.reshape(NCORES, TOTIDX)
        w = fl.reshape(NCORES, TOTIDX // 16, 16).transpose(0, 2, 1)
        return np.tile(w, (1, 8, 1)).astype(np.int16)

    src_w = wrap(src_rel)
    ad_w = wrap(ad_idx)
    dst_col = dst_rel.transpose(0, 2, 1).astype(BF16)  # [NCORES, 128, TOTC]

    meta = {
        "cpt": cpt, "woff": woff, "wlen": wlen, "TOTC": TOTC,
        "TOTIDX": TOTIDX, "groups": groups, "G": G,
    }
    return src_w, ad_w, dst_col, meta


def _build(meta):
    import concourse.bass as bass
    import concourse.bacc as bacc
    import concourse.mybir as mybir
    import concourse.tile as tile

    f32 = mybir.dt.float32
    bf16 = mybir.dt.bfloat16
    i16 = mybir.dt.int16
    AF = mybir.ActivationFunctionType
    OP = mybir.AluOpType

    cpt, woff, wlen = meta["cpt"], meta["woff"], meta["wlen"]
    TOTC, TOTIDX = meta["TOTC"], meta["TOTIDX"]

    nc = bacc.Bacc("TRN2", target_bir_lowering=False, debug=False)

    # ---- I/O ----
    xT_sl = nc.dram_tensor("xT_sl", [NFP, NPC], bf16, kind="ExternalInput")
    idx_src = nc.dram_tensor("idx_src", [128, TOTIDX // 16], i16, kind="ExternalInput")
    idx_ad = nc.dram_tensor("idx_ad", [128, TOTIDX // 16], i16, kind="ExternalInput")
    dst_col = nc.dram_tensor("dst_col", [128, TOTC], bf16, kind="ExternalInput")
    fb_p = nc.dram_tensor("fb_p", [NFP, NMEL], bf16, kind="ExternalInput")
    Wg_d = nc.dram_tensor("Wg", [NMEL, HC], f32, kind="ExternalInput")
    attb_s = nc.dram_tensor("attb_s", [HC, 4], f32, kind="ExternalInput")
    attb_d = nc.dram_tensor("attb_d", [HC, 4], f32, kind="ExternalInput")
    bias_bc = nc.dram_tensor("bias_bc", [128, HC], bf16, kind="ExternalInput")
    W1_d = nc.dram_tensor("W1", [HC, 256], bf16, kind="ExternalInput")
    b1_d = nc.dram_tensor("b1", [128, 4], f32, kind="ExternalInput")   # [b1 | -a*b1]
    W2_d = nc.dram_tensor("W2", [256, HC], bf16, kind="ExternalInput")
    b2_d = nc.dram_tensor("b2", [128, 2], f32, kind="ExternalInput")   # [b2 | -a*b2]
    W3_d = nc.dram_tensor("W3", [HC, 10], bf16, kind="ExternalInput")
    b3_d = nc.dram_tensor("b3", [128, 2], f32, kind="ExternalInput")   # [b3 | -a*b3]
    eye_f = nc.dram_tensor("eye_f", [128, 128], f32, kind="ExternalInput")
    eye_b = nc.dram_tensor("eye_b", [128, 128], bf16, kind="ExternalInput")
    iota_d = nc.dram_tensor("iota", [128, 128], bf16, kind="ExternalInput")
    ones_d = nc.dram_tensor("ones", [128, 16], bf16, kind="ExternalInput")
    flag_d = nc.dram_tensor("flagz", [1, 16], bf16, kind="ExternalInput")
    outT = nc.dram_tensor("outT", [10, NPC], f32, kind="ExternalOutput")

    core_ids = list(range(NCORES))

    with tile.TileContext(nc) as tc:
        with (
            tc.tile_pool(name="dram", bufs=1, space="DRAM") as dpool,
            tc.tile_pool(name="const", bufs=1) as cpool,
        ):
            if _KDW == 1:
                # one shared table in plain node order; barrier flag separate
                Hfull = dpool.tile([NPAD, RDX], bf16, addr_space="Shared")
                Bar = dpool.tile([8, 16], bf16, addr_space="Shared")
                flag_loc = dpool.tile([1, 16], bf16)
            elif _KDW == 2:
                Hext_loc = dpool.tile([NPC, RDX], bf16)
                Hfull = dpool.tile([NPAD, RDX], bf16, addr_space="Shared")
            else:
                Hext_loc = dpool.tile([NPC, RDX], bf16)
                Hfull_a = dpool.tile([8 * QSR[1], RDX], bf16, addr_space="Shared")
                Hfull_b = dpool.tile([8 * (NPC - QSR[1]), RDX], bf16,
                                     addr_space="Shared")
            adrep = dpool.tile([NPC, 128], bf16)

            # ---- constants to SBUF ----
            fb_t = cpool.tile([128, 5, NMEL], bf16)
            nc.sync.dma_start(fb_t[:], fb_p.rearrange("(b p) m -> p b m", p=128))
            Wg_t = cpool.tile([128, HC], f32)
            nc.sync.dma_start(Wg_t[:], Wg_d[:])
            atts_t = cpool.tile([128, 4], f32)
            nc.sync.dma_start(atts_t[:], attb_s[:])
            attd_t = cpool.tile([128, 4], f32)
            nc.sync.dma_start(attd_t[:], attb_d[:])
            bias_t = cpool.tile([128, HC], bf16)
            nc.sync.dma_start(bias_t[:], bias_bc[:])
            W1_t = cpool.tile([128, 256], bf16)
            nc.sync.dma_start(W1_t[:], W1_d[:])
            b1_t = cpool.tile([128, 4], f32)
            nc.sync.dma_start(b1_t[:], b1_d[:])
            W2_t = cpool.tile([128, 2, HC], bf16)
            nc.sync.dma_start(W2_t[:], W2_d.rearrange("(b p) m -> p b m", p=128))
            b2_t = cpool.tile([128, 2], f32)
            nc.sync.dma_start(b2_t[:], b2_d[:])
            W3_t = cpool.tile([128, 10], bf16)
            nc.sync.dma_start(W3_t[:], W3_d[:])
            b3_t = cpool.tile([128, 2], f32)
            nc.sync.dma_start(b3_t[:], b3_d[:])
            eyef_t = cpool.tile([128, 128], f32)
            nc.sync.dma_start(eyef_t[:], eye_f[:])
            eyeb_t = cpool.tile([128, 128], bf16)
            nc.sync.dma_start(eyeb_t[:], eye_b[:])
            iota_t = cpool.tile([128, 128], bf16)
            nc.sync.dma_start(iota_t[:], iota_d[:])
            ones_t = cpool.tile([128, 16], bf16)
            nc.sync.dma_start(ones_t[:], ones_d[:])
            isrc_t = cpool.tile([128, TOTIDX // 16], i16)
            iad_t = cpool.tile([128, TOTIDX // 16], i16)
            dcol_t = cpool.tile([128, TOTC], bf16)

            # Wgaug [mel 128, 136] bf16 = [Wg | Wg@att_s | Wg@att_d]
            Wgaug_t = cpool.tile([128, 136], bf16)
            with tc.tile_pool(name="cpsum", bufs=1, space="PSUM") as cpsum:
                WgT_ps = cpsum.tile([128, 128], f32)
                nc.tensor.transpose(WgT_ps[:], Wg_t[:], eyef_t[:])
                WgT_t = cpool.tile([128, 128], f32)
                nc.vector.tensor_copy(WgT_t[:], WgT_ps[:])
                Wgatt_ps = cpsum.tile([128, 8], f32)
                nc.tensor.matmul(Wgatt_ps[:, 0:4], WgT_t[:], atts_t[:])
                nc.tensor.matmul(Wgatt_ps[:, 4:8], WgT_t[:], attd_t[:])
                nc.vector.tensor_copy(Wgaug_t[:, 0:128], Wg_t[:])
                nc.vector.tensor_copy(Wgaug_t[:, 128:136], Wgatt_ps[:])

            # ================= stage A =================
            bar = None
            rows_sb = cpool.tile([128, TPC, 136], bf16, name="rows_sb") if _KDW == 1 else None
            with (
                tc.tile_pool(name="sa_sb", bufs=2) as sa,
                tc.tile_pool(name="sa_ps", bufs=2, space="PSUM") as saps,
                tc.tile_pool(name="sa_ps1", bufs=2, space="PSUM") as saps1,
            ):
                QEND = {24: 0, 49: 1}
                for g0 in range(0, TPC, 4):
                    gsz = min(4, TPC - g0)
                    gn = gsz * NT
                    h1T_ps = saps.tile([128, 512], f32, tag="h1T")
                    xtb5 = sa.tile([128, 5, 512], bf16, tag="xtb5", bufs=3)
                    nc.sync.dma_start(
                        xtb5[:, :, 0:gn],
                        bass.AP(xT_sl, g0 * NT,
                                [[NPC, 128], [128 * NPC, 5], [1, gn]]))
                    for b in range(5):
                        nc.tensor.matmul(
                            h1T_ps[:, 0:gn], fb_t[:, b, :], xtb5[:, b, 0:gn],
                            start=(b == 0), stop=(b == 4))
                    h1T = sa.tile([128, 512], bf16, tag="h1Ts")
                    nc.scalar.activation(h1T[:, 0:gn], h1T_ps[:, 0:gn], AF.Copy)
                    rows4 = sa.tile([128, 4, 136], bf16, tag="rows4")
                    adr4 = sa.tile([128, 4, 128], bf16, tag="adr4")
                    for u in range(gsz):
                        s = g0 + u
                        h_ps = saps1.tile([128, 136], f32, tag="hps")
                        nc.tensor.matmul(
                            h_ps[:], h1T[:, u * NT:(u + 1) * NT], Wgaug_t[:])
                        hrow = rows4[:, u, :]
                        nc.scalar.activation(hrow, h_ps[:], AF.Copy)
                        hoff = rows4.offset + u * 136
                        nc.vector.tensor_copy(
                            adr4[:, u, :].rearrange("p (a b) -> p a b",
                                                    a=32, b=4),
                            bass.AP(rows4.tensor, hoff + 132,
                                    [rows4.ap[0], [0, 32], [1, 4]]))
                    nc.sync.dma_start(
                        bass.AP(Hext_loc.tensor,
                                Hext_loc.offset + g0 * NT * RDX,
                                [[RDX, NT], [NT * RDX, gsz], [1, 132]]),
                        rows4[:, 0:gsz, 0:132])
                    nc.scalar.dma_start(
                        bass.AP(adrep.tensor, adrep.offset + g0 * NT * 128,
                                [[128, NT], [NT * 128, gsz], [1, 128]]),
                        adr4[:, 0:gsz, :])
                    if _KDW == 0 and (g0 + gsz) in QEND:
                        q = QEND[g0 + gsz]
                        hf_out = Hfull_a if q == 0 else Hfull_b
                        nc.gpsimd.collective_compute(
                            "AllGather", mybir.AluOpType.bypass,
                            ins=[Hext_loc[QSR[q]:QSR[q + 1], :]],
                            outs=[hf_out[:]],
                            replica_groups=[core_ids])
                if _KDW == 2:
                    nc.gpsimd.collective_compute(
                        "AllGather", mybir.AluOpType.bypass,
                        ins=[Hext_loc[:]],
                        outs=[Hfull[:]],
                        replica_groups=[core_ids])
                if _KDW == 1:
                    import concourse.bass as _b
                    fz = sa.tile([1, 16], bf16, tag="fz")
                    nc.vector.memset(fz[:], 0.0)
                    nc.sync.dma_start(flag_loc[:, :], fz[:])
                    rk = nc.sync.partition_id()
                    rk_off = rk * (NPC * RDX)
                    # single write of the whole slice into the shared table
                    w = nc.sync.dma_start(
                        bass.AP(Hfull.tensor, rk_off + Hfull.offset,
                                [[RDX, NT], [NT * RDX, TPC], [1, 132]]),
                        rows_sb[:, :, 0:132])
                    bar = nc.gpsimd.collective_compute(
                        "AllGather", mybir.AluOpType.bypass,
                        ins=[flag_loc[0:1, 0:16]],
                        outs=[Bar[:, :]],
                        replica_groups=[core_ids])
                    _b._add_dep_helper(bar.ins, w.ins, sync=True,
                                       reason="barrier after shared write")

            nc.sync.dma_start(isrc_t[:], idx_src[:])
            nc.sync.dma_start(iad_t[:], idx_ad[:])
            nc.sync.dma_start(dcol_t[:], dst_col[:])

            # compact per-edge a_d staging: [128, TOTC, 4] bf16 (~8KB/prt)
            adall = cpool.tile([128, TOTC, 4], bf16, name="adall")

            # gather table views
            if _KDW >= 1:
                TA = Hfull[0:NPAD, :]
                TB = Hfull[SPLIT:NPAD, :]
            else:
                TA = Hfull_a[:]
                TB = Hfull_b[:]

            # ================= edge phase + MLP =================
            groups = meta["groups"]
            gstarts = []
            p = 0
            for gi in groups:
                gstarts.append(p)
                p += gi["halves"][0]["n"] + gi["halves"][1]["n"]
            assert p == TOTC
            GA_MAX = max(gi["halves"][0]["n"] for gi in groups)
            GB_MAX = max(gi["halves"][1]["n"] for gi in groups)
            TOTG_MAX = max(gi["halves"][0]["n"] + gi["halves"][1]["n"]
                           for gi in groups)

            with (
                tc.tile_pool(name="eg_g", bufs=3) as egg,
                tc.tile_pool(name="eg_sb", bufs=3) as egs,
                tc.tile_pool(name="eg_acc", bufs=4, space="PSUM") as egacc,
                tc.tile_pool(name="eg_tp", bufs=1, space="PSUM") as egtp,
                tc.tile_pool(name="mlp_sb", bufs=2) as msb,
                tc.tile_pool(name="mlp_ps", bufs=1, space="PSUM") as mps,
            ):
                # --- a_d prefetch: runs on DMA engines during the AllGather
                # (adrep slices are ready as soon as stage A passes the slot;
                # compact values land in the persistent adall tile) ---
                for ginfo, gstart in zip(groups, gstarts):
                    g0 = ginfo["slots"][0]
                    gext = len(ginfo["slots"]) * NT
                    tot_g = ginfo["halves"][0]["n"] + ginfo["halves"][1]["n"]
                    adp = egg.tile([128, TOTG_MAX, 128], bf16, tag="adp",
                                   bufs=3)
                    nc.gpsimd.dma_gather(
                        adp[:, 0:tot_g, :], adrep[g0 * NT:g0 * NT + gext, :],
                        iad_t[:, gstart * 8:(gstart + tot_g) * 8],
                        num_idxs=tot_g * NT, num_idxs_reg=tot_g * NT,
                        elem_size=128, single_packet=False)
                    nc.scalar.activation(
                        adall[:, gstart:gstart + tot_g, :],
                        adp[:, 0:tot_g, 0:4], AF.Copy)

                actT4 = None
                gsz = 4
                for ginfo, gstart in zip(groups, gstarts):
                    sl = ginfo["slots"]
                    hA, hB = ginfo["halves"]
                    nA, nB = hA["n"], hB["n"]
                    tot_g = nA + nB
                    # last (half, chunk-in-half) per slot for the stop flag
                    last_of = {}
                    for hf, hh in ((0, hA), (1, hB)):
                        for j, s in enumerate(hh["slot_of"]):
                            last_of[s] = (hf, j)

                    accs = {}
                    for s in sl:
                        acc = egacc.tile([128, 132], f32, tag="acc",
                                         name=f"acc_s{s % 2}")
                        nc.vector.memset(acc[:], 0.0)
                        accs[s] = acc

                    ad = adall[:, gstart:gstart + tot_g, :]

                    gA = egg.tile([128, GA_MAX, RDX], bf16, tag="gA")
                    giA = nc.gpsimd.dma_gather(
                        gA[:, 0:nA, :], TA,
                        isrc_t[:, gstart * 8:(gstart + nA) * 8],
                        num_idxs=nA * NT, num_idxs_reg=nA * NT,
                        elem_size=RDX, single_packet=False)
                    gB = egg.tile([128, GB_MAX, RDX], bf16, tag="gB")
                    giB = nc.gpsimd.dma_gather(
                        gB[:, 0:nB, :], TB,
                        isrc_t[:, (gstart + nA) * 8:(gstart + tot_g) * 8],
                        num_idxs=nB * NT, num_idxs_reg=nB * NT,
                        elem_size=RDX, single_packet=False)
                    if bar is not None:
                        import concourse.bass as _b
                        for gi_ in (giA, giB):
                            if gi_ is not None:
                                _b._add_dep_helper(
                                    gi_.ins, bar.ins, sync=True,
                                    reason="gather after shared-table barrier")

                    ind = egs.tile([128, TOTG_MAX, 128], bf16, tag="ind")
                    for (gt, hh, c0) in ((gA, hA, 0), (gB, hB, nA)):
                        nh = hh["n"]
                        if nh == 0:
                            continue
                        # t = a_s + a_d ; lrelu = max(t, 0.2t) ; ex = exp
                        tt = egs.tile([128, TOTG_MAX, 4], bf16, tag="tt",
                                      bufs=2)
                        nc.vector.tensor_tensor(
                            tt[:, 0:nh, :], gt[:, 0:nh, 128:132],
                            ad[:, c0:c0 + nh, :], OP.add)
                        t2 = egs.tile([128, TOTG_MAX, 4], bf16, tag="t2",
                                      bufs=2)
                        nc.vector.tensor_scalar(
                            t2[:, 0:nh, :], tt[:, 0:nh, :], NEG_ATT, None,
                            OP.mult)
                        nc.vector.tensor_tensor(
                            tt[:, 0:nh, :], tt[:, 0:nh, :], t2[:, 0:nh, :],
                            OP.max)
                        nc.scalar.activation(
                            tt[:, 0:nh, :], tt[:, 0:nh, :], AF.Exp)
                        # msg *= ex; rows are head-interleaved [32ch x 4h]
                        # so every operand's last dim is packed (DVE 2x mode)
                        g4 = bass.AP(
                            gt.tensor, gt.offset,
                            [gt.ap[0], [RDX, nh], [4, 32], [1, 4]])
                        exb = bass.AP(
                            tt.tensor, tt.offset,
                            [tt.ap[0], [4, nh], [0, 32], [1, 4]])
                        nc.vector.tensor_tensor(g4, g4, exb, OP.mult)
                        # ex -> row cols 128:132 (Activation engine copy)
                        nc.scalar.activation(
                            gt[:, 0:nh, 128:132], tt[:, 0:nh, :], AF.Copy)
                        # indicator, width-aware (wl=64 chunks ordered first)
                        n64 = hh["n64"]
                        for lo, ncnt, w in ((0, n64, 64), (n64, nh - n64, 128)):
                            if ncnt == 0:
                                continue
                            iob = bass.AP(
                                iota_t.tensor, iota_t.offset,
                                [iota_t.ap[0], [0, ncnt], [1, w]])
                            dcb = bass.AP(
                                dcol_t.tensor,
                                dcol_t.offset + gstart + c0 + lo,
                                [dcol_t.ap[0], [1, ncnt], [0, w]])
                            io = bass.AP(
                                ind.tensor, ind.offset + (c0 + lo) * 128,
                                [ind.ap[0], [128, ncnt], [1, w]])
                            nc.vector.tensor_tensor(io, iob, dcb, OP.is_equal)
                        for j in range(nh):
                            s = hh["slot_of"][j]
                            cglob = gstart + c0 + j
                            wo = int(woff[cglob])
                            wl = int(wlen[cglob])
                            nc.tensor.matmul(
                                accs[s][wo:wo + wl, :],
                                ind[:, c0 + j, 0:wl], gt[:, j, 0:132],
                                start=False,
                                stop=(last_of[s] == ((0 if c0 == 0 else 1), j)),
                                skip_group_check=True)

                    for s in sl:
                        acc = accs[s]
                        # normalize + bias + ELU (node-major)
                        dinv = egs.tile([128, 4], f32, tag="dinv")
                        nc.vector.tensor_scalar(
                            dinv[:], acc[:, 128:132], 1e-12, None, OP.add)
                        nc.vector.reciprocal(dinv[:], dinv[:])
                        gat = egs.tile([128, 128], bf16, tag="gat")
                        ga = bass.AP(gat.tensor, gat.offset,
                                     [gat.ap[0], [4, 32], [1, 4]])
                        aa = bass.AP(acc.tensor, acc.offset,
                                     [acc.ap[0], [4, 32], [1, 4]])
                        db = bass.AP(dinv.tensor, dinv.offset,
                                     [dinv.ap[0], [0, 32], [1, 4]])
                        nc.vector.tensor_tensor(ga, aa, db, OP.mult)
                        nc.vector.tensor_tensor(gat[:], gat[:], bias_t[:],
                                                OP.add)
                        # ELU = relu(x) - relu(1 - exp(x))
                        t1 = egs.tile([128, 128], bf16, tag="t1")
                        nc.scalar.activation(t1[:], gat[:], AF.Exp)
                        nc.scalar.activation(t1[:], t1[:], AF.Relu, scale=-1.0,
                                             bias=1.0)
                        nc.scalar.activation(gat[:], gat[:], AF.Relu)
                        nc.vector.tensor_sub(gat[:], gat[:], t1[:])
                        # transpose -> actT4
                        sub = s % 4
                        if sub == 0:
                            gsz = min(4, TPC - s)
                            actT4 = msb.tile([128, 4 * NT], bf16, tag="actT4")
                        tp = egtp.tile([128, 128], bf16, tag="tp2")
                        nc.tensor.transpose(tp[:], gat[:], eyeb_t[:])
                        nc.vector.tensor_copy(
                            actT4[:, sub * NT:(sub + 1) * NT], tp[:])
                        self_mlp = (sub == gsz - 1)
                        if self_mlp:
                        g0 = s - sub
                        gn = gsz * NT
                        # L1: lrelu(x+b) = relu(x+b) - relu(-a*x - a*b)
                        a1 = msb.tile([128, 2, 512], bf16, tag="a1")
                        r2 = msb.tile([128, 512], bf16, tag="r2")
                        for j in range(2):
                            o1 = mps.tile([128, 512], f32, tag="o1")
                            nc.tensor.matmul(
                                o1[:, 0:gn], W1_t[:, j * 128:(j + 1) * 128],
                                actT4[:, 0:gn])
                            nc.scalar.activation(
                                a1[:, j, 0:gn], o1[:, 0:gn], AF.Relu,
                                bias=b1_t[:, j:j + 1])
                            nc.scalar.activation(
                                r2[:, 0:gn], o1[:, 0:gn], AF.Relu,
                                scale=-NEG_MLP, bias=b1_t[:, 2 + j:3 + j])
                            nc.vector.tensor_sub(
                                a1[:, j, 0:gn], a1[:, j, 0:gn], r2[:, 0:gn])
                        o2 = mps.tile([128, 512], f32, tag="o2")
                        for j in range(2):
                            nc.tensor.matmul(
                                o2[:, 0:gn], W2_t[:, j, :], a1[:, j, 0:gn],
                                start=(j == 0), stop=(j == 1))
                        a2 = msb.tile([128, 512], bf16, tag="a2")
                        r2b = msb.tile([128, 512], bf16, tag="r2b")
                        nc.scalar.activation(
                            a2[:, 0:gn], o2[:, 0:gn], AF.Relu,
                            bias=b2_t[:, 0:1])
                        nc.scalar.activation(
                            r2b[:, 0:gn], o2[:, 0:gn], AF.Relu,
                            scale=-NEG_MLP, bias=b2_t[:, 1:2])
                        nc.vector.tensor_sub(
                            a2[:, 0:gn], a2[:, 0:gn], r2b[:, 0:gn])
                        o3 = mps.tile([16, 512], f32, tag="sm", name="o3_t")
                        nc.tensor.matmul(o3[0:10, 0:gn], W3_t[:], a2[:, 0:gn])
                        z = msb.tile([16, 512], bf16, tag="z")
                        zr = msb.tile([16, 512], bf16, tag="zr")
                        nc.scalar.activation(
                            z[0:10, 0:gn], o3[0:10, 0:gn], AF.Relu,
                            bias=b3_t[0:10, 0:1])
                        nc.scalar.activation(
                            zr[0:10, 0:gn], o3[0:10, 0:gn], AF.Relu,
                            scale=-NEG_MLP, bias=b3_t[0:10, 1:2])
                        nc.vector.tensor_sub(
                            z[0:10, 0:gn], z[0:10, 0:gn], zr[0:10, 0:gn])
                        nc.scalar.activation(z[0:10, 0:gn], z[0:10, 0:gn],
                                             AF.Exp)
                        ssum = mps.tile([16, 512], f32, tag="sm",
                                        name="ssum_t")[0:1, :]
                        nc.tensor.matmul(
                            ssum[:, 0:gn], ones_t[0:10, 0:1], z[0:10, 0:gn])
                        sinv = msb.tile([1, 512], bf16, tag="sinv")
                        with nc.allow_low_precision(reason="softmax recip"):
                            nc.vector.reciprocal(sinv[:, 0:gn], ssum[:, 0:gn])
                        sx = mps.tile([16, 512], f32, tag="sm", name="sx_t")
                        nc.tensor.matmul(
                            sx[0:10, 0:gn], ones_t[0:1, 0:10], sinv[:, 0:gn])
                        res = msb.tile([16, 512], f32, tag="res")
                        nc.vector.tensor_mul(
                            res[0:10, 0:gn], z[0:10, 0:gn], sx[0:10, 0:gn])
                        nc.sync.dma_start(
                            outT[:, g0 * NT:g0 * NT + gn], res[0:10, 0:gn])

    nc.compile()
    return nc


def _inputs_per_core(inputs, src_w, ad_w, dst_col, meta):
    x = np.asarray(inputs["x"], dtype=np.float32)
    fb = np.asarray(inputs["fb"], dtype=np.float32)
    Wg = np.asarray(inputs["Wg"], dtype=np.float32)
    bias_g = np.asarray(inputs["bias_g"], dtype=np.float32)
    att_src = np.asarray(inputs["att_src"], dtype=np.float32)
    att_dst = np.asarray(inputs["att_dst"], dtype=np.float32)
    W1 = np.asarray(inputs["W1"], dtype=np.float32)
    b1 = np.asarray(inputs["b1"], dtype=np.float32)
    W2 = np.asarray(inputs["W2"], dtype=np.float32)
    b2 = np.asarray(inputs["b2"], dtype=np.float32)
    W3 = np.asarray(inputs["W3"], dtype=np.float32)
    b3 = np.asarray(inputs["b3"], dtype=np.float32)

    x_pad = np.zeros((NPAD, NFP), dtype=np.float32)
    x_pad[:N, :NF] = x
    fb_pad = np.zeros((NFP, NMEL), dtype=np.float32)
    fb_pad[:NF] = fb

    att_blk_s = np.zeros((HC, 4), dtype=np.float32)
    att_blk_d = np.zeros((HC, 4), dtype=np.float32)
    for h in range(H):
        att_blk_s[h * C:(h + 1) * C, h] = att_src[h]
        att_blk_d[h * C:(h + 1) * C, h] = att_dst[h]

    # head-interleaved feature order: new col j = old col (j%4)*32 + j//4
    perm_il = np.array([(j % 4) * 32 + j // 4 for j in range(HC)])
    Wg = np.ascontiguousarray(Wg[:, perm_il])
    att_blk_s = np.ascontiguousarray(att_blk_s[perm_il])
    att_blk_d = np.ascontiguousarray(att_blk_d[perm_il])
    bias_g = bias_g[perm_il]
    W1 = np.ascontiguousarray(W1[perm_il, :])

    b1p = np.zeros((128, 4), dtype=np.float32)
    b1p[:, 0] = b1[:128]
    b1p[:, 1] = b1[128:]
    b1p[:, 2:4] = -NEG_MLP * b1p[:, 0:2]
    b2p = np.zeros((128, 2), dtype=np.float32)
    b2p[:, 0] = b2
    b2p[:, 1] = -NEG_MLP * b2
    b3p = np.zeros((128, 2), dtype=np.float32)
    b3p[:10, 0] = b3
    b3p[:10, 1] = -NEG_MLP * b3

    common = {
        "fb_p": fb_pad.astype(BF16), "Wg": Wg,
        "attb_s": att_blk_s, "attb_d": att_blk_d,
        "bias_bc": np.tile(bias_g[None, :], (128, 1)).astype(BF16),
        "W1": W1.astype(BF16), "b1": b1p,
        "W2": W2.astype(BF16), "b2": b2p,
        "W3": W3.astype(BF16), "b3": b3p,
        "eye_f": np.eye(128, dtype=np.float32),
        "eye_b": np.eye(128).astype(BF16),
        "iota": np.tile(np.arange(128, dtype=np.float32)[None, :],
                        (128, 1)).astype(BF16),
        "ones": np.ones((128, 16)).astype(BF16),
        "flagz": np.zeros((1, 16)).astype(BF16),
    }
    xT_pad = np.ascontiguousarray(x_pad.T.astype(BF16))  # [640, NPAD]
    maps = []
    for k in range(NCORES):
        m = dict(common)
        m["xT_sl"] = np.ascontiguousarray(xT_pad[:, k * NPC:(k + 1) * NPC])
        m["idx_src"] = src_w[k]
        m["idx_ad"] = ad_w[k]
        m["dst_col"] = dst_col[k]
        maps.append(m)
    return maps


def kernel(**inputs):
    from concourse.bass_utils import run_bass_kernel_spmd

    src_w, ad_w, dst_col, meta = _prep(inputs["edge_index"])
    key = ("nc", meta["TOTC"], tuple(meta["cpt"].reshape(-1)),
           tuple(meta["woff"]))
    if key not in _CACHE:
        _CACHE.clear()
        _CACHE[key] = _build(meta)
    nc = _CACHE[key]
    maps = _inputs_per_core(inputs, src_w, ad_w, dst_col, meta)
    res = run_bass_kernel_spmd(nc, maps, core_ids=list(range(NCORES)))
    out = np.zeros((NPAD, 10), dtype=np.float32)
    for k in range(NCORES):
        out[k * NPC:(k + 1) * NPC] = res.results[k]["outT"].T
    return out[:N]


# revision 8
# speedup vs baseline: 1.2501x; 1.0173x over previous
"""GAT (gnn_message_passing) Trainium2 Bass kernel — 8-core SPMD, v2.

Contract: kernel(**inputs) -> np.ndarray with FULL inputs / FULL output.
Self-contained: hardcodes shapes; only imports the container's concourse stack.

v2 design vs v1:
  - bf16 edge path: shared table rows are 256x bf16 (512B), scatter matmuls,
    indicator and row-scaling all bf16.
  - No Lrelu on the Activation engine (no act-table reloads): attention
    leaky-relu is max(t, 0.2t) on DVE; MLP leaky-relu is
    relu(x+b) - relu(-a*x - a*b) via two Relu activations + one DVE subtract.
  - Stage A consumes host-transposed x (no PE transposes) and emits node-major
    rows [h | a_s | a_d] with one matmul against an augmented [Wg|Wg@as|Wg@ad].
  - KDW=1: each core writes its row slice straight into the shared DRAM table
    at a partition_id()-based dynamic offset, then a tiny AllGather acts as a
    barrier. KDW=0 falls back to two real bf16 AllGathers.
"""
import sys

for _p in ("/opt/trn_rl_repo", "/root/.axon_site/_ro/trn_rl_repo"):
    if _p not in sys.path:
        sys.path.append(_p)

import os
import numpy as np
import ml_dtypes

BF16 = ml_dtypes.bfloat16
# KDW modes: 0 = two half-table AllGathers (quarter row scheme),
#            1 = direct shared write + barrier (broken: scratchpad is only
#                pair-shared, kept for reference),
#            2 = ONE fat AllGather of the full 512B-pitch table, plain order
_KDW = int(os.environ.get("KDW", "2"))

# ---------------- problem constants (hardcoded per contract) ----------------
N = 50000
NF = 513
NFP = 640            # padded feature dim (5 * 128)
NMEL = 128
H, C = 4, 32
HC = H * C           # 128
E = 800000
NEG_ATT = 0.2
NEG_MLP = 0.01

NCORES = 8
TPC = 49             # tiles per core
NT = 128             # nodes per tile
NPC = TPC * NT       # 6272 nodes per core
NPAD = NCORES * NPC  # 50176
RDX = 256            # table row pitch in bf16 elems (512 B)
SPLIT = 4 * NPC      # 25088: table A/B row split (int16 idx headroom)
QSR = (0, 3072, NPC)  # KDW=0 quarter split (rows per AllGather region)

_CACHE = {}


def _prep(edge_index):
    """Host-side edge preprocessing. Returns per-core index/metadata arrays."""
    src = np.asarray(edge_index[0], dtype=np.int64)
    dst = np.asarray(edge_index[1], dtype=np.int64)
    loop = np.arange(N, dtype=np.int64)
    src = np.concatenate([src, loop])
    dst = np.concatenate([dst, loop])

    tile_g = dst // NT                # global tile id 0..391
    if _KDW >= 1:
        half = (src >= SPLIT).astype(np.int64)
        src_row = src - half * SPLIT
    else:
        r, l = src // NPC, src % NPC
        half = (l >= QSR[1]).astype(np.int64)
        src_row = np.where(half == 1,
                           r * (NPC - QSR[1]) + (l - QSR[1]),
                           r * QSR[1] + l)
    order = np.lexsort((src, dst, half, tile_g))
    src_row, dst, tile_g, half = (src_row[order], dst[order], tile_g[order],
                                  half[order])

    NTILES_G = NPAD // NT            # 392
    cnt = np.zeros((NTILES_G, 2), dtype=np.int64)
    np.add.at(cnt, (tile_g, half), 1)
    starts = np.zeros((NTILES_G, 2), dtype=np.int64)
    starts.reshape(-1)[1:] = np.cumsum(cnt.reshape(-1))[:-1]

    # chunks per (slot, half): max over cores
    cores = np.arange(NCORES)
    cpt = np.zeros((TPC, 2), dtype=np.int64)
    for s in range(TPC):
        t_ids = cores * TPC + s
        for hf in range(2):
            cpt[s, hf] = max(1, int(np.ceil(cnt[t_ids, hf].max() / NT)))
    TOTC = int(cpt.sum())
    TOTIDX = TOTC * NT

    src_rel = np.zeros((NCORES, TOTC, NT), dtype=np.int64)
    ad_idx = np.zeros((NCORES, TOTC, NT), dtype=np.int64)
    dst_rel = np.full((NCORES, TOTC, NT), 999.0, dtype=np.float32)
    dloc_all = np.zeros((NCORES, TOTC, NT), dtype=np.int64)
    valid = np.zeros((NCORES, TOTC, NT), dtype=bool)

    for k in range(NCORES):
        coff = 0
        for s in range(TPC):
            t = k * TPC + s
            for hf in range(2):
                nch = int(cpt[s, hf])
                st, cn = starts[t, hf], int(cnt[t, hf])
                src_rel[k, coff:coff + nch].reshape(-1)[:cn] = src_row[st:st + cn]
                ad_idx[k, coff:coff + nch].reshape(-1)[:cn] = (
                    dst[st:st + cn] % NPC - (s // 2) * 2 * NT)
                dloc_all[k, coff:coff + nch].reshape(-1)[:cn] = dst[st:st + cn] % NT
                valid[k, coff:coff + nch].reshape(-1)[:cn] = True
                coff += nch
        assert coff == TOTC

    assert src_rel.min() >= 0 and src_rel.max() <= 32767

    # per-chunk dst windows: 64-wide when the cross-core span fits, else 128
    woff = np.zeros(TOTC, dtype=np.int64)
    wlen = np.full(TOTC, 128, dtype=np.int64)
    for c in range(TOTC):
        v = valid[:, c, :]
        if v.any():
            dl = dloc_all[:, c, :][v]
            lo, hi = int(dl.min()), int(dl.max())
            wo = 0 if lo < 64 else 64
            if hi < wo + 64:
                woff[c] = wo
                wlen[c] = 64

    for k in range(NCORES):
        dr = dloc_all[k] - woff[:, None]
        dst_rel[k][valid[k]] = dr[valid[k]].astype(np.float32)

    # ---- regroup chunks: G slots per gather group, per (group, half) with
    # wl=64 chunks first so the indicator op can run width-aware ----
    G = 2
    coffs0 = np.concatenate([[0], np.cumsum(cpt.sum(axis=1))]).astype(int)
    perm = []          # new order -> original chunk index
    groups = []        # per group: dict
    for g0 in range(0, TPC, G):
        sl = [s for s in range(g0, min(g0 + G, TPC))]
        ginfo = {"slots": sl, "halves": []}
        for hf in range(2):
            idxs = []
            for s in sl:
                base = coffs0[s] + (0 if hf == 0 else int(cpt[s, 0]))
                idxs += [(base + j, s) for j in range(int(cpt[s, hf]))]
            idxs.sort(key=lambda t: 0 if wlen[t[0]] == 64 else 1)
            n64 = sum(1 for (c, _) in idxs if wlen[c] == 64)
            ginfo["halves"].append({
                "n": len(idxs), "n64": n64,
                "slot_of": [s for (_, s) in idxs],
            })
            perm += [c for (c, _) in idxs]
        groups.append(ginfo)
    perm = np.array(perm, dtype=np.int64)
    assert len(perm) == TOTC and len(set(perm.tolist())) == TOTC

    src_rel = src_rel[:, perm]
    ad_idx = ad_idx[:, perm]
    dst_rel = dst_rel[:, perm]
    woff = woff[perm]
    wlen = wlen[perm]

    # wrapped int16 index layout: [128, TOTIDX//16]
    def wrap(a):
        fl = a.reshape(NCORES, TOTIDX)
        w = fl.reshape(NCORES, TOTIDX // 16, 16).transpose(0, 2, 1)
        return np.tile(w, (1, 8, 1)).astype(np.int16)

    src_w = wrap(src_rel)
    ad_w = wrap(ad_idx)
    dst_col = dst_rel.transpose(0, 2, 1).astype(BF16)  # [NCORES, 128, TOTC]

    meta = {
        "cpt": cpt, "woff": woff, "wlen": wlen, "TOTC": TOTC,
        "TOTIDX": TOTIDX, "groups": groups, "G": G,
    }
    return src_w, ad_w, dst_col, meta


def _build(meta):
    import concourse.bass as bass
    import concourse.bacc as bacc
    import concourse.mybir as mybir
    import concourse.tile as tile

    f32 = mybir.dt.float32
    bf16 = mybir.dt.bfloat16
    i16 = mybir.dt.int16
    AF = mybir.ActivationFunctionType
    OP = mybir.AluOpType

    cpt, woff, wlen = meta["cpt"], meta["woff"], meta["wlen"]
    TOTC, TOTIDX = meta["TOTC"], meta["TOTIDX"]

    nc = bacc.Bacc("TRN2", target_bir_lowering=False, debug=False)

    # ---- I/O ----
    xT_sl = nc.dram_tensor("xT_sl", [NFP, NPC], bf16, kind="ExternalInput")
    idx_src = nc.dram_tensor("idx_src", [128, TOTIDX // 16], i16, kind="ExternalInput")
    idx_ad = nc.dram_tensor("idx_ad", [128, TOTIDX // 16], i16, kind="ExternalInput")
    dst_col = nc.dram_tensor("dst_col", [128, TOTC], bf16, kind="ExternalInput")
    fb_p = nc.dram_tensor("fb_p", [NFP, NMEL], bf16, kind="ExternalInput")
    Wg_d = nc.dram_tensor("Wg", [NMEL, HC], f32, kind="ExternalInput")
    attb_s = nc.dram_tensor("attb_s", [HC, 4], f32, kind="ExternalInput")
    attb_d = nc.dram_tensor("attb_d", [HC, 4], f32, kind="ExternalInput")
    bias_bc = nc.dram_tensor("bias_bc", [128, HC], bf16, kind="ExternalInput")
    W1_d = nc.dram_tensor("W1", [HC, 256], bf16, kind="ExternalInput")
    b1_d = nc.dram_tensor("b1", [128, 4], f32, kind="ExternalInput")   # [b1 | -a*b1]
    W2_d = nc.dram_tensor("W2", [256, HC], bf16, kind="ExternalInput")
    b2_d = nc.dram_tensor("b2", [128, 2], f32, kind="ExternalInput")   # [b2 | -a*b2]
    W3_d = nc.dram_tensor("W3", [HC, 10], bf16, kind="ExternalInput")
    b3_d = nc.dram_tensor("b3", [128, 2], f32, kind="ExternalInput")   # [b3 | -a*b3]
    eye_f = nc.dram_tensor("eye_f", [128, 128], f32, kind="ExternalInput")
    eye_b = nc.dram_tensor("eye_b", [128, 128], bf16, kind="ExternalInput")
    iota_d = nc.dram_tensor("iota", [128, 128], bf16, kind="ExternalInput")
    ones_d = nc.dram_tensor("ones", [128, 16], bf16, kind="ExternalInput")
    flag_d = nc.dram_tensor("flagz", [1, 16], bf16, kind="ExternalInput")
    outT = nc.dram_tensor("outT", [10, NPC], f32, kind="ExternalOutput")

    core_ids = list(range(NCORES))

    with tile.TileContext(nc) as tc:
        with (
            tc.tile_pool(name="dram", bufs=1, space="DRAM") as dpool,
            tc.tile_pool(name="const", bufs=1) as cpool,
        ):
            if _KDW == 1:
                # one shared table in plain node order; barrier flag separate
                Hfull = dpool.tile([NPAD, RDX], bf16, addr_space="Shared")
                Bar = dpool.tile([8, 16], bf16, addr_space="Shared")
                flag_loc = dpool.tile([1, 16], bf16)
            elif _KDW == 2:
                Hext_loc = dpool.tile([NPC, RDX], bf16)
                Hfull = dpool.tile([NPAD, RDX], bf16, addr_space="Shared")
            else:
                Hext_loc = dpool.tile([NPC, RDX], bf16)
                Hfull_a = dpool.tile([8 * QSR[1], RDX], bf16, addr_space="Shared")
                Hfull_b = dpool.tile([8 * (NPC - QSR[1]), RDX], bf16,
                                     addr_space="Shared")
            adrep = dpool.tile([NPC, 128], bf16)

            # ---- constants to SBUF ----
            fb_t = cpool.tile([128, 5, NMEL], bf16)
            nc.sync.dma_start(fb_t[:], fb_p.rearrange("(b p) m -> p b m", p=128))
            Wg_t = cpool.tile([128, HC], f32)
            nc.sync.dma_start(Wg_t[:], Wg_d[:])
            atts_t = cpool.tile([128, 4], f32)
            nc.sync.dma_start(atts_t[:], attb_s[:])
            attd_t = cpool.tile([128, 4], f32)
            nc.sync.dma_start(attd_t[:], attb_d[:])
            bias_t = cpool.tile([128, HC], bf16)
            nc.sync.dma_start(bias_t[:], bias_bc[:])
            W1_t = cpool.tile([128, 256], bf16)
            nc.sync.dma_start(W1_t[:], W1_d[:])
            b1_t = cpool.tile([128, 4], f32)
            nc.sync.dma_start(b1_t[:], b1_d[:])
            W2_t = cpool.tile([128, 2, HC], bf16)
            nc.sync.dma_start(W2_t[:], W2_d.rearrange("(b p) m -> p b m", p=128))
            b2_t = cpool.tile([128, 2], f32)
            nc.sync.dma_start(b2_t[:], b2_d[:])
            W3_t = cpool.tile([128, 10], bf16)
            nc.sync.dma_start(W3_t[:], W3_d[:])
            b3_t = cpool.tile([128, 2], f32)
            nc.sync.dma_start(b3_t[:], b3_d[:])
            eyef_t = cpool.tile([128, 128], f32)
            nc.sync.dma_start(eyef_t[:], eye_f[:])
            eyeb_t = cpool.tile([128, 128], bf16)
            nc.sync.dma_start(eyeb_t[:], eye_b[:])
            iota_t = cpool.tile([128, 128], bf16)
            nc.sync.dma_start(iota_t[:], iota_d[:])
            ones_t = cpool.tile([128, 16], bf16)
            nc.sync.dma_start(ones_t[:], ones_d[:])
            isrc_t = cpool.tile([128, TOTIDX // 16], i16)
            iad_t = cpool.tile([128, TOTIDX // 16], i16)
            dcol_t = cpool.tile([128, TOTC], bf16)

            # Wgaug [mel 128, 136] bf16 = [Wg | Wg@att_s | Wg@att_d]
            Wgaug_t = cpool.tile([128, 136], bf16)
            with tc.tile_pool(name="cpsum", bufs=1, space="PSUM") as cpsum:
                WgT_ps = cpsum.tile([128, 128], f32)
                nc.tensor.transpose(WgT_ps[:], Wg_t[:], eyef_t[:])
                WgT_t = cpool.tile([128, 128], f32)
                nc.vector.tensor_copy(WgT_t[:], WgT_ps[:])
                Wgatt_ps = cpsum.tile([128, 8], f32)
                nc.tensor.matmul(Wgatt_ps[:, 0:4], WgT_t[:], atts_t[:])
                nc.tensor.matmul(Wgatt_ps[:, 4:8], WgT_t[:], attd_t[:])
                nc.vector.tensor_copy(Wgaug_t[:, 0:128], Wg_t[:])
                nc.vector.tensor_copy(Wgaug_t[:, 128:136], Wgatt_ps[:])

            # ================= stage A =================
            bar = None
            rows_sb = cpool.tile([128, TPC, 136], bf16, name="rows_sb") if _KDW == 1 else None
            with (
                tc.tile_pool(name="sa_sb", bufs=2) as sa,
                tc.tile_pool(name="sa_ps", bufs=2, space="PSUM") as saps,
                tc.tile_pool(name="sa_ps1", bufs=2, space="PSUM") as saps1,
            ):
                QEND = {24: 0, 49: 1}
                for g0 in range(0, TPC, 4):
                    gsz = min(4, TPC - g0)
                    gn = gsz * NT
                    h1T_ps = saps.tile([128, 512], f32, tag="h1T")
                    xtb5 = sa.tile([128, 5, 512], bf16, tag="xtb5", bufs=3)
                    nc.sync.dma_start(
                        xtb5[:, :, 0:gn],
                        bass.AP(xT_sl, g0 * NT,
                                [[NPC, 128], [128 * NPC, 5], [1, gn]]))
                    for b in range(5):
                        nc.tensor.matmul(
                            h1T_ps[:, 0:gn], fb_t[:, b, :], xtb5[:, b, 0:gn],
                            start=(b == 0), stop=(b == 4))
                    h1T = sa.tile([128, 512], bf16, tag="h1Ts")
                    nc.scalar.activation(h1T[:, 0:gn], h1T_ps[:, 0:gn], AF.Copy)
                    rows4 = sa.tile([128, 4, 136], bf16, tag="rows4", bufs=3)
                    adr4 = sa.tile([128, 4, 128], bf16, tag="adr4", bufs=3)
                    for u in range(gsz):
                        s = g0 + u
                        h_ps = saps1.tile([128, 136], f32, tag="hps")
                        nc.tensor.matmul(
                            h_ps[:], h1T[:, u * NT:(u + 1) * NT], Wgaug_t[:])
                        hrow = rows4[:, u, :]
                        nc.scalar.activation(hrow, h_ps[:], AF.Copy)
                        hoff = rows4.offset + u * 136
                        nc.vector.tensor_copy(
                            adr4[:, u, :].rearrange("p (a b) -> p a b",
                                                    a=32, b=4),
                            bass.AP(rows4.tensor, hoff + 132,
                                    [rows4.ap[0], [0, 32], [1, 4]]))
                    nc.sync.dma_start(
                        bass.AP(Hext_loc.tensor,
                                Hext_loc.offset + g0 * NT * RDX,
                                [[RDX, NT], [NT * RDX, gsz], [1, 132]]),
                        rows4[:, 0:gsz, 0:132])
                    nc.scalar.dma_start(
                        bass.AP(adrep.tensor, adrep.offset + g0 * NT * 128,
                                [[128, NT], [NT * 128, gsz], [1, 128]]),
                        adr4[:, 0:gsz, :])
                    if _KDW == 0 and (g0 + gsz) in QEND:
                        q = QEND[g0 + gsz]
                        hf_out = Hfull_a if q == 0 else Hfull_b
                        nc.gpsimd.collective_compute(
                            "AllGather", mybir.AluOpType.bypass,
                            ins=[Hext_loc[QSR[q]:QSR[q + 1], :]],
                            outs=[hf_out[:]],
                            replica_groups=[core_ids])
                if _KDW == 2:
                    nc.gpsimd.collective_compute(
                        "AllGather", mybir.AluOpType.bypass,
                        ins=[Hext_loc[:]],
                        outs=[Hfull[:]],
                        replica_groups=[core_ids])
                if _KDW == 1:
                    import concourse.bass as _b
                    fz = sa.tile([1, 16], bf16, tag="fz")
                    nc.vector.memset(fz[:], 0.0)
                    nc.sync.dma_start(flag_loc[:, :], fz[:])
                    rk = nc.sync.partition_id()
                    rk_off = rk * (NPC * RDX)
                    # single write of the whole slice into the shared table
                    w = nc.sync.dma_start(
                        bass.AP(Hfull.tensor, rk_off + Hfull.offset,
                                [[RDX, NT], [NT * RDX, TPC], [1, 132]]),
                        rows_sb[:, :, 0:132])
                    bar = nc.gpsimd.collective_compute(
                        "AllGather", mybir.AluOpType.bypass,
                        ins=[flag_loc[0:1, 0:16]],
                        outs=[Bar[:, :]],
                        replica_groups=[core_ids])
                    _b._add_dep_helper(bar.ins, w.ins, sync=True,
                                       reason="barrier after shared write")

            nc.sync.dma_start(isrc_t[:], idx_src[:])
            nc.sync.dma_start(iad_t[:], idx_ad[:])
            nc.sync.dma_start(dcol_t[:], dst_col[:])

            # compact per-edge a_d staging: [128, TOTC, 4] bf16 (~8KB/prt)
            adall = cpool.tile([128, TOTC, 4], bf16, name="adall")

            # gather table views
            if _KDW >= 1:
                TA = Hfull[0:NPAD, :]
                TB = Hfull[SPLIT:NPAD, :]
            else:
                TA = Hfull_a[:]
                TB = Hfull_b[:]

            # ================= edge phase + MLP =================
            groups = meta["groups"]
            gstarts = []
            p = 0
            for gi in groups:
                gstarts.append(p)
                p += gi["halves"][0]["n"] + gi["halves"][1]["n"]
            assert p == TOTC
            GA_MAX = max(gi["halves"][0]["n"] for gi in groups)
            GB_MAX = max(gi["halves"][1]["n"] for gi in groups)
            TOTG_MAX = max(gi["halves"][0]["n"] + gi["halves"][1]["n"]
                           for gi in groups)

            with (
                tc.tile_pool(name="eg_g", bufs=3) as egg,
                tc.tile_pool(name="eg_sb", bufs=3) as egs,
                tc.tile_pool(name="eg_acc", bufs=4, space="PSUM") as egacc,
                tc.tile_pool(name="eg_tp", bufs=1, space="PSUM") as egtp,
                tc.tile_pool(name="mlp_sb", bufs=2) as msb,
                tc.tile_pool(name="mlp_ps", bufs=1, space="PSUM") as mps,
            ):
                # --- a_d prefetch: runs on DMA engines during the AllGather
                # (adrep slices are ready as soon as stage A passes the slot;
                # compact values land in the persistent adall tile) ---
                for ginfo, gstart in zip(groups, gstarts):
                    g0 = ginfo["slots"][0]
                    gext = len(ginfo["slots"]) * NT
                    tot_g = ginfo["halves"][0]["n"] + ginfo["halves"][1]["n"]
                    adp = egg.tile([128, TOTG_MAX, 128], bf16, tag="adp",
                                   bufs=3)
                    nc.gpsimd.dma_gather(
                        adp[:, 0:tot_g, :], adrep[g0 * NT:g0 * NT + gext, :],
                        iad_t[:, gstart * 8:(gstart + tot_g) * 8],
                        num_idxs=tot_g * NT, num_idxs_reg=tot_g * NT,
                        elem_size=128, single_packet=False)
                    nc.scalar.activation(
                        adall[:, gstart:gstart + tot_g, :],
                        adp[:, 0:tot_g, 0:4], AF.Copy)

                actT4 = None
                gsz = 4
                for ginfo, gstart in zip(groups, gstarts):
                    sl = ginfo["slots"]
                    hA, hB = ginfo["halves"]
                    nA, nB = hA["n"], hB["n"]
                    tot_g = nA + nB
                    # last (half, chunk-in-half) per slot for the stop flag
                    last_of = {}
                    for hf, hh in ((0, hA), (1, hB)):
                        for j, s in enumerate(hh["slot_of"]):
                            last_of[s] = (hf, j)

                    accs = {}
                    for s in sl:
                        acc = egacc.tile([128, 132], f32, tag="acc",
                                         name=f"acc_s{s % 2}")
                        nc.vector.memset(acc[:], 0.0)
                        accs[s] = acc

                    ad = adall[:, gstart:gstart + tot_g, :]

                    gA = egg.tile([128, GA_MAX, RDX], bf16, tag="gA")
                    giA = nc.gpsimd.dma_gather(
                        gA[:, 0:nA, :], TA,
                        isrc_t[:, gstart * 8:(gstart + nA) * 8],
                        num_idxs=nA * NT, num_idxs_reg=nA * NT,
                        elem_size=RDX, single_packet=False)
                    gB = egg.tile([128, GB_MAX, RDX], bf16, tag="gB")
                    giB = nc.gpsimd.dma_gather(
                        gB[:, 0:nB, :], TB,
                        isrc_t[:, (gstart + nA) * 8:(gstart + tot_g) * 8],
                        num_idxs=nB * NT, num_idxs_reg=nB * NT,
                        elem_size=RDX, single_packet=False)
                    if bar is not None:
                        import concourse.bass as _b
                        for gi_ in (giA, giB):
                            if gi_ is not None:
                                _b._add_dep_helper(
                                    gi_.ins, bar.ins, sync=True,
                                    reason="gather after shared-table barrier")

                    ind = egs.tile([128, TOTG_MAX, 128], bf16, tag="ind")
                    for (gt, hh, c0) in ((gA, hA, 0), (gB, hB, nA)):
                        nh = hh["n"]
                        if nh == 0:
                            continue
                        # t = a_s + a_d ; lrelu = max(t, 0.2t) ; ex = exp
                        tt = egs.tile([128, TOTG_MAX, 4], bf16, tag="tt",
                                      bufs=2)
                        nc.vector.tensor_tensor(
                            tt[:, 0:nh, :], gt[:, 0:nh, 128:132],
                            ad[:, c0:c0 + nh, :], OP.add)
                        t2 = egs.tile([128, TOTG_MAX, 4], bf16, tag="t2",
                                      bufs=2)
                        nc.vector.tensor_scalar(
                            t2[:, 0:nh, :], tt[:, 0:nh, :], NEG_ATT, None,
                            OP.mult)
                        nc.vector.tensor_tensor(
                            tt[:, 0:nh, :], tt[:, 0:nh, :], t2[:, 0:nh, :],
                            OP.max)
                        nc.scalar.activation(
                            tt[:, 0:nh, :], tt[:, 0:nh, :], AF.Exp)
                        # msg *= ex; rows are head-interleaved [32ch x 4h]
                        # so every operand's last dim is packed (DVE 2x mode)
                        g4 = bass.AP(
                            gt.tensor, gt.offset,
                            [gt.ap[0], [RDX, nh], [4, 32], [1, 4]])
                        exb = bass.AP(
                            tt.tensor, tt.offset,
                            [tt.ap[0], [4, nh], [0, 32], [1, 4]])
                        nc.vector.tensor_tensor(g4, g4, exb, OP.mult)
                        # ex -> row cols 128:132 (Activation engine copy)
                        nc.scalar.activation(
                            gt[:, 0:nh, 128:132], tt[:, 0:nh, :], AF.Copy)
                        # indicator, width-aware (wl=64 chunks ordered first)
                        n64 = hh["n64"]
                        for lo, ncnt, w in ((0, n64, 64), (n64, nh - n64, 128)):
                            if ncnt == 0:
                                continue
                            iob = bass.AP(
                                iota_t.tensor, iota_t.offset,
                                [iota_t.ap[0], [0, ncnt], [1, w]])
                            dcb = bass.AP(
                                dcol_t.tensor,
                                dcol_t.offset + gstart + c0 + lo,
                                [dcol_t.ap[0], [1, ncnt], [0, w]])
                            io = bass.AP(
                                ind.tensor, ind.offset + (c0 + lo) * 128,
                                [ind.ap[0], [128, ncnt], [1, w]])
                            nc.vector.tensor_tensor(io, iob, dcb, OP.is_equal)
                        for j in range(nh):
                            s = hh["slot_of"][j]
                            cglob = gstart + c0 + j
                            wo = int(woff[cglob])
                            wl = int(wlen[cglob])
                            nc.tensor.matmul(
                                accs[s][wo:wo + wl, :],
                                ind[:, c0 + j, 0:wl], gt[:, j, 0:132],
                                start=False,
                                stop=(last_of[s] == ((0 if c0 == 0 else 1), j)),
                                skip_group_check=True)

                    for s in sl:
                        acc = accs[s]
                        # normalize + bias + ELU (node-major)
                        dinv = egs.tile([128, 4], f32, tag="dinv")
                        nc.vector.tensor_scalar(
                            dinv[:], acc[:, 128:132], 1e-12, None, OP.add)
                        nc.vector.reciprocal(dinv[:], dinv[:])
                        gat = egs.tile([128, 128], bf16, tag="gat")
                        ga = bass.AP(gat.tensor, gat.offset,
                                     [gat.ap[0], [4, 32], [1, 4]])
                        aa = bass.AP(acc.tensor, acc.offset,
                                     [acc.ap[0], [4, 32], [1, 4]])
                        db = bass.AP(dinv.tensor, dinv.offset,
                                     [dinv.ap[0], [0, 32], [1, 4]])
                        nc.vector.tensor_tensor(ga, aa, db, OP.mult)
                        nc.vector.tensor_tensor(gat[:], gat[:], bias_t[:],
                                                OP.add)
                        # ELU = relu(x) - relu(1 - exp(x))
                        t1 = egs.tile([128, 128], bf16, tag="t1")
                        nc.scalar.activation(t1[:], gat[:], AF.Exp)
                        nc.scalar.activation(t1[:], t1[:], AF.Relu, scale=-1.0,
                                             bias=1.0)
                        nc.scalar.activation(gat[:], gat[:], AF.Relu)
                        nc.vector.tensor_sub(gat[:], gat[:], t1[:])
                        # transpose -> actT4
                        sub = s % 4
                        if sub == 0:
                            gsz = min(4, TPC - s)
                            actT4 = msb.tile([128, 4 * NT], bf16, tag="actT4")
                        tp = egtp.tile([128, 128], bf16, tag="tp2")
                        nc.tensor.transpose(tp[:], gat[:], eyeb_t[:])
                        nc.vector.tensor_copy(
                            actT4[:, sub * NT:(sub + 1) * NT], tp[:])
                        self_mlp = (sub == gsz - 1)
                        if self_mlp:
                        g0 = s - sub
                        gn = gsz * NT
                        # L1: lrelu(x+b) = relu(x+b) - relu(-a*x - a*b)
                        a1 = msb.tile([128, 2, 512], bf16, tag="a1")
                        r2 = msb.tile([128, 512], bf16, tag="r2")
                        for j in range(2):
                            o1 = mps.tile([128, 512], f32, tag="o1")
                            nc.tensor.matmul(
                                o1[:, 0:gn], W1_t[:, j * 128:(j + 1) * 128],
                                actT4[:, 0:gn])
                            nc.scalar.activation(
                                a1[:, j, 0:gn], o1[:, 0:gn], AF.Relu,
                                bias=b1_t[:, j:j + 1])
                            nc.scalar.activation(
                                r2[:, 0:gn], o1[:, 0:gn], AF.Relu,
                                scale=-NEG_MLP, bias=b1_t[:, 2 + j:3 + j])
                            nc.vector.tensor_sub(
                                a1[:, j, 0:gn], a1[:, j, 0:gn], r2[:, 0:gn])
                        o2 = mps.tile([128, 512], f32, tag="o2")
                        for j in range(2):
                            nc.tensor.matmul(
                                o2[:, 0:gn], W2_t[:, j, :], a1[:, j, 0:gn],
                                start=(j == 0), stop=(j == 1))
                        a2 = msb.tile([128, 512], bf16, tag="a2")
                        r2b = msb.tile([128, 512], bf16, tag="r2b")
                        nc.scalar.activation(
                            a2[:, 0:gn], o2[:, 0:gn], AF.Relu,
                            bias=b2_t[:, 0:1])
                        nc.scalar.activation(
                            r2b[:, 0:gn], o2[:, 0:gn], AF.Relu,
                            scale=-NEG_MLP, bias=b2_t[:, 1:2])
                        nc.vector.tensor_sub(
                            a2[:, 0:gn], a2[:, 0:gn], r2b[:, 0:gn])
                        o3 = mps.tile([16, 512], f32, tag="sm", name="o3_t")
                        nc.tensor.matmul(o3[0:10, 0:gn], W3_t[:], a2[:, 0:gn])
                        z = msb.tile([16, 512], bf16, tag="z")
                        zr = msb.tile([16, 512], bf16, tag="zr")
                        nc.scalar.activation(
                            z[0:10, 0:gn], o3[0:10, 0:gn], AF.Relu,
                            bias=b3_t[0:10, 0:1])
                        nc.scalar.activation(
                            zr[0:10, 0:gn], o3[0:10, 0:gn], AF.Relu,
                            scale=-NEG_MLP, bias=b3_t[0:10, 1:2])
                        nc.vector.tensor_sub(
                            z[0:10, 0:gn], z[0:10, 0:gn], zr[0:10, 0:gn])
                        nc.scalar.activation(z[0:10, 0:gn], z[0:10, 0:gn],
                                             AF.Exp)
                        ssum = mps.tile([16, 512], f32, tag="sm",
                                        name="ssum_t")[0:1, :]
                        nc.tensor.matmul(
                            ssum[:, 0:gn], ones_t[0:10, 0:1], z[0:10, 0:gn])
                        sinv = msb.tile([1, 512], bf16, tag="sinv")
                        with nc.allow_low_precision(reason="softmax recip"):
                            nc.vector.reciprocal(sinv[:, 0:gn], ssum[:, 0:gn])
                        sx = mps.tile([16, 512], f32, tag="sm", name="sx_t")
                        nc.tensor.matmul(
                            sx[0:10, 0:gn], ones_t[0:1, 0:10], sinv[:, 0:gn])
                        res = msb.tile([16, 512], f32, tag="res")
                        nc.vector.tensor_mul(
                            res[0:10, 0:gn], z[0:10, 0:gn], sx[0:10, 0:gn])
                        nc.sync.dma_start(
                            outT[:, g0 * NT:g0 * NT + gn], res[0:10, 0:gn])

    nc.compile()
    return nc


def _inputs_per_core(inputs, src_w, ad_w, dst_col, meta):
    x = np.asarray(inputs["x"], dtype=np.float32)
    fb = np.asarray(inputs["fb"], dtype=np.float32)
    Wg = np.asarray(inputs["Wg"], dtype=np.float32)
    bias_g = np.asarray(inputs["bias_g"], dtype=np.float32)
    att_src = np.asarray(inputs["att_src"], dtype=np.float32)
    att_dst = np.asarray(inputs["att_dst"], dtype=np.float32)
    W1 = np.asarray(inputs["W1"], dtype=np.float32)
    b1 = np.asarray(inputs["b1"], dtype=np.float32)
    W2 = np.asarray(inputs["W2"], dtype=np.float32)
    b2 = np.asarray(inputs["b2"], dtype=np.float32)
    W3 = np.asarray(inputs["W3"], dtype=np.float32)
    b3 = np.asarray(inputs["b3"], dtype=np.float32)

    x_pad = np.zeros((NPAD, NFP), dtype=np.float32)
    x_pad[:N, :NF] = x
    fb_pad = np.zeros((NFP, NMEL), dtype=np.float32)
    fb_pad[:NF] = fb

    att_blk_s = np.zeros((HC, 4), dtype=np.float32)
    att_blk_d = np.zeros((HC, 4), dtype=np.float32)
    for h in range(H):
        att_blk_s[h * C:(h + 1) * C, h] = att_src[h]
        att_blk_d[h * C:(h + 1) * C, h] = att_dst[h]

    # head-interleaved feature order: new col j = old col (j%4)*32 + j//4
    perm_il = np.array([(j % 4) * 32 + j // 4 for j in range(HC)])
    Wg = np.ascontiguousarray(Wg[:, perm_il])
    att_blk_s = np.ascontiguousarray(att_blk_s[perm_il])
    att_blk_d = np.ascontiguousarray(att_blk_d[perm_il])
    bias_g = bias_g[perm_il]
    W1 = np.ascontiguousarray(W1[perm_il, :])

    b1p = np.zeros((128, 4), dtype=np.float32)
    b1p[:, 0] = b1[:128]
    b1p[:, 1] = b1[128:]
    b1p[:, 2:4] = -NEG_MLP * b1p[:, 0:2]
    b2p = np.zeros((128, 2), dtype=np.float32)
    b2p[:, 0] = b2
    b2p[:, 1] = -NEG_MLP * b2
    b3p = np.zeros((128, 2), dtype=np.float32)
    b3p[:10, 0] = b3
    b3p[:10, 1] = -NEG_MLP * b3

    common = {
        "fb_p": fb_pad.astype(BF16), "Wg": Wg,
        "attb_s": att_blk_s, "attb_d": att_blk_d,
        "bias_bc": np.tile(bias_g[None, :], (128, 1)).astype(BF16),
        "W1": W1.astype(BF16), "b1": b1p,
        "W2": W2.astype(BF16), "b2": b2p,
        "W3": W3.astype(BF16), "b3": b3p,
        "eye_f": np.eye(128, dtype=np.float32),
        "eye_b": np.eye(128).astype(BF16),
        "iota": np.tile(np.arange(128, dtype=np.float32)[None, :],
                        (128, 1)).astype(BF16),
        "ones": np.ones((128, 16)).astype(BF16),
        "flagz": np.zeros((1, 16)).astype(BF16),
    }
    xT_pad = np.ascontiguousarray(x_pad.T.astype(BF16))  # [640, NPAD]
    maps = []
    for k in range(NCORES):
        m = dict(common)
        m["xT_sl"] = np.ascontiguousarray(xT_pad[:, k * NPC:(k + 1) * NPC])
        m["idx_src"] = src_w[k]
        m["idx_ad"] = ad_w[k]
        m["dst_col"] = dst_col[k]
        maps.append(m)
    return maps


def kernel(**inputs):
    from concourse.bass_utils import run_bass_kernel_spmd

    src_w, ad_w, dst_col, meta = _prep(inputs["edge_index"])
    key = ("nc", meta["TOTC"], tuple(meta["cpt"].reshape(-1)),
           tuple(meta["woff"]))
    if key not in _CACHE:
        _CACHE.clear()
        _CACHE[key] = _build(meta)
    nc = _CACHE[key]
    maps = _inputs_per_core(inputs, src_w, ad_w, dst_col, meta)
    res = run_bass_kernel_spmd(nc, maps, core_ids=list(range(NCORES)))
    out = np.zeros((NPAD, 10), dtype=np.float32)
    for k in range(NCORES):
        out[k * NPC:(k + 1) * NPC] = res.results[k]["outT"].T
    return out[:N]
